# revision 1
# baseline (speedup 1.0000x reference)
"""Trainium2 Bass kernel v2 for nn_Encoder_59219009077683 (Swin-style block).

Math shortcut (from baseline): softmax row-sums are 1, so attention output = v
and the whole block is: window-gather -> V-proj -> fixed permutation (L_d
layout) -> out-proj -> LN1+skip -> MLP -> LN2+skip -> window-scatter.

v2 structural changes vs baseline:
  - gather/scatter via ONE SWDGE indirect DMA per tile (host-built index
    table; the cyclic roll/wrap is baked into the indices)
  - bf16 for all SBUF intermediates and matmuls (tolerance is 2e-2)
  - K=64 out-projection: L_d stored twice (quadrant q holds T[d, c+6q]) so
    the 12 K=32 chunk-matmuls become 6 K=64 matmuls per output block
  - host-folded biases: v-bias+out-bias -> ybias table added at y-evac;
    norm1_b+norm2_b added at x-transpose evac; mlp_b1 corrected by
    -W1^T norm2_b so the norm2 beta rides the x2 skip path
  - rsqrt via tensor_scalar(pow -0.5) on DVE: scalar engine runs only
    Gelu/Identity -> a single activation-table load
  - elementwise spread across DVE / Act / GpSimd
"""
import numpy as np
from contextlib import ExitStack

import concourse.bass as bass
import concourse.bacc as bacc
import concourse.tile as tile
from concourse import mybir
from concourse.bass_utils import run_bass_kernel_spmd, ml_dtypes

F32 = mybir.dt.float32
BF16 = mybir.dt.bfloat16
U32 = mybir.dt.uint32
AF = mybir.ActivationFunctionType
OP = mybir.AluOpType

B, HH, WW, C = 32, 56, 56, 384
NH, HD, WS, DISP, MLP = 12, 32, 7, 3, 1536
NWS = 8
N = 49
J = NH * N            # 588
NCORES = 8
IMGS = B // NCORES
WT = 8                # windows per tile (= one window row)
TW = WT * N           # 392
NTOK = HH * WW        # tokens per image
EPS = 1e-5
LDPAD = 16
LDW = LDPAD + J * WT + 8

BF = ml_dtypes.bfloat16


def _ap(t, offset, dims):
    tt = t.tensor if hasattr(t, "tensor") else t
    return bass.AP(tensor=tt, offset=offset, ap=[list(d) for d in dims])


def build():
    nc = bacc.Bacc("TRN2", target_bir_lowering=False, debug=False, num_devices=NCORES)
    x_d = nc.dram_tensor("x", [IMGS * NTOK, C], F32, kind="ExternalInput")
    wv_d = nc.dram_tensor("wv_t", [128, 3 * C], BF16, kind="ExternalInput")
    wo_d = nc.dram_tensor("wo_t", [64, 18 * 128], BF16, kind="ExternalInput")
    w1_d = nc.dram_tensor("w1_t", [128, 3 * MLP], BF16, kind="ExternalInput")
    w2_d = nc.dram_tensor("w2_t", [128, 12 * C], BF16, kind="ExternalInput")
    yb_d = nc.dram_tensor("yb_t", [128, 3 * TW], F32, kind="ExternalInput")
    cn_d = nc.dram_tensor("cn_t", [128, 27], F32, kind="ExternalInput")
    on_d = nc.dram_tensor("ones_t", [128, 128], BF16, kind="ExternalInput")
    id_d = nc.dram_tensor("identf_t", [128, 128], F32, kind="ExternalInput")
    idb_d = nc.dram_tensor("identb_t", [128, 128], BF16, kind="ExternalInput")
    pm_d = nc.dram_tensor("perm_t", [WW, WW], F32, kind="ExternalInput")
    out_d = nc.dram_tensor("out", [IMGS * NTOK, C], F32, kind="ExternalOutput")

    with tile.TileContext(nc) as tc, ExitStack() as ctx:
        wpool = ctx.enter_context(tc.tile_pool(name="w", bufs=1))
        stage_pool = ctx.enter_context(tc.tile_pool(name="stage", bufs=3))
        xt_pool = ctx.enter_context(tc.tile_pool(name="xt", bufs=2))
        vt_pool = ctx.enter_context(tc.tile_pool(name="vt", bufs=2))
        ld_pool = ctx.enter_context(tc.tile_pool(name="ld", bufs=2))
        y_pool = ctx.enter_context(tc.tile_pool(name="y", bufs=2))
        x2_pool = ctx.enter_context(tc.tile_pool(name="x2", bufs=3))
        h_pool = ctx.enter_context(tc.tile_pool(name="h", bufs=2))
        h2_pool = ctx.enter_context(tc.tile_pool(name="h2", bufs=2))
        oc_pool = ctx.enter_context(tc.tile_pool(name="oc", bufs=2))
        ot_pool = ctx.enter_context(tc.tile_pool(name="ot", bufs=2))
        sm_pool = ctx.enter_context(tc.tile_pool(name="sm", bufs=2))
        ps_t = ctx.enter_context(tc.tile_pool(name="pst", bufs=2, space="PSUM"))
        ps_a = ctx.enter_context(tc.tile_pool(name="psa", bufs=2, space="PSUM"))
        ps_m = ctx.enter_context(tc.tile_pool(name="psm", bufs=2, space="PSUM"))
        ps_s = ctx.enter_context(tc.tile_pool(name="pss", bufs=2, space="PSUM"))

        # ---------- one-time setup: weights straight from host layout ----------
        wv_b = wpool.tile([128, 3 * C], BF16)
        wo_b = wpool.tile([64, 18 * 128], BF16)
        w1_b = wpool.tile([128, 3 * MLP], BF16)
        w2_b = wpool.tile([128, 12 * C], BF16)
        yb_sb = wpool.tile([128, 3 * TW], F32)
        cn_sb = wpool.tile([128, 27], F32)
        ones_b = wpool.tile([128, 128], BF16)
        identf = wpool.tile([128, 128], F32)
        identb = wpool.tile([128, 128], BF16)
        perm56 = wpool.tile([WW, WW], F32)
        eps_t = wpool.tile([128, 1], F32)
        zero_t = wpool.tile([128, 1], F32)
        nc.vector.memset(eps_t[:, :], EPS)
        nc.vector.memset(zero_t[:, :], 0.0)
        nc.sync.dma_start(perm56[:, :], pm_d[:, :])
        nc.sync.dma_start(wv_b[:, :], wv_d[:, :])
        nc.sync.dma_start(wo_b[:, :], wo_d[:, :])
        nc.sync.dma_start(w1_b[:, :], w1_d[:, :])
        nc.sync.dma_start(w2_b[:, :], w2_d[:, :])
        nc.sync.dma_start(yb_sb[:, :], yb_d[:, :])
        nc.sync.dma_start(cn_sb[:, :], cn_d[:, :])
        nc.sync.dma_start(ones_b[:, :], on_d[:, :])
        nc.sync.dma_start(identf[:, :], id_d[:, :])
        nc.sync.dma_start(identb[:, :], idb_d[:, :])
        # PE must observe the identity via a transpose once before the loop so
        # later transposes carry <=1 sync wait (bare LDWEIGHTS limitation).
        dmy = ps_t.tile([128, 392], F32, tag="t")
        nc.tensor.transpose(dmy[0:128, 0:128], identf[:, :], identf[:, :])
        dmy2 = ps_t.tile([128, 392], BF16, tag="t")
        nc.tensor.transpose(dmy2[0:128, 0:128], identb[:, :], identb[:, :])

        # ---------- software-pipelined main loop ----------
        # tile index t = img*8 + wr; phases skewed across iterations so every
        # engine (PE especially) always has ready work.
        NT = IMGS * NWS
        st = {}   # per-tile live state

        def do_gather(t):
            img, wr = divmod(t, NWS)
            stage = stage_pool.tile([WW, WS * C], F32, tag="stage")
            pst = stage[:, :].ap[0][0]
            soff = stage[:, :].offset
            rows = [(WS * wr + DISP, 0, WS)] if wr < NWS - 1 else \
                [(52, 0, 4), (0, 4, 3)]
            for (r0, tr0, ntr) in rows:
                src = _ap(x_d, (img * NTOK + r0 * WW) * C,
                          [[C, WW], [WW * C, ntr], [1, C]])
                dst = _ap(stage, soff + tr0 * C,
                          [[pst, WW], [C, ntr], [1, C]])
                nc.sync.dma_start(dst, src)
            st[t] = {"stage": stage}

        def do_p1(t):
            """transposes + V-proj + ld build"""
            d = st[t]
            stage = d.pop("stage")
            xt = xt_pool.tile([128, 3 * TW], BF16, tag="xt")
            for k in range(3):
                xps = ps_t.tile([128, 392], F32, tag="t")
                pxp = xps[:, :].ap[0][0]
                xpo = xps[:, :].offset
                for tr in range(WS):
                    nc.tensor.transpose(
                        xps[:, WW * tr:WW * tr + WW],
                        stage[0:WW, tr * C + 128 * k:tr * C + 128 * k + 128],
                        perm56[0:WW, 0:WW])
                # token order everywhere downstream: col = 56*tr + 7*win + tc
                nc.scalar.activation(xt[:, TW * k:TW * k + TW], xps[:, :],
                                     AF.Identity, bias=cn_sb[:, 21 + k:22 + k],
                                     scale=1.0)
            vt = vt_pool.tile([128, 3 * TW], BF16, tag="vt")
            pvt = vt[:, :].ap[0][0]
            vtoff = vt[:, :].offset
            for kv in range(3):
                vps = ps_a.tile([128, TW], F32, tag="a")
                for k in range(3):
                    nc.tensor.matmul(vps[:, :],
                                     wv_b[:, C * k + 128 * kv:C * k + 128 * kv + 128],
                                     xt[:, TW * k:TW * k + TW],
                                     start=(k == 0), stop=(k == 2))
                pps = vps[:, :].ap[0][0]
                nc.scalar.activation(
                    _ap(vt, vtoff + N * kv,
                        [[pvt, 128], [WS, WS], [3 * N, WT], [1, WS]]),
                    _ap(vps, vps[:, :].offset,
                        [[pps, 128], [WW, WS], [WS, WT], [1, WS]]),
                    AF.Identity, bias=zero_t[:, :], scale=1.0)
            ld = ld_pool.tile([64, LDW], BF16, tag="ld")
            pld = ld[:, :].ap[0][0]
            ldoff = ld[:, :].offset
            for s in range(4):
                for q in range(2):
                    src = _ap(vt, vtoff + (32 * s) * pvt,
                              [[pvt, 32], [N, 24], [1, N]])
                    dst = _ap(ld, ldoff + (32 * q) * pld + LDPAD + N * s - 6 * q,
                              [[pld, 32], [4 * N, 24], [1, N]])
                    nc.sync.dma_start(dst, src)
            d["xt"] = xt
            d["ld"] = (ld, pld, ldoff)

        def do_p2a(t):
            """out-proj + ybias evac + ysq"""
            d = st[t]
            ld, pld, ldoff = d.pop("ld")
            y = y_pool.tile([128, 3 * TW], BF16, tag="y")
            for kj in range(3):
                yps = ps_a.tile([128, TW], F32, tag="a")
                for r in range(6):
                    rhs = _ap(ld, ldoff + LDPAD + r,
                              [[pld, 64], [84, WS], [J, WT], [12, WS]])
                    nc.tensor.matmul(yps[:, :],
                                     wo_b[:, (r * 3 + kj) * 128:(r * 3 + kj) * 128 + 128],
                                     rhs, start=(r == 0), stop=(r == 5))
                nc.vector.tensor_add(y[:, TW * kj:TW * kj + TW], yps[:, :],
                                     yb_sb[:, TW * kj:TW * kj + TW])
            ysq = sm_pool.tile([128, 3 * TW], BF16, tag="ysq")
            for k in range(3):
                nc.gpsimd.tensor_mul(ysq[:, TW * k:TW * k + TW],
                                     y[:, TW * k:TW * k + TW], y[:, TW * k:TW * k + TW])
            d["y"] = y
            d["ysq"] = ysq

        def _ln(y, ysq, gcol, skip, out):
            """matmul sums -> stats -> apply; out = (y-mu)*rst*g + skip."""
            s1 = ps_s.tile([128, TW], F32, tag="s")
            s2 = ps_s.tile([128, TW], F32, tag="s")
            for k in range(3):
                nc.tensor.matmul(s1[:, :], ones_b[:, :], y[:, TW * k:TW * k + TW],
                                 start=(k == 0), stop=(k == 2))
                nc.tensor.matmul(s2[:, :], ones_b[:, :], ysq[:, TW * k:TW * k + TW],
                                 start=(k == 0), stop=(k == 2))
            mu = sm_pool.tile([128, TW], F32, tag="mu")
            m2 = sm_pool.tile([128, TW], F32, tag="m2")
            veps = sm_pool.tile([128, TW], F32, tag="veps")
            rst = sm_pool.tile([128, TW], F32, tag="rst")
            nc.vector.tensor_copy(mu[:, :], s1[:, :])
            nc.gpsimd.tensor_mul(m2[:, :], mu[:, :], mu[:, :])
            nc.vector.tensor_sub(veps[:, :], s2[:, :], m2[:, :])
            # std = sqrt(var + eps) on Act (eps folded into the bias), then
            # 1/std via the fast custom-DVE reciprocal (~51 ULP, plenty here)
            nc.scalar.activation(veps[:, :], veps[:, :], AF.Sqrt,
                                 bias=eps_t[:, :], scale=1.0)
            nc.vector.reciprocal_approx_fast(rst[:, :], veps[:, :])
            for k in range(3):
                u = sm_pool.tile([128, TW], BF16, tag="u")
                v = sm_pool.tile([128, TW], BF16, tag="v")
                nc.vector.tensor_sub(u[:, :], y[:, TW * k:TW * k + TW], mu[:, :])
                nc.gpsimd.tensor_mul(v[:, :], u[:, :], rst[:, :])
                nc.vector.scalar_tensor_tensor(out[:, TW * k:TW * k + TW],
                                               v[:, :],
                                               cn_sb[:, gcol + k:gcol + k + 1],
                                               skip[:, TW * k:TW * k + TW],
                                               op0=OP.mult, op1=OP.add)

        def do_p2b(t):
            d = st[t]
            x2 = x2_pool.tile([128, 3 * TW], BF16, tag="x2")
            _ln(d.pop("y"), d.pop("ysq"), 15, d.pop("xt"), x2)
            d["x2"] = x2

        def do_p3(t):
            d = st[t]
            x2 = d["x2"]
            hsb = h_pool.tile([128, 12 * TW], BF16, tag="h")
            for m in range(12):
                hps = ps_m.tile([128, TW], F32, tag="m")
                for k in range(3):
                    nc.tensor.matmul(hps[:, :],
                                     w1_b[:, MLP * k + 128 * m:MLP * k + 128 * m + 128],
                                     x2[:, TW * k:TW * k + TW],
                                     start=(k == 0), stop=(k == 2))
                nc.scalar.activation(hsb[:, TW * m:TW * m + TW], hps[:, :],
                                     AF.Gelu, bias=cn_sb[:, m:m + 1], scale=1.0)
            d["hsb"] = hsb

        def do_p4a(t):
            d = st[t]
            hsb = d.pop("hsb")
            h2 = h2_pool.tile([128, 3 * TW], BF16, tag="h2")
            for kj in range(3):
                h2ps = ps_a.tile([128, TW], F32, tag="a")
                for k2 in range(12):
                    nc.tensor.matmul(h2ps[:, :],
                                     w2_b[:, C * k2 + 128 * kj:C * k2 + 128 * kj + 128],
                                     hsb[:, TW * k2:TW * k2 + TW],
                                     start=(k2 == 0), stop=(k2 == 11))
                nc.vector.tensor_scalar(h2[:, TW * kj:TW * kj + TW], h2ps[:, :],
                                        cn_sb[:, 12 + kj:13 + kj], None, op0=OP.add)
            hsq = sm_pool.tile([128, 3 * TW], BF16, tag="ysq")
            for k in range(3):
                nc.gpsimd.tensor_mul(hsq[:, TW * k:TW * k + TW],
                                     h2[:, TW * k:TW * k + TW], h2[:, TW * k:TW * k + TW])
            d["h2"] = h2
            d["hsq"] = hsq

        def do_p4b(t):
            d = st[t]
            ocm = oc_pool.tile([128, 3 * TW], BF16, tag="oc")
            _ln(d.pop("h2"), d.pop("hsq"), 18, d.pop("x2"), ocm)
            d["ocm"] = ocm

        def do_p5(t):
            d = st.pop(t)
            img, wr = divmod(t, NWS)
            ocm = d["ocm"]
            poc = ocm[:, :].ap[0][0]
            oco = ocm[:, :].offset
            otr = ot_pool.tile([WW, WS * C], BF16, tag="ot")
            pot = otr[:, :].ap[0][0]
            poff = otr[:, :].offset
            for tr in range(WS):
                ops_t = ps_t.tile([128, 392], BF16, tag="t")
                for k in range(3):
                    nc.tensor.transpose(
                        ops_t[0:WW, 128 * k:128 * k + 128],
                        ocm[:, TW * k + WW * tr:TW * k + WW * tr + WW],
                        identb[:, :])
                nc.vector.tensor_copy(otr[0:WW, tr * C:tr * C + C],
                                      ops_t[0:WW, 0:C])
            rows = [(WS * wr + DISP, 0, WS)] if wr < NWS - 1 else \
                [(52, 0, 4), (0, 4, 3)]
            for (r0, tr0, ntr) in rows:
                src = _ap(otr, poff + tr0 * C, [[pot, 53], [C, ntr], [1, C]])
                dst = _ap(out_d, (img * NTOK + r0 * WW + DISP) * C,
                          [[C, 53], [WW * C, ntr], [1, C]])
                nc.gpsimd.dma_start(dst, src)
                src = _ap(otr, poff + 53 * pot + tr0 * C,
                          [[pot, 3], [C, ntr], [1, C]])
                dst = _ap(out_d, (img * NTOK + r0 * WW) * C,
                          [[C, 3], [WW * C, ntr], [1, C]])
                nc.gpsimd.dma_start(dst, src)

        for i in range(NT + 5):
            if i < NT:
                do_gather(i)
            if 0 <= i - 2 < NT:
                do_p2a(i - 2)
            if 0 <= i - 1 < NT:
                do_p1(i - 1)
            if 0 <= i - 4 < NT:
                do_p4a(i - 4)
            if 0 <= i - 5 < NT:
                do_p5(i - 5)
            if 0 <= i - 2 < NT:
                do_p2b(i - 2)
            if 0 <= i - 3 < NT:
                do_p3(i - 3)
            if 0 <= i - 4 < NT:
                do_p4b(i - 4)
    nc.compile()
    return nc


def _to_bf(a):
    return np.ascontiguousarray(a.astype(BF))


def prep_inputs(inputs):
    """Host-side weight/bias reformatting (layout only + tiny bias algebra)."""
    f = {k: np.asarray(v, dtype=np.float32) for k, v in inputs.items()}
    qkv_w, qkv_b = f["qkv_w"], f["qkv_b"]
    out_w, out_b = f["out_w"], f["out_b"]
    w1, b1, w2, b2 = f["mlp_w1"], f["mlp_b1"], f["mlp_w2"], f["mlp_b2"]
    g1, be1 = f["norm1_g"], f["norm1_b"]
    g2, be2 = f["norm2_g"], f["norm2_b"]

    wv = qkv_w[:, 2 * C:3 * C]
    bv = qkv_b[2 * C:3 * C]

    wv_t = np.zeros((128, 3 * C), np.float32)
    for k in range(3):
        wv_t[:, C * k:C * k + C] = wv[128 * k:128 * k + 128, :]

    wo_t = np.zeros((64, 18 * 128), np.float32)
    for r in range(6):
        for q in range(2):
            for kj in range(3):
                wo_t[32 * q:32 * q + 32, (r * 3 + kj) * 128:(r * 3 + kj) * 128 + 128] = \
                    out_w[32 * (r + 6 * q):32 * (r + 6 * q) + 32, 128 * kj:128 * kj + 128]

    w1_t = np.zeros((128, 3 * MLP), np.float32)
    for k in range(3):
        w1_t[:, MLP * k:MLP * k + MLP] = w1[128 * k:128 * k + 128, :]
    w2_t = np.zeros((128, 12 * C), np.float32)
    for k2 in range(12):
        w2_t[:, C * k2:C * k2 + C] = w2[128 * k2:128 * k2 + 128, :]

    # ybias[n2, :] = P(bv)[n2] @ out_w + out_b
    n2 = np.arange(N)[:, None]
    co = np.arange(12)[None, :]
    hh = (12 * n2 + co) // N                    # (49, 12)
    pb = bv.reshape(12, 32)[hh]                 # (49, 12, 32)
    ybias = pb.reshape(N, C) @ out_w + out_b    # (49, 384)
    yb_t = np.zeros((128, 3 * TW), np.float32)
    tr_ = np.arange(WS)[:, None, None]
    wn_ = np.arange(WT)[None, :, None]
    tc_ = np.arange(WS)[None, None, :]
    pc = (WS * tr_ + tc_ + 0 * wn_).reshape(TW)   # n2 per permuted column
    for kj in range(3):
        blk = ybias[:, 128 * kj:128 * kj + 128].T          # (128, 49)
        yb_t[:, TW * kj:TW * kj + TW] = blk[:, pc]

    b1p = b1 - be2 @ w1                          # mlp bias corrected for +be2 on x2
    be12 = be1 + be2
    cn_t = np.zeros((128, 27), np.float32)
    for m in range(12):
        cn_t[:, m] = b1p[128 * m:128 * m + 128]
    for k in range(3):
        cn_t[:, 12 + k] = b2[128 * k:128 * k + 128]
        cn_t[:, 15 + k] = g1[128 * k:128 * k + 128]
        cn_t[:, 18 + k] = g2[128 * k:128 * k + 128]
        cn_t[:, 21 + k] = be12[128 * k:128 * k + 128]

    ones_t = np.full((128, 128), 1.0 / C, np.float32)
    ident = np.eye(128, dtype=np.float32)

    # column-roll permutation for the input transposes
    perm = np.zeros((WW, WW), np.float32)
    jj = np.arange(WW)
    perm[(jj + DISP) % WW, jj] = 1.0

    return {
        "wv_t": _to_bf(wv_t), "wo_t": _to_bf(wo_t),
        "w1_t": _to_bf(w1_t), "w2_t": _to_bf(w2_t),
        "yb_t": np.ascontiguousarray(yb_t), "cn_t": np.ascontiguousarray(cn_t),
        "ones_t": _to_bf(ones_t), "identf_t": np.ascontiguousarray(ident),
        "identb_t": _to_bf(ident),
        "perm_t": np.ascontiguousarray(perm),
    }


_CACHE = {}


def kernel(**inputs):
    if "nc" not in _CACHE:
        _CACHE["nc"] = build()
    nc = _CACHE["nc"]
    x = np.ascontiguousarray(np.asarray(inputs["x"], dtype=np.float32))
    base = prep_inputs(inputs)
    in_maps = []
    for c in range(NCORES):
        m = dict(base)
        m["x"] = np.ascontiguousarray(
            x[IMGS * c:IMGS * (c + 1)].reshape(IMGS * NTOK, C))
        in_maps.append(m)
    import os
    trace = bool(int(os.environ.get("KERNEL_TRACE", "0")))
    res = run_bass_kernel_spmd(nc, in_maps, core_ids=list(range(NCORES)),
                               trace=trace)
    _CACHE["last_res"] = res
    out = np.concatenate([r["out"].reshape(IMGS, HH, WW, C) for r in res.results],
                         axis=0)
    return out



# revision 22
# speedup vs baseline: 1.0032x; 1.0032x over previous
"""Trainium2 Bass kernel v2 for nn_Encoder_59219009077683 (Swin-style block).

Math shortcut (from baseline): softmax row-sums are 1, so attention output = v
and the whole block is: window-gather -> V-proj -> fixed permutation (L_d
layout) -> out-proj -> LN1+skip -> MLP -> LN2+skip -> window-scatter.

v2 structural changes vs baseline:
  - gather/scatter via ONE SWDGE indirect DMA per tile (host-built index
    table; the cyclic roll/wrap is baked into the indices)
  - bf16 for all SBUF intermediates and matmuls (tolerance is 2e-2)
  - K=64 out-projection: L_d stored twice (quadrant q holds T[d, c+6q]) so
    the 12 K=32 chunk-matmuls become 6 K=64 matmuls per output block
  - host-folded biases: v-bias+out-bias -> ybias table added at y-evac;
    norm1_b+norm2_b added at x-transpose evac; mlp_b1 corrected by
    -W1^T norm2_b so the norm2 beta rides the x2 skip path
  - rsqrt via tensor_scalar(pow -0.5) on DVE: scalar engine runs only
    Gelu/Identity -> a single activation-table load
  - elementwise spread across DVE / Act / GpSimd
"""
import numpy as np
from contextlib import ExitStack

import concourse.bass as bass
import concourse.bacc as bacc
import concourse.tile as tile
from concourse import mybir
from concourse.bass_utils import run_bass_kernel_spmd, ml_dtypes

F32 = mybir.dt.float32
BF16 = mybir.dt.bfloat16
U32 = mybir.dt.uint32
AF = mybir.ActivationFunctionType
OP = mybir.AluOpType

B, HH, WW, C = 32, 56, 56, 384
NH, HD, WS, DISP, MLP = 12, 32, 7, 3, 1536
NWS = 8
N = 49
J = NH * N            # 588
NCORES = 8
IMGS = B // NCORES
WT = 8                # windows per tile (= one window row)
TW = WT * N           # 392
NTOK = HH * WW        # tokens per image
EPS = 1e-5
LDPAD = 16
LDW = LDPAD + J * WT + 8

BF = ml_dtypes.bfloat16


def _ap(t, offset, dims):
    tt = t.tensor if hasattr(t, "tensor") else t
    return bass.AP(tensor=tt, offset=offset, ap=[list(d) for d in dims])


def build():
    nc = bacc.Bacc("TRN2", target_bir_lowering=False, debug=False, num_devices=NCORES)
    x_d = nc.dram_tensor("x", [IMGS * NTOK, C], BF16, kind="ExternalInput")
    wv_d = nc.dram_tensor("wv_t", [128, 3 * C], BF16, kind="ExternalInput")
    wo_d = nc.dram_tensor("wo_t", [64, 18 * 128], BF16, kind="ExternalInput")
    w1_d = nc.dram_tensor("w1_t", [128, 3 * MLP], BF16, kind="ExternalInput")
    w2_d = nc.dram_tensor("w2_t", [128, 12 * C], BF16, kind="ExternalInput")
    yb_d = nc.dram_tensor("yb_t", [128, 3 * TW], F32, kind="ExternalInput")
    cn_d = nc.dram_tensor("cn_t", [128, 27], F32, kind="ExternalInput")
    on_d = nc.dram_tensor("ones_t", [128, 128], BF16, kind="ExternalInput")
    idb_d = nc.dram_tensor("identb_t", [128, 128], BF16, kind="ExternalInput")
    pm_d = nc.dram_tensor("perm_t", [WW, WW], BF16, kind="ExternalInput")
    out_d = nc.dram_tensor("out", [IMGS * NTOK, C], F32, kind="ExternalOutput")

    with tile.TileContext(nc) as tc, ExitStack() as ctx:
        wpool = ctx.enter_context(tc.tile_pool(name="w", bufs=1))
        stage_pool = ctx.enter_context(tc.tile_pool(name="stage", bufs=3))
        xt_pool = ctx.enter_context(tc.tile_pool(name="xt", bufs=2))
        vt_pool = ctx.enter_context(tc.tile_pool(name="vt", bufs=2))
        ld_pool = ctx.enter_context(tc.tile_pool(name="ld", bufs=2))
        y_pool = ctx.enter_context(tc.tile_pool(name="y", bufs=2))
        x2_pool = ctx.enter_context(tc.tile_pool(name="x2", bufs=3))
        h_pool = ctx.enter_context(tc.tile_pool(name="h", bufs=2))
        h2_pool = ctx.enter_context(tc.tile_pool(name="h2", bufs=2))
        oc_pool = ctx.enter_context(tc.tile_pool(name="oc", bufs=2))
        ot_pool = ctx.enter_context(tc.tile_pool(name="ot", bufs=2))
        sm_pool = ctx.enter_context(tc.tile_pool(name="sm", bufs=2))
        ps_t = ctx.enter_context(tc.tile_pool(name="pst", bufs=2, space="PSUM"))
        ps_a = ctx.enter_context(tc.tile_pool(name="psa", bufs=2, space="PSUM"))
        ps_m = ctx.enter_context(tc.tile_pool(name="psm", bufs=2, space="PSUM"))
        ps_s = ctx.enter_context(tc.tile_pool(name="pss", bufs=2, space="PSUM"))

        # ---------- one-time setup: weights straight from host layout ----------
        wv_b = wpool.tile([128, 3 * C], BF16)
        wo_b = wpool.tile([64, 18 * 128], BF16)
        w1_b = wpool.tile([128, 3 * MLP], BF16)
        w2_b = wpool.tile([128, 12 * C], BF16)
        yb_sb = wpool.tile([128, 3 * TW], F32)
        cn_sb = wpool.tile([128, 27], F32)
        ones_b = wpool.tile([128, 128], BF16)
        identb = wpool.tile([128, 128], BF16)
        perm56 = wpool.tile([WW, WW], BF16)
        eps_t = wpool.tile([128, 1], F32)
        zero_t = wpool.tile([128, 1], F32)
        nc.vector.memset(eps_t[:, :], EPS)
        nc.vector.memset(zero_t[:, :], 0.0)
        nc.sync.dma_start(perm56[:, :], pm_d[:, :])
        nc.sync.dma_start(wv_b[:, :], wv_d[:, :])
        nc.sync.dma_start(wo_b[:, :], wo_d[:, :])
        nc.sync.dma_start(w1_b[:, :], w1_d[:, :])
        nc.sync.dma_start(w2_b[:, :], w2_d[:, :])
        nc.sync.dma_start(yb_sb[:, :], yb_d[:, :])
        nc.sync.dma_start(cn_sb[:, :], cn_d[:, :])
        nc.sync.dma_start(ones_b[:, :], on_d[:, :])
        nc.sync.dma_start(identb[:, :], idb_d[:, :])
        # PE must observe the identity via a transpose once before the loop so
        # later transposes carry <=1 sync wait (bare LDWEIGHTS limitation).
        dmy = ps_t.tile([128, 392], BF16, tag="t")
        nc.tensor.transpose(dmy[0:WW, 0:WW], perm56[0:WW, 0:WW], perm56[0:WW, 0:WW])
        dmy2 = ps_t.tile([128, 392], BF16, tag="t")
        nc.tensor.transpose(dmy2[0:128, 0:128], identb[:, :], identb[:, :])

        # ---------- software-pipelined main loop ----------
        # tile index t = img*8 + wr; phases skewed across iterations so every
        # engine (PE especially) always has ready work.
        NT = IMGS * NWS
        st = {}   # per-tile live state

        def do_gather(t):
            img, wr = divmod(t, NWS)
            stage = stage_pool.tile([WW, WS * C], BF16, tag="stage")
            pst = stage[:, :].ap[0][0]
            soff = stage[:, :].offset
            rows = [(WS * wr + DISP, 0, WS)] if wr < NWS - 1 else \
                [(52, 0, 4), (0, 4, 3)]
            for (r0, tr0, ntr) in rows:
                src = _ap(x_d, (img * NTOK + r0 * WW) * C,
                          [[C, WW], [WW * C, ntr], [1, C]])
                dst = _ap(stage, soff + tr0 * C,
                          [[pst, WW], [C, ntr], [1, C]])
                nc.sync.dma_start(dst, src)
            st[t] = {"stage": stage}

        def do_p1(t):
            """transposes + V-proj + ld build"""
            d = st[t]
            stage = d.pop("stage")
            xt = xt_pool.tile([128, 3 * TW], BF16, tag="xt")
            for k in range(3):
                xps = ps_t.tile([128, 392], BF16, tag="t")
                pxp = xps[:, :].ap[0][0]
                xpo = xps[:, :].offset
                for tr in range(WS):
                    nc.tensor.transpose(
                        xps[:, WW * tr:WW * tr + WW],
                        stage[0:WW, tr * C + 128 * k:tr * C + 128 * k + 128],
                        perm56[0:WW, 0:WW])
                # token order everywhere downstream: col = 56*tr + 7*win + tc
                nc.scalar.activation(xt[:, TW * k:TW * k + TW], xps[:, :],
                                     AF.Identity, bias=cn_sb[:, 21 + k:22 + k],
                                     scale=1.0)
            vt = vt_pool.tile([128, 3 * TW], BF16, tag="vt")
            pvt = vt[:, :].ap[0][0]
            vtoff = vt[:, :].offset
            for kv in range(3):
                vps = ps_a.tile([128, TW], F32, tag="a")
                for k in range(3):
                    nc.tensor.matmul(vps[:, :],
                                     wv_b[:, C * k + 128 * kv:C * k + 128 * kv + 128],
                                     xt[:, TW * k:TW * k + TW],
                                     start=(k == 0), stop=(k == 2))
                pps = vps[:, :].ap[0][0]
                nc.scalar.activation(
                    _ap(vt, vtoff + N * kv,
                        [[pvt, 128], [WS, WS], [3 * N, WT], [1, WS]]),
                    _ap(vps, vps[:, :].offset,
                        [[pps, 128], [WW, WS], [WS, WT], [1, WS]]),
                    AF.Identity, bias=zero_t[:, :], scale=1.0)
            ld = ld_pool.tile([64, LDW], BF16, tag="ld")
            pld = ld[:, :].ap[0][0]
            ldoff = ld[:, :].offset
            for s4 in range(4):
                for q in range(2):
                    src = _ap(vt, vtoff + (32 * s4) * pvt,
                              [[pvt, 32], [N, 24], [1, N]])
                    dst = _ap(ld, ldoff + (32 * q) * pld + LDPAD + N * s4 - 6 * q,
                              [[pld, 32], [4 * N, 24], [1, N]])
                    nc.sync.dma_start(dst, src)
            d["xt"] = xt
            d["ld"] = (ld, pld, ldoff)

        def do_p2a(t):
            """out-proj + ybias evac + ysq"""
            d = st[t]
            ld, pld, ldoff = d.pop("ld")
            y = y_pool.tile([128, 3 * TW], BF16, tag="y")
            for kj in range(3):
                yps = ps_a.tile([128, TW], F32, tag="a")
                for r in range(6):
                    rhs = _ap(ld, ldoff + LDPAD + r,
                              [[pld, 64], [84, WS], [J, WT], [12, WS]])
                    nc.tensor.matmul(yps[:, :],
                                     wo_b[:, (r * 3 + kj) * 128:(r * 3 + kj) * 128 + 128],
                                     rhs, start=(r == 0), stop=(r == 5))
                nc.vector.tensor_add(y[:, TW * kj:TW * kj + TW], yps[:, :],
                                     yb_sb[:, TW * kj:TW * kj + TW])
            ysq = sm_pool.tile([128, 3 * TW], BF16, tag="ysq")
            for k in range(3):
                nc.gpsimd.tensor_mul(ysq[:, TW * k:TW * k + TW],
                                     y[:, TW * k:TW * k + TW], y[:, TW * k:TW * k + TW])
            d["y"] = y
            d["ysq"] = ysq

        def _ln(y, ysq, gcol, skip, out):
            """matmul sums -> stats -> apply; out = (y-mu)*rst*g + skip."""
            s1 = ps_s.tile([128, TW], F32, tag="s")
            s2 = ps_s.tile([128, TW], F32, tag="s")
            for k in range(3):
                nc.tensor.matmul(s1[:, :], ones_b[:, :], y[:, TW * k:TW * k + TW],
                                 start=(k == 0), stop=(k == 2))
                nc.tensor.matmul(s2[:, :], ones_b[:, :], ysq[:, TW * k:TW * k + TW],
                                 start=(k == 0), stop=(k == 2))
            mu = sm_pool.tile([128, TW], F32, tag="mu")
            m2 = sm_pool.tile([128, TW], F32, tag="m2")
            veps = sm_pool.tile([128, TW], F32, tag="veps")
            rst = sm_pool.tile([128, TW], F32, tag="rst")
            nc.vector.tensor_copy(mu[:, :], s1[:, :])
            nc.gpsimd.tensor_mul(m2[:, :], mu[:, :], mu[:, :])
            nc.vector.tensor_sub(veps[:, :], s2[:, :], m2[:, :])
            # std = sqrt(var + eps) on Act (eps folded into the bias), then
            # 1/std via the fast custom-DVE reciprocal (~51 ULP, plenty here)
            nc.scalar.activation(veps[:, :], veps[:, :], AF.Sqrt,
                                 bias=eps_t[:, :], scale=1.0)
            nc.vector.reciprocal_approx_fast(rst[:, :], veps[:, :])
            for k in range(3):
                u = sm_pool.tile([128, TW], BF16, tag="u")
                v = sm_pool.tile([128, TW], BF16, tag="v")
                nc.vector.tensor_sub(u[:, :], y[:, TW * k:TW * k + TW], mu[:, :])
                nc.gpsimd.tensor_mul(v[:, :], u[:, :], rst[:, :])
                nc.vector.scalar_tensor_tensor(out[:, TW * k:TW * k + TW],
                                               v[:, :],
                                               cn_sb[:, gcol + k:gcol + k + 1],
                                               skip[:, TW * k:TW * k + TW],
                                               op0=OP.mult, op1=OP.add)

        def do_p2b(t):
            d = st[t]
            x2 = x2_pool.tile([128, 3 * TW], BF16, tag="x2")
            _ln(d.pop("y"), d.pop("ysq"), 15, d.pop("xt"), x2)
            d["x2"] = x2

        def do_p3(t):
            d = st[t]
            x2 = d["x2"]
            hsb = h_pool.tile([128, 12 * TW], BF16, tag="h")
            for m in range(12):
                hps = ps_m.tile([128, TW], F32, tag="m")
                for k in range(3):
                    nc.tensor.matmul(hps[:, :],
                                     w1_b[:, MLP * k + 128 * m:MLP * k + 128 * m + 128],
                                     x2[:, TW * k:TW * k + TW],
                                     start=(k == 0), stop=(k == 2))
                nc.scalar.activation(hsb[:, TW * m:TW * m + TW], hps[:, :],
                                     AF.Gelu, bias=cn_sb[:, m:m + 1], scale=1.0)
            d["hsb"] = hsb

        def do_p4a(t):
            d = st[t]
            hsb = d.pop("hsb")
            h2 = h2_pool.tile([128, 3 * TW], BF16, tag="h2")
            for kj in range(3):
                h2ps = ps_a.tile([128, TW], F32, tag="a")
                for k2 in range(12):
                    nc.tensor.matmul(h2ps[:, :],
                                     w2_b[:, C * k2 + 128 * kj:C * k2 + 128 * kj + 128],
                                     hsb[:, TW * k2:TW * k2 + TW],
                                     start=(k2 == 0), stop=(k2 == 11))
                nc.vector.tensor_scalar(h2[:, TW * kj:TW * kj + TW], h2ps[:, :],
                                        cn_sb[:, 12 + kj:13 + kj], None, op0=OP.add)
            hsq = sm_pool.tile([128, 3 * TW], BF16, tag="ysq")
            for k in range(3):
                nc.gpsimd.tensor_mul(hsq[:, TW * k:TW * k + TW],
                                     h2[:, TW * k:TW * k + TW], h2[:, TW * k:TW * k + TW])
            d["h2"] = h2
            d["hsq"] = hsq

        def do_p4b(t):
            d = st[t]
            ocm = oc_pool.tile([128, 3 * TW], BF16, tag="oc")
            _ln(d.pop("h2"), d.pop("hsq"), 18, d.pop("x2"), ocm)
            d["ocm"] = ocm

        def do_p5(t):
            d = st.pop(t)
            img, wr = divmod(t, NWS)
            ocm = d["ocm"]
            poc = ocm[:, :].ap[0][0]
            oco = ocm[:, :].offset
            otr = ot_pool.tile([WW, WS * C], BF16, tag="ot")
            pot = otr[:, :].ap[0][0]
            poff = otr[:, :].offset
            for tr in range(WS):
                ops_t = ps_t.tile([128, 392], BF16, tag="t")
                for k in range(3):
                    nc.tensor.transpose(
                        ops_t[0:WW, 128 * k:128 * k + 128],
                        ocm[:, TW * k + WW * tr:TW * k + WW * tr + WW],
                        identb[:, :])
                nc.vector.tensor_copy(otr[0:WW, tr * C:tr * C + C],
                                      ops_t[0:WW, 0:C])
            rows = [(WS * wr + DISP, 0, WS)] if wr < NWS - 1 else \
                [(52, 0, 4), (0, 4, 3)]
            for (r0, tr0, ntr) in rows:
                src = _ap(otr, poff + tr0 * C, [[pot, 53], [C, ntr], [1, C]])
                dst = _ap(out_d, (img * NTOK + r0 * WW + DISP) * C,
                          [[C, 53], [WW * C, ntr], [1, C]])
                nc.gpsimd.dma_start(dst, src)
                src = _ap(otr, poff + 53 * pot + tr0 * C,
                          [[pot, 3], [C, ntr], [1, C]])
                dst = _ap(out_d, (img * NTOK + r0 * WW) * C,
                          [[C, 3], [WW * C, ntr], [1, C]])
                nc.gpsimd.dma_start(dst, src)

        for i in range(NT + 5):
            if i < NT:
                do_gather(i)
            if 0 <= i - 2 < NT:
                do_p2a(i - 2)
            if 0 <= i - 1 < NT:
                do_p1(i - 1)
            if 0 <= i - 4 < NT:
                do_p4a(i - 4)
            if 0 <= i - 5 < NT:
                do_p5(i - 5)
            if 0 <= i - 2 < NT:
                do_p2b(i - 2)
            if 0 <= i - 3 < NT:
                do_p3(i - 3)
            if 0 <= i - 4 < NT:
                do_p4b(i - 4)
    nc.compile()
    return nc


def _to_bf(a):
    return np.ascontiguousarray(a.astype(BF))


def prep_inputs(inputs):
    """Host-side weight/bias reformatting (layout only + tiny bias algebra)."""
    f = {k: np.asarray(v, dtype=np.float32) for k, v in inputs.items()}
    qkv_w, qkv_b = f["qkv_w"], f["qkv_b"]
    out_w, out_b = f["out_w"], f["out_b"]
    w1, b1, w2, b2 = f["mlp_w1"], f["mlp_b1"], f["mlp_w2"], f["mlp_b2"]
    g1, be1 = f["norm1_g"], f["norm1_b"]
    g2, be2 = f["norm2_g"], f["norm2_b"]

    wv = qkv_w[:, 2 * C:3 * C]
    bv = qkv_b[2 * C:3 * C]

    wv_t = np.zeros((128, 3 * C), np.float32)
    for k in range(3):
        wv_t[:, C * k:C * k + C] = wv[128 * k:128 * k + 128, :]

    wo_t = np.zeros((64, 18 * 128), np.float32)
    for r in range(6):
        for q in range(2):
            for kj in range(3):
                wo_t[32 * q:32 * q + 32, (r * 3 + kj) * 128:(r * 3 + kj) * 128 + 128] = \
                    out_w[32 * (r + 6 * q):32 * (r + 6 * q) + 32, 128 * kj:128 * kj + 128]

    w1_t = np.zeros((128, 3 * MLP), np.float32)
    for k in range(3):
        w1_t[:, MLP * k:MLP * k + MLP] = w1[128 * k:128 * k + 128, :]
    w2_t = np.zeros((128, 12 * C), np.float32)
    for k2 in range(12):
        w2_t[:, C * k2:C * k2 + C] = w2[128 * k2:128 * k2 + 128, :]

    # ybias[n2, :] = P(bv)[n2] @ out_w + out_b
    n2 = np.arange(N)[:, None]
    co = np.arange(12)[None, :]
    hh = (12 * n2 + co) // N                    # (49, 12)
    pb = bv.reshape(12, 32)[hh]                 # (49, 12, 32)
    ybias = pb.reshape(N, C) @ out_w + out_b    # (49, 384)
    yb_t = np.zeros((128, 3 * TW), np.float32)
    tr_ = np.arange(WS)[:, None, None]
    wn_ = np.arange(WT)[None, :, None]
    tc_ = np.arange(WS)[None, None, :]
    pc = (WS * tr_ + tc_ + 0 * wn_).reshape(TW)   # n2 per permuted column
    for kj in range(3):
        blk = ybias[:, 128 * kj:128 * kj + 128].T          # (128, 49)
        yb_t[:, TW * kj:TW * kj + TW] = blk[:, pc]

    b1p = b1 - be2 @ w1                          # mlp bias corrected for +be2 on x2
    be12 = be1 + be2
    cn_t = np.zeros((128, 27), np.float32)
    for m in range(12):
        cn_t[:, m] = b1p[128 * m:128 * m + 128]
    for k in range(3):
        cn_t[:, 12 + k] = b2[128 * k:128 * k + 128]
        cn_t[:, 15 + k] = g1[128 * k:128 * k + 128]
        cn_t[:, 18 + k] = g2[128 * k:128 * k + 128]
        cn_t[:, 21 + k] = be12[128 * k:128 * k + 128]

    ones_t = np.full((128, 128), 1.0 / C, np.float32)
    ident = np.eye(128, dtype=np.float32)

    # column-roll permutation for the input transposes
    perm = np.zeros((WW, WW), np.float32)
    jj = np.arange(WW)
    perm[(jj + DISP) % WW, jj] = 1.0

    return {
        "wv_t": _to_bf(wv_t), "wo_t": _to_bf(wo_t),
        "w1_t": _to_bf(w1_t), "w2_t": _to_bf(w2_t),
        "yb_t": np.ascontiguousarray(yb_t), "cn_t": np.ascontiguousarray(cn_t),
        "ones_t": _to_bf(ones_t), "identb_t": _to_bf(ident),
        "perm_t": _to_bf(perm),
    }


_CACHE = {}


def kernel(**inputs):
    if "nc" not in _CACHE:
        _CACHE["nc"] = build()
    nc = _CACHE["nc"]
    x = np.asarray(inputs["x"], dtype=np.float32).astype(BF)
    base = prep_inputs(inputs)
    in_maps = []
    for c in range(NCORES):
        m = dict(base)
        m["x"] = np.ascontiguousarray(
            x[IMGS * c:IMGS * (c + 1)].reshape(IMGS * NTOK, C))
        in_maps.append(m)
    import os
    trace = bool(int(os.environ.get("KERNEL_TRACE", "0")))
    res = run_bass_kernel_spmd(nc, in_maps, core_ids=list(range(NCORES)),
                               trace=trace)
    _CACHE["last_res"] = res
    out = np.concatenate([r["out"].reshape(IMGS, HH, WW, C) for r in res.results],
                         axis=0)
    return out



# revision 24
# speedup vs baseline: 1.1340x; 1.1304x over previous
"""Trainium2 Bass kernel v2 for nn_Encoder_59219009077683 (Swin-style block).

Math shortcut (from baseline): softmax row-sums are 1, so attention output = v
and the whole block is: window-gather -> V-proj -> fixed permutation (L_d
layout) -> out-proj -> LN1+skip -> MLP -> LN2+skip -> window-scatter.

v2 structural changes vs baseline:
  - gather/scatter via ONE SWDGE indirect DMA per tile (host-built index
    table; the cyclic roll/wrap is baked into the indices)
  - bf16 for all SBUF intermediates and matmuls (tolerance is 2e-2)
  - K=64 out-projection: L_d stored twice (quadrant q holds T[d, c+6q]) so
    the 12 K=32 chunk-matmuls become 6 K=64 matmuls per output block
  - host-folded biases: v-bias+out-bias -> ybias table added at y-evac;
    norm1_b+norm2_b added at x-transpose evac; mlp_b1 corrected by
    -W1^T norm2_b so the norm2 beta rides the x2 skip path
  - rsqrt via tensor_scalar(pow -0.5) on DVE: scalar engine runs only
    Gelu/Identity -> a single activation-table load
  - elementwise spread across DVE / Act / GpSimd
"""
import numpy as np
from contextlib import ExitStack

import concourse.bass as bass
import concourse.bacc as bacc
import concourse.tile as tile
from concourse import mybir
from concourse.bass_utils import run_bass_kernel_spmd, ml_dtypes

F32 = mybir.dt.float32
BF16 = mybir.dt.bfloat16
U32 = mybir.dt.uint32
AF = mybir.ActivationFunctionType
OP = mybir.AluOpType

B, HH, WW, C = 32, 56, 56, 384
NH, HD, WS, DISP, MLP = 12, 32, 7, 3, 1536
NWS = 8
N = 49
J = NH * N            # 588
NCORES = 8
IMGS = B // NCORES
WT = 8                # windows per tile (= one window row)
TW = WT * N           # 392
NTOK = HH * WW        # tokens per image
EPS = 1e-5
LDPAD = 16
LDW = LDPAD + J * WT + 8

BF = ml_dtypes.bfloat16


def _ap(t, offset, dims):
    tt = t.tensor if hasattr(t, "tensor") else t
    return bass.AP(tensor=tt, offset=offset, ap=[list(d) for d in dims])


def build():
    nc = bacc.Bacc("TRN2", target_bir_lowering=False, debug=False, num_devices=NCORES)
    x_d = nc.dram_tensor("x", [IMGS * NTOK, C], BF16, kind="ExternalInput")
    wv_d = nc.dram_tensor("wv_t", [128, 3 * C], BF16, kind="ExternalInput")
    wo_d = nc.dram_tensor("wo_t", [64, 18 * 128], BF16, kind="ExternalInput")
    w1_d = nc.dram_tensor("w1_t", [128, 3 * MLP], BF16, kind="ExternalInput")
    w2_d = nc.dram_tensor("w2_t", [128, 12 * C], BF16, kind="ExternalInput")
    yb_d = nc.dram_tensor("yb_t", [128, 3 * TW], F32, kind="ExternalInput")
    cn_d = nc.dram_tensor("cn_t", [128, 27], F32, kind="ExternalInput")
    on_d = nc.dram_tensor("ones_t", [128, 128], BF16, kind="ExternalInput")
    idb_d = nc.dram_tensor("identb_t", [128, 128], BF16, kind="ExternalInput")
    pm_d = nc.dram_tensor("perm_t", [WW, WW], BF16, kind="ExternalInput")
    out_d = nc.dram_tensor("out", [IMGS * NTOK, C], F32, kind="ExternalOutput")

    with tile.TileContext(nc) as tc, ExitStack() as ctx:
        wpool = ctx.enter_context(tc.tile_pool(name="w", bufs=1))
        stage_pool = ctx.enter_context(tc.tile_pool(name="stage", bufs=3))
        xt_pool = ctx.enter_context(tc.tile_pool(name="xt", bufs=4))
        vt_pool = ctx.enter_context(tc.tile_pool(name="vt", bufs=3))
        ld_pool = ctx.enter_context(tc.tile_pool(name="ld", bufs=2))
        y_pool = ctx.enter_context(tc.tile_pool(name="y", bufs=2))
        x2_pool = ctx.enter_context(tc.tile_pool(name="x2", bufs=3))
        h_pool = ctx.enter_context(tc.tile_pool(name="h", bufs=3))
        h2_pool = ctx.enter_context(tc.tile_pool(name="h2", bufs=2))
        oc_pool = ctx.enter_context(tc.tile_pool(name="oc", bufs=2))
        ot_pool = ctx.enter_context(tc.tile_pool(name="ot", bufs=2))
        sm_pool = ctx.enter_context(tc.tile_pool(name="sm", bufs=3))
        ps_t = ctx.enter_context(tc.tile_pool(name="pst", bufs=1, space="PSUM"))
        ps_a = ctx.enter_context(tc.tile_pool(name="psa", bufs=2, space="PSUM"))
        ps_m = ctx.enter_context(tc.tile_pool(name="psm", bufs=2, space="PSUM"))
        ps_s = ctx.enter_context(tc.tile_pool(name="pss", bufs=1, space="PSUM"))
        ps_o = ctx.enter_context(tc.tile_pool(name="pso", bufs=2, space="PSUM"))

        # ---------- one-time setup: weights straight from host layout ----------
        wv_b = wpool.tile([128, 3 * C], BF16)
        wo_b = wpool.tile([64, 18 * 128], BF16)
        w1_b = wpool.tile([128, 3 * MLP], BF16)
        w2_b = wpool.tile([128, 12 * C], BF16)
        yb_sb = wpool.tile([128, 3 * TW], F32)
        cn_sb = wpool.tile([128, 27], F32)
        ones_b = wpool.tile([128, 128], BF16)
        identb = wpool.tile([128, 128], BF16)
        perm56 = wpool.tile([WW, WW], BF16)
        eps_t = wpool.tile([128, 1], F32)
        zero_t = wpool.tile([128, 1], F32)
        nc.vector.memset(eps_t[:, :], EPS)
        nc.vector.memset(zero_t[:, :], 0.0)
        nc.sync.dma_start(perm56[:, :], pm_d[:, :])
        nc.sync.dma_start(wv_b[:, :], wv_d[:, :])
        nc.sync.dma_start(wo_b[:, :], wo_d[:, :])
        nc.sync.dma_start(w1_b[:, :], w1_d[:, :])
        nc.sync.dma_start(w2_b[:, :], w2_d[:, :])
        nc.sync.dma_start(yb_sb[:, :], yb_d[:, :])
        nc.sync.dma_start(cn_sb[:, :], cn_d[:, :])
        nc.sync.dma_start(ones_b[:, :], on_d[:, :])
        nc.sync.dma_start(identb[:, :], idb_d[:, :])
        # PE must observe the identity via a transpose once before the loop so
        # later transposes carry <=1 sync wait (bare LDWEIGHTS limitation).
        dmy = ps_t.tile([128, 392], BF16, tag="t")
        nc.tensor.transpose(dmy[0:WW, 0:WW], perm56[0:WW, 0:WW], perm56[0:WW, 0:WW])
        dmy2 = ps_t.tile([128, 392], BF16, tag="t")
        nc.tensor.transpose(dmy2[0:128, 0:128], identb[:, :], identb[:, :])

        # ---------- software-pipelined main loop ----------
        # tile index t = img*8 + wr; phases skewed across iterations so every
        # engine (PE especially) always has ready work.
        NT = IMGS * NWS
        st = {}   # per-tile live state

        def do_gather(t):
            img, wr = divmod(t, NWS)
            stage = stage_pool.tile([WW, WS * C], BF16, tag="stage")
            pst = stage[:, :].ap[0][0]
            soff = stage[:, :].offset
            rows = [(WS * wr + DISP, 0, WS)] if wr < NWS - 1 else \
                [(52, 0, 4), (0, 4, 3)]
            for (r0, tr0, ntr) in rows:
                src = _ap(x_d, (img * NTOK + r0 * WW) * C,
                          [[C, WW], [WW * C, ntr], [1, C]])
                dst = _ap(stage, soff + tr0 * C,
                          [[pst, WW], [C, ntr], [1, C]])
                nc.sync.dma_start(dst, src)
            st[t] = {"stage": stage}

        def do_p1(t):
            """transposes + V-proj + ld build"""
            d = st[t]
            stage = d.pop("stage")
            xt = xt_pool.tile([128, 3 * TW], BF16, tag="xt")
            for k in range(3):
                xps = ps_t.tile([128, 392], BF16, tag="t")
                pxp = xps[:, :].ap[0][0]
                xpo = xps[:, :].offset
                for tr in range(WS):
                    nc.tensor.transpose(
                        xps[:, WW * tr:WW * tr + WW],
                        stage[0:WW, tr * C + 128 * k:tr * C + 128 * k + 128],
                        perm56[0:WW, 0:WW])
                # token order everywhere downstream: col = 56*tr + 7*win + tc
                nc.vector.tensor_scalar(xt[:, TW * k:TW * k + TW], xps[:, :],
                                        cn_sb[:, 21 + k:22 + k], None, op0=OP.add)
            vt = vt_pool.tile([128, 3 * TW], BF16, tag="vt")
            pvt = vt[:, :].ap[0][0]
            vtoff = vt[:, :].offset
            for kv in range(3):
                vps = ps_a.tile([128, TW], F32, tag="a")
                for k in range(3):
                    nc.tensor.matmul(vps[:, :],
                                     wv_b[:, C * k + 128 * kv:C * k + 128 * kv + 128],
                                     xt[:, TW * k:TW * k + TW],
                                     start=(k == 0), stop=(k == 2))
                pps = vps[:, :].ap[0][0]
                nc.scalar.activation(
                    _ap(vt, vtoff + N * kv,
                        [[pvt, 128], [WS, WS], [3 * N, WT], [1, WS]]),
                    _ap(vps, vps[:, :].offset,
                        [[pps, 128], [WW, WS], [WS, WT], [1, WS]]),
                    AF.Identity, bias=zero_t[:, :], scale=1.0)
            ld = ld_pool.tile([64, LDW], BF16, tag="ld")
            pld = ld[:, :].ap[0][0]
            ldoff = ld[:, :].offset
            for s4 in range(4):
                for q in range(2):
                    src = _ap(vt, vtoff + (32 * s4) * pvt,
                              [[pvt, 32], [N, 24], [1, N]])
                    dst = _ap(ld, ldoff + (32 * q) * pld + LDPAD + N * s4 - 6 * q,
                              [[pld, 32], [4 * N, 24], [1, N]])
                    nc.sync.dma_start(dst, src)
            d["xt"] = xt
            d["ld"] = (ld, pld, ldoff)

        def do_p2a(t):
            """out-proj + ybias evac + ysq"""
            d = st[t]
            ld, pld, ldoff = d.pop("ld")
            y = y_pool.tile([128, 3 * TW], BF16, tag="y")
            for kj in range(3):
                yps = ps_o.tile([128, TW], F32, tag="o")
                for r in range(6):
                    rhs = _ap(ld, ldoff + LDPAD + r,
                              [[pld, 64], [84, WS], [J, WT], [12, WS]])
                    nc.tensor.matmul(yps[:, :],
                                     wo_b[:, (r * 3 + kj) * 128:(r * 3 + kj) * 128 + 128],
                                     rhs, start=(r == 0), stop=(r == 5))
                nc.vector.tensor_add(y[:, TW * kj:TW * kj + TW], yps[:, :],
                                     yb_sb[:, TW * kj:TW * kj + TW])
            ysq = sm_pool.tile([128, 3 * TW], BF16, tag="ysq")
            for k in range(3):
                nc.gpsimd.tensor_mul(ysq[:, TW * k:TW * k + TW],
                                     y[:, TW * k:TW * k + TW], y[:, TW * k:TW * k + TW])
            d["y"] = y
            d["ysq"] = ysq

        def _ln(y, ysq, gcol, skip, out):
            """matmul sums -> stats -> apply; out = (y-mu)*rst*g + skip."""
            s1 = ps_s.tile([128, TW], F32, tag="s")
            s2 = ps_s.tile([128, TW], F32, tag="s")
            for k in range(3):
                nc.tensor.matmul(s1[:, :], ones_b[:, :], y[:, TW * k:TW * k + TW],
                                 start=(k == 0), stop=(k == 2))
                nc.tensor.matmul(s2[:, :], ones_b[:, :], ysq[:, TW * k:TW * k + TW],
                                 start=(k == 0), stop=(k == 2))
            mu = sm_pool.tile([128, TW], F32, tag="mu")
            m2 = sm_pool.tile([128, TW], F32, tag="m2")
            veps = sm_pool.tile([128, TW], F32, tag="veps")
            rst = sm_pool.tile([128, TW], F32, tag="rst")
            nc.vector.tensor_copy(mu[:, :], s1[:, :])
            nc.gpsimd.tensor_mul(m2[:, :], mu[:, :], mu[:, :])
            nc.vector.tensor_sub(veps[:, :], s2[:, :], m2[:, :])
            # std = sqrt(var + eps) on Act (eps folded into the bias), then
            # 1/std via the fast custom-DVE reciprocal (~51 ULP, plenty here)
            nc.scalar.activation(veps[:, :], veps[:, :], AF.Sqrt,
                                 bias=eps_t[:, :], scale=1.0)
            nc.vector.reciprocal_approx_fast(rst[:, :], veps[:, :])
            for k in range(3):
                u = sm_pool.tile([128, TW], BF16, tag="u")
                v = sm_pool.tile([128, TW], BF16, tag="v")
                nc.vector.tensor_sub(u[:, :], y[:, TW * k:TW * k + TW], mu[:, :])
                nc.gpsimd.tensor_mul(v[:, :], u[:, :], rst[:, :])
                nc.vector.scalar_tensor_tensor(out[:, TW * k:TW * k + TW],
                                               v[:, :],
                                               cn_sb[:, gcol + k:gcol + k + 1],
                                               skip[:, TW * k:TW * k + TW],
                                               op0=OP.mult, op1=OP.add)

        def do_p2b(t):
            d = st[t]
            x2 = x2_pool.tile([128, 3 * TW], BF16, tag="x2")
            _ln(d.pop("y"), d.pop("ysq"), 15, d.pop("xt"), x2)
            d["x2"] = x2

        def do_p3(t):
            d = st[t]
            x2 = d["x2"]
            hsb = h_pool.tile([128, 12 * TW], BF16, tag="h")
            for m in range(12):
                hps = ps_m.tile([128, TW], F32, tag="m")
                for k in range(3):
                    nc.tensor.matmul(hps[:, :],
                                     w1_b[:, MLP * k + 128 * m:MLP * k + 128 * m + 128],
                                     x2[:, TW * k:TW * k + TW],
                                     start=(k == 0), stop=(k == 2))
                nc.scalar.activation(hsb[:, TW * m:TW * m + TW], hps[:, :],
                                     AF.Gelu, bias=cn_sb[:, m:m + 1], scale=1.0)
            d["hsb"] = hsb

        def do_p4a(t):
            d = st[t]
            hsb = d.pop("hsb")
            h2 = h2_pool.tile([128, 3 * TW], BF16, tag="h2")
            for kj in range(3):
                h2ps = ps_a.tile([128, TW], F32, tag="a")
                for k2 in range(12):
                    nc.tensor.matmul(h2ps[:, :],
                                     w2_b[:, C * k2 + 128 * kj:C * k2 + 128 * kj + 128],
                                     hsb[:, TW * k2:TW * k2 + TW],
                                     start=(k2 == 0), stop=(k2 == 11))
                nc.vector.tensor_scalar(h2[:, TW * kj:TW * kj + TW], h2ps[:, :],
                                        cn_sb[:, 12 + kj:13 + kj], None, op0=OP.add)
            hsq = sm_pool.tile([128, 3 * TW], BF16, tag="ysq")
            for k in range(3):
                nc.gpsimd.tensor_mul(hsq[:, TW * k:TW * k + TW],
                                     h2[:, TW * k:TW * k + TW], h2[:, TW * k:TW * k + TW])
            d["h2"] = h2
            d["hsq"] = hsq

        def do_p4b(t):
            d = st[t]
            ocm = oc_pool.tile([128, 3 * TW], BF16, tag="oc")
            _ln(d.pop("h2"), d.pop("hsq"), 18, d.pop("x2"), ocm)
            d["ocm"] = ocm

        def do_p5(t):
            d = st.pop(t)
            img, wr = divmod(t, NWS)
            ocm = d["ocm"]
            poc = ocm[:, :].ap[0][0]
            oco = ocm[:, :].offset
            otr = ot_pool.tile([WW, WS * C], BF16, tag="ot")
            pot = otr[:, :].ap[0][0]
            poff = otr[:, :].offset
            for tr in range(WS):
                ops_t = ps_t.tile([128, 392], BF16, tag="t")
                for k in range(3):
                    nc.tensor.transpose(
                        ops_t[0:WW, 128 * k:128 * k + 128],
                        ocm[:, TW * k + WW * tr:TW * k + WW * tr + WW],
                        identb[:, :])
                nc.vector.tensor_copy(otr[0:WW, tr * C:tr * C + C],
                                      ops_t[0:WW, 0:C])
            rows = [(WS * wr + DISP, 0, WS)] if wr < NWS - 1 else \
                [(52, 0, 4), (0, 4, 3)]
            for (r0, tr0, ntr) in rows:
                src = _ap(otr, poff + tr0 * C, [[pot, 53], [C, ntr], [1, C]])
                dst = _ap(out_d, (img * NTOK + r0 * WW + DISP) * C,
                          [[C, 53], [WW * C, ntr], [1, C]])
                nc.gpsimd.dma_start(dst, src)
                src = _ap(otr, poff + 53 * pot + tr0 * C,
                          [[pot, 3], [C, ntr], [1, C]])
                dst = _ap(out_d, (img * NTOK + r0 * WW) * C,
                          [[C, 3], [WW * C, ntr], [1, C]])
                nc.gpsimd.dma_start(dst, src)

        for i in range(NT + 5):
            if i < NT:
                do_gather(i)
            if 0 <= i - 2 < NT:
                do_p2a(i - 2)
            if 0 <= i - 1 < NT:
                do_p1(i - 1)
            if 0 <= i - 4 < NT:
                do_p4a(i - 4)
            if 0 <= i - 5 < NT:
                do_p5(i - 5)
            if 0 <= i - 2 < NT:
                do_p2b(i - 2)
            if 0 <= i - 3 < NT:
                do_p3(i - 3)
            if 0 <= i - 4 < NT:
                do_p4b(i - 4)
    nc.compile()
    return nc


def _to_bf(a):
    return np.ascontiguousarray(a.astype(BF))


def prep_inputs(inputs):
    """Host-side weight/bias reformatting (layout only + tiny bias algebra)."""
    f = {k: np.asarray(v, dtype=np.float32) for k, v in inputs.items()}
    qkv_w, qkv_b = f["qkv_w"], f["qkv_b"]
    out_w, out_b = f["out_w"], f["out_b"]
    w1, b1, w2, b2 = f["mlp_w1"], f["mlp_b1"], f["mlp_w2"], f["mlp_b2"]
    g1, be1 = f["norm1_g"], f["norm1_b"]
    g2, be2 = f["norm2_g"], f["norm2_b"]

    wv = qkv_w[:, 2 * C:3 * C]
    bv = qkv_b[2 * C:3 * C]

    wv_t = np.zeros((128, 3 * C), np.float32)
    for k in range(3):
        wv_t[:, C * k:C * k + C] = wv[128 * k:128 * k + 128, :]

    wo_t = np.zeros((64, 18 * 128), np.float32)
    for r in range(6):
        for q in range(2):
            for kj in range(3):
                wo_t[32 * q:32 * q + 32, (r * 3 + kj) * 128:(r * 3 + kj) * 128 + 128] = \
                    out_w[32 * (r + 6 * q):32 * (r + 6 * q) + 32, 128 * kj:128 * kj + 128]

    w1_t = np.zeros((128, 3 * MLP), np.float32)
    for k in range(3):
        w1_t[:, MLP * k:MLP * k + MLP] = w1[128 * k:128 * k + 128, :]
    w2_t = np.zeros((128, 12 * C), np.float32)
    for k2 in range(12):
        w2_t[:, C * k2:C * k2 + C] = w2[128 * k2:128 * k2 + 128, :]

    # ybias[n2, :] = P(bv)[n2] @ out_w + out_b
    n2 = np.arange(N)[:, None]
    co = np.arange(12)[None, :]
    hh = (12 * n2 + co) // N                    # (49, 12)
    pb = bv.reshape(12, 32)[hh]                 # (49, 12, 32)
    ybias = pb.reshape(N, C) @ out_w + out_b    # (49, 384)
    yb_t = np.zeros((128, 3 * TW), np.float32)
    tr_ = np.arange(WS)[:, None, None]
    wn_ = np.arange(WT)[None, :, None]
    tc_ = np.arange(WS)[None, None, :]
    pc = (WS * tr_ + tc_ + 0 * wn_).reshape(TW)   # n2 per permuted column
    for kj in range(3):
        blk = ybias[:, 128 * kj:128 * kj + 128].T          # (128, 49)
        yb_t[:, TW * kj:TW * kj + TW] = blk[:, pc]

    b1p = b1 - be2 @ w1                          # mlp bias corrected for +be2 on x2
    be12 = be1 + be2
    cn_t = np.zeros((128, 27), np.float32)
    for m in range(12):
        cn_t[:, m] = b1p[128 * m:128 * m + 128]
    for k in range(3):
        cn_t[:, 12 + k] = b2[128 * k:128 * k + 128]
        cn_t[:, 15 + k] = g1[128 * k:128 * k + 128]
        cn_t[:, 18 + k] = g2[128 * k:128 * k + 128]
        cn_t[:, 21 + k] = be12[128 * k:128 * k + 128]

    ones_t = np.full((128, 128), 1.0 / C, np.float32)
    ident = np.eye(128, dtype=np.float32)

    # column-roll permutation for the input transposes
    perm = np.zeros((WW, WW), np.float32)
    jj = np.arange(WW)
    perm[(jj + DISP) % WW, jj] = 1.0

    return {
        "wv_t": _to_bf(wv_t), "wo_t": _to_bf(wo_t),
        "w1_t": _to_bf(w1_t), "w2_t": _to_bf(w2_t),
        "yb_t": np.ascontiguousarray(yb_t), "cn_t": np.ascontiguousarray(cn_t),
        "ones_t": _to_bf(ones_t), "identb_t": _to_bf(ident),
        "perm_t": _to_bf(perm),
    }


_CACHE = {}


def kernel(**inputs):
    if "nc" not in _CACHE:
        _CACHE["nc"] = build()
    nc = _CACHE["nc"]
    x = np.asarray(inputs["x"], dtype=np.float32).astype(BF)
    base = prep_inputs(inputs)
    in_maps = []
    for c in range(NCORES):
        m = dict(base)
        m["x"] = np.ascontiguousarray(
            x[IMGS * c:IMGS * (c + 1)].reshape(IMGS * NTOK, C))
        in_maps.append(m)
    import os
    trace = bool(int(os.environ.get("KERNEL_TRACE", "0")))
    res = run_bass_kernel_spmd(nc, in_maps, core_ids=list(range(NCORES)),
                               trace=trace)
    _CACHE["last_res"] = res
    out = np.concatenate([r["out"].reshape(IMGS, HH, WW, C) for r in res.results],
                         axis=0)
    return out



# revision 26
# speedup vs baseline: 1.1475x; 1.0119x over previous
"""Trainium2 Bass kernel v2 for nn_Encoder_59219009077683 (Swin-style block).

Math shortcut (from baseline): softmax row-sums are 1, so attention output = v
and the whole block is: window-gather -> V-proj -> fixed permutation (L_d
layout) -> out-proj -> LN1+skip -> MLP -> LN2+skip -> window-scatter.

v2 structural changes vs baseline:
  - gather/scatter via ONE SWDGE indirect DMA per tile (host-built index
    table; the cyclic roll/wrap is baked into the indices)
  - bf16 for all SBUF intermediates and matmuls (tolerance is 2e-2)
  - K=64 out-projection: L_d stored twice (quadrant q holds T[d, c+6q]) so
    the 12 K=32 chunk-matmuls become 6 K=64 matmuls per output block
  - host-folded biases: v-bias+out-bias -> ybias table added at y-evac;
    norm1_b+norm2_b added at x-transpose evac; mlp_b1 corrected by
    -W1^T norm2_b so the norm2 beta rides the x2 skip path
  - rsqrt via tensor_scalar(pow -0.5) on DVE: scalar engine runs only
    Gelu/Identity -> a single activation-table load
  - elementwise spread across DVE / Act / GpSimd
"""
import numpy as np
from contextlib import ExitStack

import concourse.bass as bass
import concourse.bacc as bacc
import concourse.tile as tile
from concourse import mybir
from concourse.bass_utils import run_bass_kernel_spmd, ml_dtypes

F32 = mybir.dt.float32
BF16 = mybir.dt.bfloat16
U32 = mybir.dt.uint32
AF = mybir.ActivationFunctionType
OP = mybir.AluOpType

B, HH, WW, C = 32, 56, 56, 384
NH, HD, WS, DISP, MLP = 12, 32, 7, 3, 1536
NWS = 8
N = 49
J = NH * N            # 588
NCORES = 8
IMGS = B // NCORES
WT = 8                # windows per tile (= one window row)
TW = WT * N           # 392
NTOK = HH * WW        # tokens per image
EPS = 1e-5
LDPAD = 16
LDW = LDPAD + J * WT + 8

BF = ml_dtypes.bfloat16


def _ap(t, offset, dims):
    tt = t.tensor if hasattr(t, "tensor") else t
    return bass.AP(tensor=tt, offset=offset, ap=[list(d) for d in dims])


def build():
    nc = bacc.Bacc("TRN2", target_bir_lowering=False, debug=False, num_devices=NCORES)
    x_d = nc.dram_tensor("x", [IMGS * NTOK, C], BF16, kind="ExternalInput")
    wv_d = nc.dram_tensor("wv_t", [128, 3 * C], BF16, kind="ExternalInput")
    wo_d = nc.dram_tensor("wo_t", [64, 18 * 128], BF16, kind="ExternalInput")
    w1_d = nc.dram_tensor("w1_t", [128, 3 * MLP], BF16, kind="ExternalInput")
    w2_d = nc.dram_tensor("w2_t", [128, 12 * C], BF16, kind="ExternalInput")
    yb_d = nc.dram_tensor("yb_t", [128, 3 * TW], F32, kind="ExternalInput")
    cn_d = nc.dram_tensor("cn_t", [128, 27], F32, kind="ExternalInput")
    on_d = nc.dram_tensor("ones_t", [128, 128], BF16, kind="ExternalInput")
    idb_d = nc.dram_tensor("identb_t", [128, 128], BF16, kind="ExternalInput")
    pm_d = nc.dram_tensor("perm_t", [WW, WW], BF16, kind="ExternalInput")
    out_d = nc.dram_tensor("out", [IMGS * NTOK, C], F32, kind="ExternalOutput")

    with tile.TileContext(nc) as tc, ExitStack() as ctx:
        wpool = ctx.enter_context(tc.tile_pool(name="w", bufs=1))
        stage_pool = ctx.enter_context(tc.tile_pool(name="stage", bufs=3))
        xt_pool = ctx.enter_context(tc.tile_pool(name="xt", bufs=4))
        vt_pool = ctx.enter_context(tc.tile_pool(name="vt", bufs=3))
        ld_pool = ctx.enter_context(tc.tile_pool(name="ld", bufs=2))
        y_pool = ctx.enter_context(tc.tile_pool(name="y", bufs=2))
        x2_pool = ctx.enter_context(tc.tile_pool(name="x2", bufs=3))
        h_pool = ctx.enter_context(tc.tile_pool(name="h", bufs=3))
        h2_pool = ctx.enter_context(tc.tile_pool(name="h2", bufs=2))
        oc_pool = ctx.enter_context(tc.tile_pool(name="oc", bufs=2))
        ot_pool = ctx.enter_context(tc.tile_pool(name="ot", bufs=2))
        sm_pool = ctx.enter_context(tc.tile_pool(name="sm", bufs=3))
        ps_t = ctx.enter_context(tc.tile_pool(name="pst", bufs=1, space="PSUM"))
        ps_a = ctx.enter_context(tc.tile_pool(name="psa", bufs=2, space="PSUM"))
        ps_m = ctx.enter_context(tc.tile_pool(name="psm", bufs=2, space="PSUM"))
        ps_s = ctx.enter_context(tc.tile_pool(name="pss", bufs=1, space="PSUM"))
        ps_o = ctx.enter_context(tc.tile_pool(name="pso", bufs=2, space="PSUM"))

        # ---------- one-time setup: weights straight from host layout ----------
        wv_b = wpool.tile([128, 3 * C], BF16)
        wo_b = wpool.tile([64, 18 * 128], BF16)
        w1_b = wpool.tile([128, 3 * MLP], BF16)
        w2_b = wpool.tile([128, 12 * C], BF16)
        yb_sb = wpool.tile([128, 3 * TW], F32)
        cn_sb = wpool.tile([128, 27], F32)
        ones_b = wpool.tile([128, 128], BF16)
        identb = wpool.tile([128, 128], BF16)
        perm56 = wpool.tile([WW, WW], BF16)
        eps_t = wpool.tile([128, 1], F32)
        zero_t = wpool.tile([128, 1], F32)
        nc.vector.memset(eps_t[:, :], EPS)
        nc.vector.memset(zero_t[:, :], 0.0)
        nc.sync.dma_start(perm56[:, :], pm_d[:, :])
        nc.sync.dma_start(wv_b[:, :], wv_d[:, :])
        nc.sync.dma_start(wo_b[:, :], wo_d[:, :])
        nc.sync.dma_start(w1_b[:, :], w1_d[:, :])
        nc.sync.dma_start(w2_b[:, :], w2_d[:, :])
        nc.sync.dma_start(yb_sb[:, :], yb_d[:, :])
        nc.sync.dma_start(cn_sb[:, :], cn_d[:, :])
        nc.sync.dma_start(ones_b[:, :], on_d[:, :])
        nc.sync.dma_start(identb[:, :], idb_d[:, :])
        # PE must observe the identity via a transpose once before the loop so
        # later transposes carry <=1 sync wait (bare LDWEIGHTS limitation).
        dmy = ps_t.tile([128, 392], BF16, tag="t")
        nc.tensor.transpose(dmy[0:WW, 0:WW], perm56[0:WW, 0:WW], perm56[0:WW, 0:WW])
        dmy2 = ps_t.tile([128, 392], BF16, tag="t")
        nc.tensor.transpose(dmy2[0:128, 0:128], identb[:, :], identb[:, :])

        # ---------- software-pipelined main loop ----------
        # tile index t = img*8 + wr; phases skewed across iterations so every
        # engine (PE especially) always has ready work.
        NT = IMGS * NWS
        st = {}   # per-tile live state

        def do_gather(t):
            img, wr = divmod(t, NWS)
            stage = stage_pool.tile([WW, WS * C], BF16, tag="stage")
            pst = stage[:, :].ap[0][0]
            soff = stage[:, :].offset
            rows = [(WS * wr + DISP, 0, WS)] if wr < NWS - 1 else \
                [(52, 0, 4), (0, 4, 3)]
            for (r0, tr0, ntr) in rows:
                src = _ap(x_d, (img * NTOK + r0 * WW) * C,
                          [[C, WW], [WW * C, ntr], [1, C]])
                dst = _ap(stage, soff + tr0 * C,
                          [[pst, WW], [C, ntr], [1, C]])
                nc.sync.dma_start(dst, src)
            st[t] = {"stage": stage}

        def do_p1(t):
            """transposes + V-proj + ld build"""
            d = st[t]
            stage = d.pop("stage")
            xt = xt_pool.tile([128, 3 * TW], BF16, tag="xt")
            for k in range(3):
                xps = ps_t.tile([128, 392], BF16, tag="t")
                pxp = xps[:, :].ap[0][0]
                xpo = xps[:, :].offset
                for tr in range(WS):
                    nc.tensor.transpose(
                        xps[:, WW * tr:WW * tr + WW],
                        stage[0:WW, tr * C + 128 * k:tr * C + 128 * k + 128],
                        perm56[0:WW, 0:WW])
                # token order everywhere downstream: col = 56*tr + 7*win + tc
                nc.vector.tensor_scalar(xt[:, TW * k:TW * k + TW], xps[:, :],
                                        cn_sb[:, 21 + k:22 + k], None, op0=OP.add)
            vt = vt_pool.tile([128, 3 * TW], BF16, tag="vt")
            pvt = vt[:, :].ap[0][0]
            vtoff = vt[:, :].offset
            for kv in range(3):
                vps = ps_a.tile([128, TW], F32, tag="a")
                for k in range(3):
                    nc.tensor.matmul(vps[:, :],
                                     wv_b[:, C * k + 128 * kv:C * k + 128 * kv + 128],
                                     xt[:, TW * k:TW * k + TW],
                                     start=(k == 0), stop=(k == 2))
                pps = vps[:, :].ap[0][0]
                nc.scalar.activation(
                    _ap(vt, vtoff + N * kv,
                        [[pvt, 128], [WS, WS], [3 * N, WT], [1, WS]]),
                    _ap(vps, vps[:, :].offset,
                        [[pps, 128], [WW, WS], [WS, WT], [1, WS]]),
                    AF.Identity, bias=zero_t[:, :], scale=1.0)
            ld = ld_pool.tile([64, LDW], BF16, tag="ld")
            pld = ld[:, :].ap[0][0]
            ldoff = ld[:, :].offset
            for s4 in range(4):
                for q in range(2):
                    src = _ap(vt, vtoff + (32 * s4) * pvt,
                              [[pvt, 32], [N, 24], [1, N]])
                    dst = _ap(ld, ldoff + (32 * q) * pld + LDPAD + N * s4 - 6 * q,
                              [[pld, 32], [4 * N, 24], [1, N]])
                    nc.sync.dma_start(dst, src)
            d["xt"] = xt
            d["ld"] = (ld, pld, ldoff)

        def do_p2a(t):
            """out-proj + ybias evac + ysq"""
            d = st[t]
            ld, pld, ldoff = d.pop("ld")
            y = y_pool.tile([128, 3 * TW], BF16, tag="y")
            for kj in range(3):
                yps = ps_o.tile([128, TW], F32, tag="o")
                for r in range(6):
                    rhs = _ap(ld, ldoff + LDPAD + r,
                              [[pld, 64], [84, WS], [J, WT], [12, WS]])
                    nc.tensor.matmul(yps[:, :],
                                     wo_b[:, (r * 3 + kj) * 128:(r * 3 + kj) * 128 + 128],
                                     rhs, start=(r == 0), stop=(r == 5))
                nc.vector.tensor_add(y[:, TW * kj:TW * kj + TW], yps[:, :],
                                     yb_sb[:, TW * kj:TW * kj + TW])
            ysq = sm_pool.tile([128, 3 * TW], BF16, tag="ysq")
            for k in range(3):
                nc.gpsimd.tensor_mul(ysq[:, TW * k:TW * k + TW],
                                     y[:, TW * k:TW * k + TW], y[:, TW * k:TW * k + TW])
            d["y"] = y
            d["ysq"] = ysq

        def _ln(y, ysq, gcol, skip, out):
            """matmul sums -> stats -> apply; out = (y-mu)*rst*g + skip."""
            s1 = ps_s.tile([128, TW], F32, tag="s")
            s2 = ps_s.tile([128, TW], F32, tag="s")
            for k in range(3):
                nc.tensor.matmul(s1[:, :], ones_b[:, :], y[:, TW * k:TW * k + TW],
                                 start=(k == 0), stop=(k == 2))
                nc.tensor.matmul(s2[:, :], ones_b[:, :], ysq[:, TW * k:TW * k + TW],
                                 start=(k == 0), stop=(k == 2))
            mu = sm_pool.tile([128, TW], F32, tag="mu")
            m2 = sm_pool.tile([128, TW], F32, tag="m2")
            veps = sm_pool.tile([128, TW], F32, tag="veps")
            rst = sm_pool.tile([128, TW], F32, tag="rst")
            nc.vector.tensor_copy(mu[:, :], s1[:, :])
            nc.gpsimd.tensor_mul(m2[:, :], mu[:, :], mu[:, :])
            nc.vector.tensor_sub(veps[:, :], s2[:, :], m2[:, :])
            # std = sqrt(var + eps) on Act (eps folded into the bias), then
            # 1/std via the fast custom-DVE reciprocal (~51 ULP, plenty here)
            nc.scalar.activation(veps[:, :], veps[:, :], AF.Sqrt,
                                 bias=eps_t[:, :], scale=1.0)
            nc.vector.reciprocal_approx_fast(rst[:, :], veps[:, :])
            for k in range(3):
                u = sm_pool.tile([128, TW], BF16, tag="u")
                v = sm_pool.tile([128, TW], BF16, tag="v")
                nc.vector.tensor_sub(u[:, :], y[:, TW * k:TW * k + TW], mu[:, :])
                nc.gpsimd.tensor_mul(v[:, :], u[:, :], rst[:, :])
                nc.vector.scalar_tensor_tensor(out[:, TW * k:TW * k + TW],
                                               v[:, :],
                                               cn_sb[:, gcol + k:gcol + k + 1],
                                               skip[:, TW * k:TW * k + TW],
                                               op0=OP.mult, op1=OP.add)

        def do_p2b(t):
            d = st[t]
            x2 = x2_pool.tile([128, 3 * TW], BF16, tag="x2")
            _ln(d.pop("y"), d.pop("ysq"), 15, d.pop("xt"), x2)
            d["x2"] = x2

        def do_p3(t):
            d = st[t]
            x2 = d["x2"]
            hsb = h_pool.tile([128, 12 * TW], BF16, tag="h")
            for m in range(12):
                hps = ps_m.tile([128, TW], F32, tag="m")
                for k in range(3):
                    nc.tensor.matmul(hps[:, :],
                                     w1_b[:, MLP * k + 128 * m:MLP * k + 128 * m + 128],
                                     x2[:, TW * k:TW * k + TW],
                                     start=(k == 0), stop=(k == 2))
                nc.scalar.activation(hsb[:, TW * m:TW * m + TW], hps[:, :],
                                     AF.Gelu, bias=cn_sb[:, m:m + 1], scale=1.0)
            d["hsb"] = hsb

        def do_p4a(t):
            d = st[t]
            hsb = d.pop("hsb")
            h2 = h2_pool.tile([128, 3 * TW], BF16, tag="h2")
            for kj in range(3):
                h2ps = ps_a.tile([128, TW], F32, tag="a")
                for k2 in range(12):
                    nc.tensor.matmul(h2ps[:, :],
                                     w2_b[:, C * k2 + 128 * kj:C * k2 + 128 * kj + 128],
                                     hsb[:, TW * k2:TW * k2 + TW],
                                     start=(k2 == 0), stop=(k2 == 11))
                nc.vector.tensor_scalar(h2[:, TW * kj:TW * kj + TW], h2ps[:, :],
                                        cn_sb[:, 12 + kj:13 + kj], None, op0=OP.add)
            hsq = sm_pool.tile([128, 3 * TW], BF16, tag="ysq")
            for k in range(3):
                nc.gpsimd.tensor_mul(hsq[:, TW * k:TW * k + TW],
                                     h2[:, TW * k:TW * k + TW], h2[:, TW * k:TW * k + TW])
            d["h2"] = h2
            d["hsq"] = hsq

        def do_p4b(t):
            d = st[t]
            ocm = oc_pool.tile([128, 3 * TW], BF16, tag="oc")
            _ln(d.pop("h2"), d.pop("hsq"), 18, d.pop("x2"), ocm)
            d["ocm"] = ocm

        def do_p5(t):
            d = st.pop(t)
            img, wr = divmod(t, NWS)
            ocm = d["ocm"]
            poc = ocm[:, :].ap[0][0]
            oco = ocm[:, :].offset
            # paired: otr[56*(tr%2)+x, 384*(tr//2)+c]; 12 transposes not 21
            otr = ot_pool.tile([112, 4 * C], BF16, tag="ot")
            pot = otr[:, :].ap[0][0]
            poff = otr[:, :].offset
            for p in range(4):
                prow = 112 if p < 3 else 56
                ops_t = ps_t.tile([128, 392], BF16, tag="t")
                for k in range(3):
                    nc.tensor.transpose(
                        ops_t[0:prow, 128 * k:128 * k + 128],
                        ocm[:, TW * k + 112 * p:TW * k + 112 * p + prow],
                        identb[:, :])
                nc.vector.tensor_copy(otr[0:prow, p * C:p * C + C],
                                      ops_t[0:prow, 0:C])

            def scat(prow0, npair, r0, odd):
                base = poff + (56 * odd) * pot + 384 * prow0
                dst0 = (img * NTOK + r0 * WW) * C
                src = _ap(otr, base, [[pot, 53], [C, npair], [1, C]])
                dst = _ap(out_d, dst0 + DISP * C,
                          [[C, 53], [2 * WW * C, npair], [1, C]])
                nc.gpsimd.dma_start(dst, src)
                src = _ap(otr, base + 53 * pot, [[pot, 3], [C, npair], [1, C]])
                dst = _ap(out_d, dst0, [[C, 3], [2 * WW * C, npair], [1, C]])
                nc.gpsimd.dma_start(dst, src)

            if wr < NWS - 1:
                r0 = WS * wr + DISP
                scat(0, 4, r0, 0)
                scat(0, 3, r0 + 1, 1)
            else:
                scat(0, 2, 52, 0)
                scat(0, 2, 53, 1)
                scat(2, 2, 0, 0)
                scat(2, 1, 1, 1)

        for i in range(NT + 5):
            if i < NT:
                do_gather(i)
            if 0 <= i - 2 < NT:
                do_p2a(i - 2)
            if 0 <= i - 1 < NT:
                do_p1(i - 1)
            if 0 <= i - 4 < NT:
                do_p4a(i - 4)
            if 0 <= i - 5 < NT:
                do_p5(i - 5)
            if 0 <= i - 2 < NT:
                do_p2b(i - 2)
            if 0 <= i - 3 < NT:
                do_p3(i - 3)
            if 0 <= i - 4 < NT:
                do_p4b(i - 4)
    nc.compile()
    return nc


def _to_bf(a):
    return np.ascontiguousarray(a.astype(BF))


def prep_inputs(inputs):
    """Host-side weight/bias reformatting (layout only + tiny bias algebra)."""
    f = {k: np.asarray(v, dtype=np.float32) for k, v in inputs.items()}
    qkv_w, qkv_b = f["qkv_w"], f["qkv_b"]
    out_w, out_b = f["out_w"], f["out_b"]
    w1, b1, w2, b2 = f["mlp_w1"], f["mlp_b1"], f["mlp_w2"], f["mlp_b2"]
    g1, be1 = f["norm1_g"], f["norm1_b"]
    g2, be2 = f["norm2_g"], f["norm2_b"]

    wv = qkv_w[:, 2 * C:3 * C]
    bv = qkv_b[2 * C:3 * C]

    wv_t = np.zeros((128, 3 * C), np.float32)
    for k in range(3):
        wv_t[:, C * k:C * k + C] = wv[128 * k:128 * k + 128, :]

    wo_t = np.zeros((64, 18 * 128), np.float32)
    for r in range(6):
        for q in range(2):
            for kj in range(3):
                wo_t[32 * q:32 * q + 32, (r * 3 + kj) * 128:(r * 3 + kj) * 128 + 128] = \
                    out_w[32 * (r + 6 * q):32 * (r + 6 * q) + 32, 128 * kj:128 * kj + 128]

    w1_t = np.zeros((128, 3 * MLP), np.float32)
    for k in range(3):
        w1_t[:, MLP * k:MLP * k + MLP] = w1[128 * k:128 * k + 128, :]
    w2_t = np.zeros((128, 12 * C), np.float32)
    for k2 in range(12):
        w2_t[:, C * k2:C * k2 + C] = w2[128 * k2:128 * k2 + 128, :]

    # ybias[n2, :] = P(bv)[n2] @ out_w + out_b
    n2 = np.arange(N)[:, None]
    co = np.arange(12)[None, :]
    hh = (12 * n2 + co) // N                    # (49, 12)
    pb = bv.reshape(12, 32)[hh]                 # (49, 12, 32)
    ybias = pb.reshape(N, C) @ out_w + out_b    # (49, 384)
    yb_t = np.zeros((128, 3 * TW), np.float32)
    tr_ = np.arange(WS)[:, None, None]
    wn_ = np.arange(WT)[None, :, None]
    tc_ = np.arange(WS)[None, None, :]
    pc = (WS * tr_ + tc_ + 0 * wn_).reshape(TW)   # n2 per permuted column
    for kj in range(3):
        blk = ybias[:, 128 * kj:128 * kj + 128].T          # (128, 49)
        yb_t[:, TW * kj:TW * kj + TW] = blk[:, pc]

    b1p = b1 - be2 @ w1                          # mlp bias corrected for +be2 on x2
    be12 = be1 + be2
    cn_t = np.zeros((128, 27), np.float32)
    for m in range(12):
        cn_t[:, m] = b1p[128 * m:128 * m + 128]
    for k in range(3):
        cn_t[:, 12 + k] = b2[128 * k:128 * k + 128]
        cn_t[:, 15 + k] = g1[128 * k:128 * k + 128]
        cn_t[:, 18 + k] = g2[128 * k:128 * k + 128]
        cn_t[:, 21 + k] = be12[128 * k:128 * k + 128]

    ones_t = np.full((128, 128), 1.0 / C, np.float32)
    ident = np.eye(128, dtype=np.float32)

    # column-roll permutation for the input transposes
    perm = np.zeros((WW, WW), np.float32)
    jj = np.arange(WW)
    perm[(jj + DISP) % WW, jj] = 1.0

    return {
        "wv_t": _to_bf(wv_t), "wo_t": _to_bf(wo_t),
        "w1_t": _to_bf(w1_t), "w2_t": _to_bf(w2_t),
        "yb_t": np.ascontiguousarray(yb_t), "cn_t": np.ascontiguousarray(cn_t),
        "ones_t": _to_bf(ones_t), "identb_t": _to_bf(ident),
        "perm_t": _to_bf(perm),
    }


_CACHE = {}


def kernel(**inputs):
    if "nc" not in _CACHE:
        _CACHE["nc"] = build()
    nc = _CACHE["nc"]
    x = np.asarray(inputs["x"], dtype=np.float32).astype(BF)
    base = prep_inputs(inputs)
    in_maps = []
    for c in range(NCORES):
        m = dict(base)
        m["x"] = np.ascontiguousarray(
            x[IMGS * c:IMGS * (c + 1)].reshape(IMGS * NTOK, C))
        in_maps.append(m)
    import os
    trace = bool(int(os.environ.get("KERNEL_TRACE", "0")))
    res = run_bass_kernel_spmd(nc, in_maps, core_ids=list(range(NCORES)),
                               trace=trace)
    _CACHE["last_res"] = res
    out = np.concatenate([r["out"].reshape(IMGS, HH, WW, C) for r in res.results],
                         axis=0)
    return out



# revision 28
# speedup vs baseline: 1.1534x; 1.0052x over previous
"""Trainium2 Bass kernel v2 for nn_Encoder_59219009077683 (Swin-style block).

Math shortcut (from baseline): softmax row-sums are 1, so attention output = v
and the whole block is: window-gather -> V-proj -> fixed permutation (L_d
layout) -> out-proj -> LN1+skip -> MLP -> LN2+skip -> window-scatter.

v2 structural changes vs baseline:
  - gather/scatter via ONE SWDGE indirect DMA per tile (host-built index
    table; the cyclic roll/wrap is baked into the indices)
  - bf16 for all SBUF intermediates and matmuls (tolerance is 2e-2)
  - K=64 out-projection: L_d stored twice (quadrant q holds T[d, c+6q]) so
    the 12 K=32 chunk-matmuls become 6 K=64 matmuls per output block
  - host-folded biases: v-bias+out-bias -> ybias table added at y-evac;
    norm1_b+norm2_b added at x-transpose evac; mlp_b1 corrected by
    -W1^T norm2_b so the norm2 beta rides the x2 skip path
  - rsqrt via tensor_scalar(pow -0.5) on DVE: scalar engine runs only
    Gelu/Identity -> a single activation-table load
  - elementwise spread across DVE / Act / GpSimd
"""
import numpy as np
from contextlib import ExitStack

import concourse.bass as bass
import concourse.bacc as bacc
import concourse.tile as tile
from concourse import mybir
from concourse.bass_utils import run_bass_kernel_spmd, ml_dtypes

F32 = mybir.dt.float32
BF16 = mybir.dt.bfloat16
U32 = mybir.dt.uint32
AF = mybir.ActivationFunctionType
OP = mybir.AluOpType

B, HH, WW, C = 32, 56, 56, 384
NH, HD, WS, DISP, MLP = 12, 32, 7, 3, 1536
NWS = 8
N = 49
J = NH * N            # 588
NCORES = 8
IMGS = B // NCORES
WT = 8                # windows per tile (= one window row)
TW = WT * N           # 392
NTOK = HH * WW        # tokens per image
EPS = 1e-5
LDPAD = 16
LDW = LDPAD + J * WT + 8

BF = ml_dtypes.bfloat16


def _ap(t, offset, dims):
    tt = t.tensor if hasattr(t, "tensor") else t
    return bass.AP(tensor=tt, offset=offset, ap=[list(d) for d in dims])


def build():
    nc = bacc.Bacc("TRN2", target_bir_lowering=False, debug=False, num_devices=NCORES)
    x_d = nc.dram_tensor("x", [IMGS * NTOK, C], BF16, kind="ExternalInput")
    wv_d = nc.dram_tensor("wv_t", [128, 3 * C], BF16, kind="ExternalInput")
    wo_d = nc.dram_tensor("wo_t", [64, 18 * 128], BF16, kind="ExternalInput")
    w1_d = nc.dram_tensor("w1_t", [128, 3 * MLP], BF16, kind="ExternalInput")
    w2_d = nc.dram_tensor("w2_t", [128, 12 * C], BF16, kind="ExternalInput")
    yb_d = nc.dram_tensor("yb_t", [128, 3 * TW], F32, kind="ExternalInput")
    cn_d = nc.dram_tensor("cn_t", [128, 27], F32, kind="ExternalInput")
    on_d = nc.dram_tensor("ones_t", [128, 128], BF16, kind="ExternalInput")
    idb_d = nc.dram_tensor("identb_t", [128, 128], BF16, kind="ExternalInput")
    pm_d = nc.dram_tensor("perm_t", [WW, WW], BF16, kind="ExternalInput")
    out_d = nc.dram_tensor("out", [IMGS * NTOK, C], F32, kind="ExternalOutput")

    with tile.TileContext(nc) as tc, ExitStack() as ctx:
        wpool = ctx.enter_context(tc.tile_pool(name="w", bufs=1))
        stage_pool = ctx.enter_context(tc.tile_pool(name="stage", bufs=4))
        xt_pool = ctx.enter_context(tc.tile_pool(name="xt", bufs=4))
        vt_pool = ctx.enter_context(tc.tile_pool(name="vt", bufs=3))
        ld_pool = ctx.enter_context(tc.tile_pool(name="ld", bufs=2))
        y_pool = ctx.enter_context(tc.tile_pool(name="y", bufs=2))
        x2_pool = ctx.enter_context(tc.tile_pool(name="x2", bufs=3))
        h_pool = ctx.enter_context(tc.tile_pool(name="h", bufs=3))
        h2_pool = ctx.enter_context(tc.tile_pool(name="h2", bufs=2))
        oc_pool = ctx.enter_context(tc.tile_pool(name="oc", bufs=2))
        ot_pool = ctx.enter_context(tc.tile_pool(name="ot", bufs=3))
        sm_pool = ctx.enter_context(tc.tile_pool(name="sm", bufs=4))
        ps_t = ctx.enter_context(tc.tile_pool(name="pst", bufs=1, space="PSUM"))
        ps_a = ctx.enter_context(tc.tile_pool(name="psa", bufs=2, space="PSUM"))
        ps_m = ctx.enter_context(tc.tile_pool(name="psm", bufs=2, space="PSUM"))
        ps_s = ctx.enter_context(tc.tile_pool(name="pss", bufs=1, space="PSUM"))
        ps_o = ctx.enter_context(tc.tile_pool(name="pso", bufs=2, space="PSUM"))

        # ---------- one-time setup: weights straight from host layout ----------
        wv_b = wpool.tile([128, 3 * C], BF16)
        wo_b = wpool.tile([64, 18 * 128], BF16)
        w1_b = wpool.tile([128, 3 * MLP], BF16)
        w2_b = wpool.tile([128, 12 * C], BF16)
        yb_sb = wpool.tile([128, 3 * TW], F32)
        cn_sb = wpool.tile([128, 27], F32)
        ones_b = wpool.tile([128, 128], BF16)
        identb = wpool.tile([128, 128], BF16)
        perm56 = wpool.tile([WW, WW], BF16)
        eps_t = wpool.tile([128, 1], F32)
        zero_t = wpool.tile([128, 1], F32)
        nc.vector.memset(eps_t[:, :], EPS)
        nc.vector.memset(zero_t[:, :], 0.0)
        nc.sync.dma_start(perm56[:, :], pm_d[:, :])
        nc.sync.dma_start(wv_b[:, :], wv_d[:, :])
        nc.sync.dma_start(wo_b[:, :], wo_d[:, :])
        nc.sync.dma_start(w1_b[:, :], w1_d[:, :])
        nc.sync.dma_start(w2_b[:, :], w2_d[:, :])
        nc.sync.dma_start(yb_sb[:, :], yb_d[:, :])
        nc.sync.dma_start(cn_sb[:, :], cn_d[:, :])
        nc.sync.dma_start(ones_b[:, :], on_d[:, :])
        nc.sync.dma_start(identb[:, :], idb_d[:, :])
        # PE must observe the identity via a transpose once before the loop so
        # later transposes carry <=1 sync wait (bare LDWEIGHTS limitation).
        dmy = ps_t.tile([128, 392], BF16, tag="t")
        nc.tensor.transpose(dmy[0:WW, 0:WW], perm56[0:WW, 0:WW], perm56[0:WW, 0:WW])
        dmy2 = ps_t.tile([128, 392], BF16, tag="t")
        nc.tensor.transpose(dmy2[0:128, 0:128], identb[:, :], identb[:, :])

        # ---------- software-pipelined main loop ----------
        # tile index t = img*8 + wr; phases skewed across iterations so every
        # engine (PE especially) always has ready work.
        NT = IMGS * NWS
        st = {}   # per-tile live state

        def do_gather(t):
            img, wr = divmod(t, NWS)
            stage = stage_pool.tile([WW, WS * C], BF16, tag="stage")
            pst = stage[:, :].ap[0][0]
            soff = stage[:, :].offset
            rows = [(WS * wr + DISP, 0, WS)] if wr < NWS - 1 else \
                [(52, 0, 4), (0, 4, 3)]
            for (r0, tr0, ntr) in rows:
                src = _ap(x_d, (img * NTOK + r0 * WW) * C,
                          [[C, WW], [WW * C, ntr], [1, C]])
                dst = _ap(stage, soff + tr0 * C,
                          [[pst, WW], [C, ntr], [1, C]])
                nc.sync.dma_start(dst, src)
            st[t] = {"stage": stage}

        def do_p1(t):
            """transposes + V-proj + ld build"""
            d = st[t]
            stage = d.pop("stage")
            xt = xt_pool.tile([128, 3 * TW], BF16, tag="xt")
            for k in range(3):
                xps = ps_t.tile([128, 392], BF16, tag="t")
                pxp = xps[:, :].ap[0][0]
                xpo = xps[:, :].offset
                for tr in range(WS):
                    nc.tensor.transpose(
                        xps[:, WW * tr:WW * tr + WW],
                        stage[0:WW, tr * C + 128 * k:tr * C + 128 * k + 128],
                        perm56[0:WW, 0:WW])
                # token order everywhere downstream: col = 56*tr + 7*win + tc
                nc.vector.tensor_scalar(xt[:, TW * k:TW * k + TW], xps[:, :],
                                        cn_sb[:, 21 + k:22 + k], None, op0=OP.add)
            vt = vt_pool.tile([128, 3 * TW], BF16, tag="vt")
            pvt = vt[:, :].ap[0][0]
            vtoff = vt[:, :].offset
            for kv in range(3):
                vps = ps_a.tile([128, TW], F32, tag="a")
                for k in range(3):
                    nc.tensor.matmul(vps[:, :],
                                     wv_b[:, C * k + 128 * kv:C * k + 128 * kv + 128],
                                     xt[:, TW * k:TW * k + TW],
                                     start=(k == 0), stop=(k == 2))
                pps = vps[:, :].ap[0][0]
                nc.scalar.activation(
                    _ap(vt, vtoff + N * kv,
                        [[pvt, 128], [WS, WS], [3 * N, WT], [1, WS]]),
                    _ap(vps, vps[:, :].offset,
                        [[pps, 128], [WW, WS], [WS, WT], [1, WS]]),
                    AF.Identity, bias=zero_t[:, :], scale=1.0)
            ld = ld_pool.tile([64, LDW], BF16, tag="ld")
            pld = ld[:, :].ap[0][0]
            ldoff = ld[:, :].offset
            for s4 in range(4):
                for q in range(2):
                    src = _ap(vt, vtoff + (32 * s4) * pvt,
                              [[pvt, 32], [N, 24], [1, N]])
                    dst = _ap(ld, ldoff + (32 * q) * pld + LDPAD + N * s4 - 6 * q,
                              [[pld, 32], [4 * N, 24], [1, N]])
                    nc.sync.dma_start(dst, src)
            d["xt"] = xt
            d["ld"] = (ld, pld, ldoff)

        def do_p2a(t):
            """out-proj + ybias evac + ysq"""
            d = st[t]
            ld, pld, ldoff = d.pop("ld")
            y = y_pool.tile([128, 3 * TW], BF16, tag="y")
            for kj in range(3):
                yps = ps_o.tile([128, TW], F32, tag="o")
                for r in range(6):
                    rhs = _ap(ld, ldoff + LDPAD + r,
                              [[pld, 64], [84, WS], [J, WT], [12, WS]])
                    nc.tensor.matmul(yps[:, :],
                                     wo_b[:, (r * 3 + kj) * 128:(r * 3 + kj) * 128 + 128],
                                     rhs, start=(r == 0), stop=(r == 5))
                nc.vector.tensor_add(y[:, TW * kj:TW * kj + TW], yps[:, :],
                                     yb_sb[:, TW * kj:TW * kj + TW])
            ysq = sm_pool.tile([128, 3 * TW], BF16, tag="ysq")
            for k in range(3):
                nc.gpsimd.tensor_mul(ysq[:, TW * k:TW * k + TW],
                                     y[:, TW * k:TW * k + TW], y[:, TW * k:TW * k + TW])
            d["y"] = y
            d["ysq"] = ysq

        def _ln(y, ysq, gcol, skip, out):
            """matmul sums -> stats -> apply; out = (y-mu)*rst*g + skip."""
            s1 = ps_s.tile([128, TW], F32, tag="s")
            s2 = ps_s.tile([128, TW], F32, tag="s")
            for k in range(3):
                nc.tensor.matmul(s1[:, :], ones_b[:, :], y[:, TW * k:TW * k + TW],
                                 start=(k == 0), stop=(k == 2))
                nc.tensor.matmul(s2[:, :], ones_b[:, :], ysq[:, TW * k:TW * k + TW],
                                 start=(k == 0), stop=(k == 2))
            mu = sm_pool.tile([128, TW], F32, tag="mu")
            m2 = sm_pool.tile([128, TW], F32, tag="m2")
            veps = sm_pool.tile([128, TW], F32, tag="veps")
            rst = sm_pool.tile([128, TW], F32, tag="rst")
            nc.vector.tensor_copy(mu[:, :], s1[:, :])
            nc.gpsimd.tensor_mul(m2[:, :], mu[:, :], mu[:, :])
            nc.vector.tensor_sub(veps[:, :], s2[:, :], m2[:, :])
            # std = sqrt(var + eps) on Act (eps folded into the bias), then
            # 1/std via the fast custom-DVE reciprocal (~51 ULP, plenty here)
            nc.scalar.activation(veps[:, :], veps[:, :], AF.Sqrt,
                                 bias=eps_t[:, :], scale=1.0)
            nc.vector.reciprocal_approx_fast(rst[:, :], veps[:, :])
            for k in range(3):
                u = sm_pool.tile([128, TW], BF16, tag="u")
                v = sm_pool.tile([128, TW], BF16, tag="v")
                nc.vector.tensor_sub(u[:, :], y[:, TW * k:TW * k + TW], mu[:, :])
                nc.gpsimd.tensor_mul(v[:, :], u[:, :], rst[:, :])
                nc.vector.scalar_tensor_tensor(out[:, TW * k:TW * k + TW],
                                               v[:, :],
                                               cn_sb[:, gcol + k:gcol + k + 1],
                                               skip[:, TW * k:TW * k + TW],
                                               op0=OP.mult, op1=OP.add)

        def do_p2b(t):
            d = st[t]
            x2 = x2_pool.tile([128, 3 * TW], BF16, tag="x2")
            _ln(d.pop("y"), d.pop("ysq"), 15, d.pop("xt"), x2)
            d["x2"] = x2

        def do_p3(t):
            d = st[t]
            x2 = d["x2"]
            hsb = h_pool.tile([128, 12 * TW], BF16, tag="h")
            for m in range(12):
                hps = ps_m.tile([128, TW], F32, tag="m")
                for k in range(3):
                    nc.tensor.matmul(hps[:, :],
                                     w1_b[:, MLP * k + 128 * m:MLP * k + 128 * m + 128],
                                     x2[:, TW * k:TW * k + TW],
                                     start=(k == 0), stop=(k == 2))
                nc.scalar.activation(hsb[:, TW * m:TW * m + TW], hps[:, :],
                                     AF.Gelu, bias=cn_sb[:, m:m + 1], scale=1.0)
            d["hsb"] = hsb

        def do_p4a(t):
            d = st[t]
            hsb = d.pop("hsb")
            h2 = h2_pool.tile([128, 3 * TW], BF16, tag="h2")
            for kj in range(3):
                h2ps = ps_a.tile([128, TW], F32, tag="a")
                for k2 in range(12):
                    nc.tensor.matmul(h2ps[:, :],
                                     w2_b[:, C * k2 + 128 * kj:C * k2 + 128 * kj + 128],
                                     hsb[:, TW * k2:TW * k2 + TW],
                                     start=(k2 == 0), stop=(k2 == 11))
                nc.vector.tensor_scalar(h2[:, TW * kj:TW * kj + TW], h2ps[:, :],
                                        cn_sb[:, 12 + kj:13 + kj], None, op0=OP.add)
            hsq = sm_pool.tile([128, 3 * TW], BF16, tag="ysq")
            for k in range(3):
                nc.gpsimd.tensor_mul(hsq[:, TW * k:TW * k + TW],
                                     h2[:, TW * k:TW * k + TW], h2[:, TW * k:TW * k + TW])
            d["h2"] = h2
            d["hsq"] = hsq

        def do_p4b(t):
            d = st[t]
            ocm = oc_pool.tile([128, 3 * TW], BF16, tag="oc")
            _ln(d.pop("h2"), d.pop("hsq"), 18, d.pop("x2"), ocm)
            d["ocm"] = ocm

        def do_p5(t):
            d = st.pop(t)
            img, wr = divmod(t, NWS)
            ocm = d["ocm"]
            poc = ocm[:, :].ap[0][0]
            oco = ocm[:, :].offset
            # paired: otr[56*(tr%2)+x, 384*(tr//2)+c]; 12 transposes not 21
            otr = ot_pool.tile([112, 4 * C], BF16, tag="ot")
            pot = otr[:, :].ap[0][0]
            poff = otr[:, :].offset
            for p in range(4):
                prow = 112 if p < 3 else 56
                ops_t = ps_t.tile([128, 392], BF16, tag="t")
                for k in range(3):
                    nc.tensor.transpose(
                        ops_t[0:prow, 128 * k:128 * k + 128],
                        ocm[:, TW * k + 112 * p:TW * k + 112 * p + prow],
                        identb[:, :])
                nc.vector.tensor_copy(otr[0:prow, p * C:p * C + C],
                                      ops_t[0:prow, 0:C])

            def scat(prow0, npair, r0, odd):
                base = poff + (56 * odd) * pot + 384 * prow0
                dst0 = (img * NTOK + r0 * WW) * C
                src = _ap(otr, base, [[pot, 53], [C, npair], [1, C]])
                dst = _ap(out_d, dst0 + DISP * C,
                          [[C, 53], [2 * WW * C, npair], [1, C]])
                nc.gpsimd.dma_start(dst, src)
                src = _ap(otr, base + 53 * pot, [[pot, 3], [C, npair], [1, C]])
                dst = _ap(out_d, dst0, [[C, 3], [2 * WW * C, npair], [1, C]])
                nc.gpsimd.dma_start(dst, src)

            if wr < NWS - 1:
                r0 = WS * wr + DISP
                scat(0, 4, r0, 0)
                scat(0, 3, r0 + 1, 1)
            else:
                scat(0, 2, 52, 0)
                scat(0, 2, 53, 1)
                scat(2, 2, 0, 0)
                scat(2, 1, 1, 1)

        for i in range(NT + 5):
            if i < NT:
                do_gather(i)
            if 0 <= i - 2 < NT:
                do_p2a(i - 2)
            if 0 <= i - 1 < NT:
                do_p1(i - 1)
            if 0 <= i - 4 < NT:
                do_p4a(i - 4)
            if 0 <= i - 5 < NT:
                do_p5(i - 5)
            if 0 <= i - 2 < NT:
                do_p2b(i - 2)
            if 0 <= i - 3 < NT:
                do_p3(i - 3)
            if 0 <= i - 4 < NT:
                do_p4b(i - 4)
    nc.compile()
    return nc


def _to_bf(a):
    return np.ascontiguousarray(a.astype(BF))


def prep_inputs(inputs):
    """Host-side weight/bias reformatting (layout only + tiny bias algebra)."""
    f = {k: np.asarray(v, dtype=np.float32) for k, v in inputs.items()}
    qkv_w, qkv_b = f["qkv_w"], f["qkv_b"]
    out_w, out_b = f["out_w"], f["out_b"]
    w1, b1, w2, b2 = f["mlp_w1"], f["mlp_b1"], f["mlp_w2"], f["mlp_b2"]
    g1, be1 = f["norm1_g"], f["norm1_b"]
    g2, be2 = f["norm2_g"], f["norm2_b"]

    wv = qkv_w[:, 2 * C:3 * C]
    bv = qkv_b[2 * C:3 * C]

    wv_t = np.zeros((128, 3 * C), np.float32)
    for k in range(3):
        wv_t[:, C * k:C * k + C] = wv[128 * k:128 * k + 128, :]

    wo_t = np.zeros((64, 18 * 128), np.float32)
    for r in range(6):
        for q in range(2):
            for kj in range(3):
                wo_t[32 * q:32 * q + 32, (r * 3 + kj) * 128:(r * 3 + kj) * 128 + 128] = \
                    out_w[32 * (r + 6 * q):32 * (r + 6 * q) + 32, 128 * kj:128 * kj + 128]

    w1_t = np.zeros((128, 3 * MLP), np.float32)
    for k in range(3):
        w1_t[:, MLP * k:MLP * k + MLP] = w1[128 * k:128 * k + 128, :]
    w2_t = np.zeros((128, 12 * C), np.float32)
    for k2 in range(12):
        w2_t[:, C * k2:C * k2 + C] = w2[128 * k2:128 * k2 + 128, :]

    # ybias[n2, :] = P(bv)[n2] @ out_w + out_b
    n2 = np.arange(N)[:, None]
    co = np.arange(12)[None, :]
    hh = (12 * n2 + co) // N                    # (49, 12)
    pb = bv.reshape(12, 32)[hh]                 # (49, 12, 32)
    ybias = pb.reshape(N, C) @ out_w + out_b    # (49, 384)
    yb_t = np.zeros((128, 3 * TW), np.float32)
    tr_ = np.arange(WS)[:, None, None]
    wn_ = np.arange(WT)[None, :, None]
    tc_ = np.arange(WS)[None, None, :]
    pc = (WS * tr_ + tc_ + 0 * wn_).reshape(TW)   # n2 per permuted column
    for kj in range(3):
        blk = ybias[:, 128 * kj:128 * kj + 128].T          # (128, 49)
        yb_t[:, TW * kj:TW * kj + TW] = blk[:, pc]

    b1p = b1 - be2 @ w1                          # mlp bias corrected for +be2 on x2
    be12 = be1 + be2
    cn_t = np.zeros((128, 27), np.float32)
    for m in range(12):
        cn_t[:, m] = b1p[128 * m:128 * m + 128]
    for k in range(3):
        cn_t[:, 12 + k] = b2[128 * k:128 * k + 128]
        cn_t[:, 15 + k] = g1[128 * k:128 * k + 128]
        cn_t[:, 18 + k] = g2[128 * k:128 * k + 128]
        cn_t[:, 21 + k] = be12[128 * k:128 * k + 128]

    ones_t = np.full((128, 128), 1.0 / C, np.float32)
    ident = np.eye(128, dtype=np.float32)

    # column-roll permutation for the input transposes
    perm = np.zeros((WW, WW), np.float32)
    jj = np.arange(WW)
    perm[(jj + DISP) % WW, jj] = 1.0

    return {
        "wv_t": _to_bf(wv_t), "wo_t": _to_bf(wo_t),
        "w1_t": _to_bf(w1_t), "w2_t": _to_bf(w2_t),
        "yb_t": np.ascontiguousarray(yb_t), "cn_t": np.ascontiguousarray(cn_t),
        "ones_t": _to_bf(ones_t), "identb_t": _to_bf(ident),
        "perm_t": _to_bf(perm),
    }


_CACHE = {}


def kernel(**inputs):
    if "nc" not in _CACHE:
        _CACHE["nc"] = build()
    nc = _CACHE["nc"]
    x = np.asarray(inputs["x"], dtype=np.float32).astype(BF)
    base = prep_inputs(inputs)
    in_maps = []
    for c in range(NCORES):
        m = dict(base)
        m["x"] = np.ascontiguousarray(
            x[IMGS * c:IMGS * (c + 1)].reshape(IMGS * NTOK, C))
        in_maps.append(m)
    import os
    trace = bool(int(os.environ.get("KERNEL_TRACE", "0")))
    res = run_bass_kernel_spmd(nc, in_maps, core_ids=list(range(NCORES)),
                               trace=trace)
    _CACHE["last_res"] = res
    out = np.concatenate([r["out"].reshape(IMGS, HH, WW, C) for r in res.results],
                         axis=0)
    return out



# revision 30
# speedup vs baseline: 1.1583x; 1.0042x over previous
"""Trainium2 Bass kernel v2 for nn_Encoder_59219009077683 (Swin-style block).

Math shortcut (from baseline): softmax row-sums are 1, so attention output = v
and the whole block is: window-gather -> V-proj -> fixed permutation (L_d
layout) -> out-proj -> LN1+skip -> MLP -> LN2+skip -> window-scatter.

v2 structural changes vs baseline:
  - gather/scatter via ONE SWDGE indirect DMA per tile (host-built index
    table; the cyclic roll/wrap is baked into the indices)
  - bf16 for all SBUF intermediates and matmuls (tolerance is 2e-2)
  - K=64 out-projection: L_d stored twice (quadrant q holds T[d, c+6q]) so
    the 12 K=32 chunk-matmuls become 6 K=64 matmuls per output block
  - host-folded biases: v-bias+out-bias -> ybias table added at y-evac;
    norm1_b+norm2_b added at x-transpose evac; mlp_b1 corrected by
    -W1^T norm2_b so the norm2 beta rides the x2 skip path
  - rsqrt via tensor_scalar(pow -0.5) on DVE: scalar engine runs only
    Gelu/Identity -> a single activation-table load
  - elementwise spread across DVE / Act / GpSimd
"""
import numpy as np
from contextlib import ExitStack

import concourse.bass as bass
import concourse.bacc as bacc
import concourse.tile as tile
from concourse import mybir
from concourse.bass_utils import run_bass_kernel_spmd, ml_dtypes

F32 = mybir.dt.float32
BF16 = mybir.dt.bfloat16
U32 = mybir.dt.uint32
AF = mybir.ActivationFunctionType
OP = mybir.AluOpType

B, HH, WW, C = 32, 56, 56, 384
NH, HD, WS, DISP, MLP = 12, 32, 7, 3, 1536
NWS = 8
N = 49
J = NH * N            # 588
NCORES = 8
IMGS = B // NCORES
WT = 8                # windows per tile (= one window row)
TW = WT * N           # 392
NTOK = HH * WW        # tokens per image
EPS = 1e-5
LDPAD = 16
LDW = LDPAD + J * WT + 8

BF = ml_dtypes.bfloat16


def _ap(t, offset, dims):
    tt = t.tensor if hasattr(t, "tensor") else t
    return bass.AP(tensor=tt, offset=offset, ap=[list(d) for d in dims])


def build():
    nc = bacc.Bacc("TRN2", target_bir_lowering=False, debug=False, num_devices=NCORES)
    x_d = nc.dram_tensor("x", [IMGS * NTOK, C], BF16, kind="ExternalInput")
    wv_d = nc.dram_tensor("wv_t", [128, 3 * C], BF16, kind="ExternalInput")
    wo_d = nc.dram_tensor("wo_t", [64, 18 * 128], BF16, kind="ExternalInput")
    w1_d = nc.dram_tensor("w1_t", [128, 3 * MLP], BF16, kind="ExternalInput")
    w2_d = nc.dram_tensor("w2_t", [128, 12 * C], BF16, kind="ExternalInput")
    yb_d = nc.dram_tensor("yb_t", [128, 3 * TW], F32, kind="ExternalInput")
    cn_d = nc.dram_tensor("cn_t", [128, 27], F32, kind="ExternalInput")
    on_d = nc.dram_tensor("ones_t", [128, 128], BF16, kind="ExternalInput")
    idb_d = nc.dram_tensor("identb_t", [128, 128], BF16, kind="ExternalInput")
    pm_d = nc.dram_tensor("perm_t", [WW, WW], BF16, kind="ExternalInput")
    out_d = nc.dram_tensor("out", [IMGS * NTOK, C], F32, kind="ExternalOutput")

    with tile.TileContext(nc) as tc, ExitStack() as ctx:
        wpool = ctx.enter_context(tc.tile_pool(name="w", bufs=1))
        stage_pool = ctx.enter_context(tc.tile_pool(name="stage", bufs=4))
        xt_pool = ctx.enter_context(tc.tile_pool(name="xt", bufs=4))
        vt_pool = ctx.enter_context(tc.tile_pool(name="vt", bufs=3))
        ld_pool = ctx.enter_context(tc.tile_pool(name="ld", bufs=2))
        y_pool = ctx.enter_context(tc.tile_pool(name="y", bufs=3))
        x2_pool = ctx.enter_context(tc.tile_pool(name="x2", bufs=4))
        h_pool = ctx.enter_context(tc.tile_pool(name="h", bufs=3))
        h2_pool = ctx.enter_context(tc.tile_pool(name="h2", bufs=3))
        oc_pool = ctx.enter_context(tc.tile_pool(name="oc", bufs=3))
        ot_pool = ctx.enter_context(tc.tile_pool(name="ot", bufs=3))
        sm_pool = ctx.enter_context(tc.tile_pool(name="sm", bufs=5))
        ps_t = ctx.enter_context(tc.tile_pool(name="pst", bufs=1, space="PSUM"))
        ps_a = ctx.enter_context(tc.tile_pool(name="psa", bufs=2, space="PSUM"))
        ps_m = ctx.enter_context(tc.tile_pool(name="psm", bufs=2, space="PSUM"))
        ps_s = ctx.enter_context(tc.tile_pool(name="pss", bufs=1, space="PSUM"))
        ps_o = ctx.enter_context(tc.tile_pool(name="pso", bufs=2, space="PSUM"))

        # ---------- one-time setup: weights straight from host layout ----------
        wv_b = wpool.tile([128, 3 * C], BF16)
        wo_b = wpool.tile([64, 18 * 128], BF16)
        w1_b = wpool.tile([128, 3 * MLP], BF16)
        w2_b = wpool.tile([128, 12 * C], BF16)
        yb_sb = wpool.tile([128, 3 * TW], F32)
        cn_sb = wpool.tile([128, 27], F32)
        ones_b = wpool.tile([128, 128], BF16)
        identb = wpool.tile([128, 128], BF16)
        perm56 = wpool.tile([WW, WW], BF16)
        eps_t = wpool.tile([128, 1], F32)
        zero_t = wpool.tile([128, 1], F32)
        nc.vector.memset(eps_t[:, :], EPS)
        nc.vector.memset(zero_t[:, :], 0.0)
        nc.sync.dma_start(perm56[:, :], pm_d[:, :])
        nc.sync.dma_start(wv_b[:, :], wv_d[:, :])
        nc.sync.dma_start(wo_b[:, :], wo_d[:, :])
        nc.sync.dma_start(w1_b[:, :], w1_d[:, :])
        nc.sync.dma_start(w2_b[:, :], w2_d[:, :])
        nc.sync.dma_start(yb_sb[:, :], yb_d[:, :])
        nc.sync.dma_start(cn_sb[:, :], cn_d[:, :])
        nc.sync.dma_start(ones_b[:, :], on_d[:, :])
        nc.sync.dma_start(identb[:, :], idb_d[:, :])
        # PE must observe the identity via a transpose once before the loop so
        # later transposes carry <=1 sync wait (bare LDWEIGHTS limitation).
        dmy = ps_t.tile([128, 392], BF16, tag="t")
        nc.tensor.transpose(dmy[0:WW, 0:WW], perm56[0:WW, 0:WW], perm56[0:WW, 0:WW])
        dmy2 = ps_t.tile([128, 392], BF16, tag="t")
        nc.tensor.transpose(dmy2[0:128, 0:128], identb[:, :], identb[:, :])

        # ---------- software-pipelined main loop ----------
        # tile index t = img*8 + wr; phases skewed across iterations so every
        # engine (PE especially) always has ready work.
        NT = IMGS * NWS
        st = {}   # per-tile live state

        def do_gather(t):
            img, wr = divmod(t, NWS)
            stage = stage_pool.tile([WW, WS * C], BF16, tag="stage")
            pst = stage[:, :].ap[0][0]
            soff = stage[:, :].offset
            rows = [(WS * wr + DISP, 0, WS)] if wr < NWS - 1 else \
                [(52, 0, 4), (0, 4, 3)]
            for (r0, tr0, ntr) in rows:
                src = _ap(x_d, (img * NTOK + r0 * WW) * C,
                          [[C, WW], [WW * C, ntr], [1, C]])
                dst = _ap(stage, soff + tr0 * C,
                          [[pst, WW], [C, ntr], [1, C]])
                nc.sync.dma_start(dst, src)
            st[t] = {"stage": stage}

        def do_p1(t):
            """transposes + V-proj + ld build"""
            d = st[t]
            stage = d.pop("stage")
            xt = xt_pool.tile([128, 3 * TW], BF16, tag="xt")
            for k in range(3):
                xps = ps_t.tile([128, 392], BF16, tag="t")
                pxp = xps[:, :].ap[0][0]
                xpo = xps[:, :].offset
                for tr in range(WS):
                    nc.tensor.transpose(
                        xps[:, WW * tr:WW * tr + WW],
                        stage[0:WW, tr * C + 128 * k:tr * C + 128 * k + 128],
                        perm56[0:WW, 0:WW])
                # token order everywhere downstream: col = 56*tr + 7*win + tc
                nc.vector.tensor_scalar(xt[:, TW * k:TW * k + TW], xps[:, :],
                                        cn_sb[:, 21 + k:22 + k], None, op0=OP.add)
            vt = vt_pool.tile([128, 3 * TW], BF16, tag="vt")
            pvt = vt[:, :].ap[0][0]
            vtoff = vt[:, :].offset
            for kv in range(3):
                vps = ps_a.tile([128, TW], F32, tag="a")
                for k in range(3):
                    nc.tensor.matmul(vps[:, :],
                                     wv_b[:, C * k + 128 * kv:C * k + 128 * kv + 128],
                                     xt[:, TW * k:TW * k + TW],
                                     start=(k == 0), stop=(k == 2))
                pps = vps[:, :].ap[0][0]
                nc.scalar.activation(
                    _ap(vt, vtoff + N * kv,
                        [[pvt, 128], [WS, WS], [3 * N, WT], [1, WS]]),
                    _ap(vps, vps[:, :].offset,
                        [[pps, 128], [WW, WS], [WS, WT], [1, WS]]),
                    AF.Identity, bias=zero_t[:, :], scale=1.0)
            ld = ld_pool.tile([64, LDW], BF16, tag="ld")
            pld = ld[:, :].ap[0][0]
            ldoff = ld[:, :].offset
            for s4 in range(4):
                for q in range(2):
                    src = _ap(vt, vtoff + (32 * s4) * pvt,
                              [[pvt, 32], [N, 24], [1, N]])
                    dst = _ap(ld, ldoff + (32 * q) * pld + LDPAD + N * s4 - 6 * q,
                              [[pld, 32], [4 * N, 24], [1, N]])
                    nc.sync.dma_start(dst, src)
            d["xt"] = xt
            d["ld"] = (ld, pld, ldoff)

        def do_p2a(t):
            """out-proj + ybias evac + ysq"""
            d = st[t]
            ld, pld, ldoff = d.pop("ld")
            y = y_pool.tile([128, 3 * TW], BF16, tag="y")
            for kj in range(3):
                yps = ps_o.tile([128, TW], F32, tag="o")
                for r in range(6):
                    rhs = _ap(ld, ldoff + LDPAD + r,
                              [[pld, 64], [84, WS], [J, WT], [12, WS]])
                    nc.tensor.matmul(yps[:, :],
                                     wo_b[:, (r * 3 + kj) * 128:(r * 3 + kj) * 128 + 128],
                                     rhs, start=(r == 0), stop=(r == 5))
                nc.vector.tensor_add(y[:, TW * kj:TW * kj + TW], yps[:, :],
                                     yb_sb[:, TW * kj:TW * kj + TW])
            ysq = sm_pool.tile([128, 3 * TW], BF16, tag="ysq")
            for k in range(3):
                nc.gpsimd.tensor_mul(ysq[:, TW * k:TW * k + TW],
                                     y[:, TW * k:TW * k + TW], y[:, TW * k:TW * k + TW])
            d["y"] = y
            d["ysq"] = ysq

        def _ln(y, ysq, gcol, skip, out):
            """matmul sums -> stats -> apply; out = (y-mu)*rst*g + skip."""
            s1 = ps_s.tile([128, TW], F32, tag="s")
            s2 = ps_s.tile([128, TW], F32, tag="s")
            for k in range(3):
                nc.tensor.matmul(s1[:, :], ones_b[:, :], y[:, TW * k:TW * k + TW],
                                 start=(k == 0), stop=(k == 2))
                nc.tensor.matmul(s2[:, :], ones_b[:, :], ysq[:, TW * k:TW * k + TW],
                                 start=(k == 0), stop=(k == 2))
            mu = sm_pool.tile([128, TW], F32, tag="mu")
            m2 = sm_pool.tile([128, TW], F32, tag="m2")
            veps = sm_pool.tile([128, TW], F32, tag="veps")
            rst = sm_pool.tile([128, TW], F32, tag="rst")
            nc.vector.tensor_copy(mu[:, :], s1[:, :])
            nc.gpsimd.tensor_mul(m2[:, :], mu[:, :], mu[:, :])
            nc.vector.tensor_sub(veps[:, :], s2[:, :], m2[:, :])
            # std = sqrt(var + eps) on Act (eps folded into the bias), then
            # 1/std via the fast custom-DVE reciprocal (~51 ULP, plenty here)
            nc.scalar.activation(veps[:, :], veps[:, :], AF.Sqrt,
                                 bias=eps_t[:, :], scale=1.0)
            nc.vector.reciprocal_approx_fast(rst[:, :], veps[:, :])
            for k in range(3):
                u = sm_pool.tile([128, TW], BF16, tag="u")
                v = sm_pool.tile([128, TW], BF16, tag="v")
                nc.vector.tensor_sub(u[:, :], y[:, TW * k:TW * k + TW], mu[:, :])
                nc.gpsimd.tensor_mul(v[:, :], u[:, :], rst[:, :])
                nc.vector.scalar_tensor_tensor(out[:, TW * k:TW * k + TW],
                                               v[:, :],
                                               cn_sb[:, gcol + k:gcol + k + 1],
                                               skip[:, TW * k:TW * k + TW],
                                               op0=OP.mult, op1=OP.add)

        def do_p2b(t):
            d = st[t]
            x2 = x2_pool.tile([128, 3 * TW], BF16, tag="x2")
            _ln(d.pop("y"), d.pop("ysq"), 15, d.pop("xt"), x2)
            d["x2"] = x2

        def do_p3(t):
            d = st[t]
            x2 = d["x2"]
            hsb = h_pool.tile([128, 12 * TW], BF16, tag="h")
            for m in range(12):
                hps = ps_m.tile([128, TW], F32, tag="m")
                for k in range(3):
                    nc.tensor.matmul(hps[:, :],
                                     w1_b[:, MLP * k + 128 * m:MLP * k + 128 * m + 128],
                                     x2[:, TW * k:TW * k + TW],
                                     start=(k == 0), stop=(k == 2))
                nc.scalar.activation(hsb[:, TW * m:TW * m + TW], hps[:, :],
                                     AF.Gelu, bias=cn_sb[:, m:m + 1], scale=1.0)
            d["hsb"] = hsb

        def do_p4a(t):
            d = st[t]
            hsb = d.pop("hsb")
            h2 = h2_pool.tile([128, 3 * TW], BF16, tag="h2")
            for kj in range(3):
                h2ps = ps_a.tile([128, TW], F32, tag="a")
                for k2 in range(12):
                    nc.tensor.matmul(h2ps[:, :],
                                     w2_b[:, C * k2 + 128 * kj:C * k2 + 128 * kj + 128],
                                     hsb[:, TW * k2:TW * k2 + TW],
                                     start=(k2 == 0), stop=(k2 == 11))
                nc.vector.tensor_scalar(h2[:, TW * kj:TW * kj + TW], h2ps[:, :],
                                        cn_sb[:, 12 + kj:13 + kj], None, op0=OP.add)
            hsq = sm_pool.tile([128, 3 * TW], BF16, tag="ysq")
            for k in range(3):
                nc.gpsimd.tensor_mul(hsq[:, TW * k:TW * k + TW],
                                     h2[:, TW * k:TW * k + TW], h2[:, TW * k:TW * k + TW])
            d["h2"] = h2
            d["hsq"] = hsq

        def do_p4b(t):
            d = st[t]
            ocm = oc_pool.tile([128, 3 * TW], BF16, tag="oc")
            _ln(d.pop("h2"), d.pop("hsq"), 18, d.pop("x2"), ocm)
            d["ocm"] = ocm

        def do_p5(t):
            d = st.pop(t)
            img, wr = divmod(t, NWS)
            ocm = d["ocm"]
            poc = ocm[:, :].ap[0][0]
            oco = ocm[:, :].offset
            # paired: otr[56*(tr%2)+x, 384*(tr//2)+c]; 12 transposes not 21
            otr = ot_pool.tile([112, 4 * C], BF16, tag="ot")
            pot = otr[:, :].ap[0][0]
            poff = otr[:, :].offset
            for p in range(4):
                prow = 112 if p < 3 else 56
                ops_t = ps_t.tile([128, 392], BF16, tag="t")
                for k in range(3):
                    nc.tensor.transpose(
                        ops_t[0:prow, 128 * k:128 * k + 128],
                        ocm[:, TW * k + 112 * p:TW * k + 112 * p + prow],
                        identb[:, :])
                nc.vector.tensor_copy(otr[0:prow, p * C:p * C + C],
                                      ops_t[0:prow, 0:C])

            def scat(prow0, npair, r0, odd):
                base = poff + (56 * odd) * pot + 384 * prow0
                dst0 = (img * NTOK + r0 * WW) * C
                src = _ap(otr, base, [[pot, 53], [C, npair], [1, C]])
                dst = _ap(out_d, dst0 + DISP * C,
                          [[C, 53], [2 * WW * C, npair], [1, C]])
                nc.gpsimd.dma_start(dst, src)
                src = _ap(otr, base + 53 * pot, [[pot, 3], [C, npair], [1, C]])
                dst = _ap(out_d, dst0, [[C, 3], [2 * WW * C, npair], [1, C]])
                nc.gpsimd.dma_start(dst, src)

            if wr < NWS - 1:
                r0 = WS * wr + DISP
                scat(0, 4, r0, 0)
                scat(0, 3, r0 + 1, 1)
            else:
                scat(0, 2, 52, 0)
                scat(0, 2, 53, 1)
                scat(2, 2, 0, 0)
                scat(2, 1, 1, 1)

        for i in range(NT + 7):
            if i < NT:
                do_gather(i)
            if 0 <= i - 2 < NT:
                do_p2a(i - 2)
            if 0 <= i - 1 < NT:
                do_p1(i - 1)
            if 0 <= i - 5 < NT:
                do_p4a(i - 5)
            if 0 <= i - 7 < NT:
                do_p5(i - 7)
            if 0 <= i - 3 < NT:
                do_p2b(i - 3)
            if 0 <= i - 4 < NT:
                do_p3(i - 4)
            if 0 <= i - 6 < NT:
                do_p4b(i - 6)
    nc.compile()
    return nc


def _to_bf(a):
    return np.ascontiguousarray(a.astype(BF))


def prep_inputs(inputs):
    """Host-side weight/bias reformatting (layout only + tiny bias algebra)."""
    f = {k: np.asarray(v, dtype=np.float32) for k, v in inputs.items()}
    qkv_w, qkv_b = f["qkv_w"], f["qkv_b"]
    out_w, out_b = f["out_w"], f["out_b"]
    w1, b1, w2, b2 = f["mlp_w1"], f["mlp_b1"], f["mlp_w2"], f["mlp_b2"]
    g1, be1 = f["norm1_g"], f["norm1_b"]
    g2, be2 = f["norm2_g"], f["norm2_b"]

    wv = qkv_w[:, 2 * C:3 * C]
    bv = qkv_b[2 * C:3 * C]

    wv_t = np.zeros((128, 3 * C), np.float32)
    for k in range(3):
        wv_t[:, C * k:C * k + C] = wv[128 * k:128 * k + 128, :]

    wo_t = np.zeros((64, 18 * 128), np.float32)
    for r in range(6):
        for q in range(2):
            for kj in range(3):
                wo_t[32 * q:32 * q + 32, (r * 3 + kj) * 128:(r * 3 + kj) * 128 + 128] = \
                    out_w[32 * (r + 6 * q):32 * (r + 6 * q) + 32, 128 * kj:128 * kj + 128]

    w1_t = np.zeros((128, 3 * MLP), np.float32)
    for k in range(3):
        w1_t[:, MLP * k:MLP * k + MLP] = w1[128 * k:128 * k + 128, :]
    w2_t = np.zeros((128, 12 * C), np.float32)
    for k2 in range(12):
        w2_t[:, C * k2:C * k2 + C] = w2[128 * k2:128 * k2 + 128, :]

    # ybias[n2, :] = P(bv)[n2] @ out_w + out_b
    n2 = np.arange(N)[:, None]
    co = np.arange(12)[None, :]
    hh = (12 * n2 + co) // N                    # (49, 12)
    pb = bv.reshape(12, 32)[hh]                 # (49, 12, 32)
    ybias = pb.reshape(N, C) @ out_w + out_b    # (49, 384)
    yb_t = np.zeros((128, 3 * TW), np.float32)
    tr_ = np.arange(WS)[:, None, None]
    wn_ = np.arange(WT)[None, :, None]
    tc_ = np.arange(WS)[None, None, :]
    pc = (WS * tr_ + tc_ + 0 * wn_).reshape(TW)   # n2 per permuted column
    for kj in range(3):
        blk = ybias[:, 128 * kj:128 * kj + 128].T          # (128, 49)
        yb_t[:, TW * kj:TW * kj + TW] = blk[:, pc]

    b1p = b1 - be2 @ w1                          # mlp bias corrected for +be2 on x2
    be12 = be1 + be2
    cn_t = np.zeros((128, 27), np.float32)
    for m in range(12):
        cn_t[:, m] = b1p[128 * m:128 * m + 128]
    for k in range(3):
        cn_t[:, 12 + k] = b2[128 * k:128 * k + 128]
        cn_t[:, 15 + k] = g1[128 * k:128 * k + 128]
        cn_t[:, 18 + k] = g2[128 * k:128 * k + 128]
        cn_t[:, 21 + k] = be12[128 * k:128 * k + 128]

    ones_t = np.full((128, 128), 1.0 / C, np.float32)
    ident = np.eye(128, dtype=np.float32)

    # column-roll permutation for the input transposes
    perm = np.zeros((WW, WW), np.float32)
    jj = np.arange(WW)
    perm[(jj + DISP) % WW, jj] = 1.0

    return {
        "wv_t": _to_bf(wv_t), "wo_t": _to_bf(wo_t),
        "w1_t": _to_bf(w1_t), "w2_t": _to_bf(w2_t),
        "yb_t": np.ascontiguousarray(yb_t), "cn_t": np.ascontiguousarray(cn_t),
        "ones_t": _to_bf(ones_t), "identb_t": _to_bf(ident),
        "perm_t": _to_bf(perm),
    }


_CACHE = {}


def kernel(**inputs):
    if "nc" not in _CACHE:
        _CACHE["nc"] = build()
    nc = _CACHE["nc"]
    x = np.asarray(inputs["x"], dtype=np.float32).astype(BF)
    base = prep_inputs(inputs)
    in_maps = []
    for c in range(NCORES):
        m = dict(base)
        m["x"] = np.ascontiguousarray(
            x[IMGS * c:IMGS * (c + 1)].reshape(IMGS * NTOK, C))
        in_maps.append(m)
    import os
    trace = bool(int(os.environ.get("KERNEL_TRACE", "0")))
    res = run_bass_kernel_spmd(nc, in_maps, core_ids=list(range(NCORES)),
                               trace=trace)
    _CACHE["last_res"] = res
    out = np.concatenate([r["out"].reshape(IMGS, HH, WW, C) for r in res.results],
                         axis=0)
    return out



# revision 32
# speedup vs baseline: 1.1887x; 1.0263x over previous
"""Trainium2 Bass kernel v2 for nn_Encoder_59219009077683 (Swin-style block).

Math shortcut (from baseline): softmax row-sums are 1, so attention output = v
and the whole block is: window-gather -> V-proj -> fixed permutation (L_d
layout) -> out-proj -> LN1+skip -> MLP -> LN2+skip -> window-scatter.

v2 structural changes vs baseline:
  - gather/scatter via ONE SWDGE indirect DMA per tile (host-built index
    table; the cyclic roll/wrap is baked into the indices)
  - bf16 for all SBUF intermediates and matmuls (tolerance is 2e-2)
  - K=64 out-projection: L_d stored twice (quadrant q holds T[d, c+6q]) so
    the 12 K=32 chunk-matmuls become 6 K=64 matmuls per output block
  - host-folded biases: v-bias+out-bias -> ybias table added at y-evac;
    norm1_b+norm2_b added at x-transpose evac; mlp_b1 corrected by
    -W1^T norm2_b so the norm2 beta rides the x2 skip path
  - rsqrt via tensor_scalar(pow -0.5) on DVE: scalar engine runs only
    Gelu/Identity -> a single activation-table load
  - elementwise spread across DVE / Act / GpSimd
"""
import numpy as np
from contextlib import ExitStack

import concourse.bass as bass
import concourse.bacc as bacc
import concourse.tile as tile
from concourse import mybir
from concourse.bass_utils import run_bass_kernel_spmd, ml_dtypes

F32 = mybir.dt.float32
BF16 = mybir.dt.bfloat16
U32 = mybir.dt.uint32
AF = mybir.ActivationFunctionType
OP = mybir.AluOpType

B, HH, WW, C = 32, 56, 56, 384
NH, HD, WS, DISP, MLP = 12, 32, 7, 3, 1536
NWS = 8
N = 49
J = NH * N            # 588
NCORES = 8
IMGS = B // NCORES
WT = 8                # windows per tile (= one window row)
TW = WT * N           # 392
NTOK = HH * WW        # tokens per image
EPS = 1e-5
LDPAD = 16
LDW = LDPAD + J * WT + 8

BF = ml_dtypes.bfloat16


def _ap(t, offset, dims):
    tt = t.tensor if hasattr(t, "tensor") else t
    return bass.AP(tensor=tt, offset=offset, ap=[list(d) for d in dims])


def build():
    nc = bacc.Bacc("TRN2", target_bir_lowering=False, debug=False, num_devices=NCORES)
    x_d = nc.dram_tensor("x", [IMGS * NTOK, C], BF16, kind="ExternalInput")
    wv_d = nc.dram_tensor("wv_t", [128, 3 * C], BF16, kind="ExternalInput")
    wo_d = nc.dram_tensor("wo_t", [64, 18 * 128], BF16, kind="ExternalInput")
    w1_d = nc.dram_tensor("w1_t", [128, 3 * MLP], BF16, kind="ExternalInput")
    w2_d = nc.dram_tensor("w2_t", [128, 12 * C], BF16, kind="ExternalInput")
    yb_d = nc.dram_tensor("yb_t", [128, 3 * TW], F32, kind="ExternalInput")
    cn_d = nc.dram_tensor("cn_t", [128, 27], F32, kind="ExternalInput")
    on_d = nc.dram_tensor("ones_t", [128, 128], BF16, kind="ExternalInput")
    idb_d = nc.dram_tensor("identb_t", [128, 128], BF16, kind="ExternalInput")
    pm_d = nc.dram_tensor("perm_t", [WW, WW], BF16, kind="ExternalInput")
    out_d = nc.dram_tensor("out", [IMGS * NTOK, C], F32, kind="ExternalOutput")

    with tile.TileContext(nc) as tc, ExitStack() as ctx:
        wpool = ctx.enter_context(tc.tile_pool(name="w", bufs=1))
        stage_pool = ctx.enter_context(tc.tile_pool(name="stage", bufs=4))
        xt_pool = ctx.enter_context(tc.tile_pool(name="xt", bufs=4))
        vt_pool = ctx.enter_context(tc.tile_pool(name="vt", bufs=3))
        ld_pool = ctx.enter_context(tc.tile_pool(name="ld", bufs=2))
        y_pool = ctx.enter_context(tc.tile_pool(name="y", bufs=3))
        x2_pool = ctx.enter_context(tc.tile_pool(name="x2", bufs=4))
        h_pool = ctx.enter_context(tc.tile_pool(name="h", bufs=3))
        h2_pool = ctx.enter_context(tc.tile_pool(name="h2", bufs=3))
        oc_pool = ctx.enter_context(tc.tile_pool(name="oc", bufs=3))
        ot_pool = ctx.enter_context(tc.tile_pool(name="ot", bufs=3))
        sm_pool = ctx.enter_context(tc.tile_pool(name="sm", bufs=5))
        ps_t = ctx.enter_context(tc.tile_pool(name="pst", bufs=1, space="PSUM"))
        ps_a = ctx.enter_context(tc.tile_pool(name="psa", bufs=2, space="PSUM"))
        ps_m = ctx.enter_context(tc.tile_pool(name="psm", bufs=2, space="PSUM"))
        ps_s = ctx.enter_context(tc.tile_pool(name="pss", bufs=1, space="PSUM"))
        ps_o = ctx.enter_context(tc.tile_pool(name="pso", bufs=2, space="PSUM"))

        # ---------- one-time setup: weights straight from host layout ----------
        wv_b = wpool.tile([128, 3 * C], BF16)
        wo_b = wpool.tile([64, 18 * 128], BF16)
        w1_b = wpool.tile([128, 3 * MLP], BF16)
        w2_b = wpool.tile([128, 12 * C], BF16)
        yb_sb = wpool.tile([128, 3 * TW], F32)
        cn_sb = wpool.tile([128, 27], F32)
        ones_b = wpool.tile([128, 128], BF16)
        identb = wpool.tile([128, 128], BF16)
        perm56 = wpool.tile([WW, WW], BF16)
        eps_t = wpool.tile([128, 1], F32)
        zero_t = wpool.tile([128, 1], F32)
        nc.vector.memset(eps_t[:, :], EPS)
        nc.vector.memset(zero_t[:, :], 0.0)
        nc.sync.dma_start(perm56[:, :], pm_d[:, :])
        nc.sync.dma_start(wv_b[:, :], wv_d[:, :])
        nc.sync.dma_start(wo_b[:, :], wo_d[:, :])
        nc.sync.dma_start(w1_b[:, :], w1_d[:, :])
        nc.sync.dma_start(w2_b[:, :], w2_d[:, :])
        nc.sync.dma_start(yb_sb[:, :], yb_d[:, :])
        nc.sync.dma_start(cn_sb[:, :], cn_d[:, :])
        nc.sync.dma_start(ones_b[:, :], on_d[:, :])
        nc.sync.dma_start(identb[:, :], idb_d[:, :])
        # PE must observe the identity via a transpose once before the loop so
        # later transposes carry <=1 sync wait (bare LDWEIGHTS limitation).
        dmy = ps_t.tile([128, 392], BF16, tag="t")
        nc.tensor.transpose(dmy[0:WW, 0:WW], perm56[0:WW, 0:WW], perm56[0:WW, 0:WW])
        dmy2 = ps_t.tile([128, 392], BF16, tag="t")
        nc.tensor.transpose(dmy2[0:128, 0:128], identb[:, :], identb[:, :])

        # ---------- software-pipelined main loop ----------
        # tile index t = img*8 + wr; phases skewed across iterations so every
        # engine (PE especially) always has ready work.
        NT = IMGS * NWS
        st = {}   # per-tile live state

        def do_gather(t):
            img, wr = divmod(t, NWS)
            stage = stage_pool.tile([WW, WS * C], BF16, tag="stage")
            pst = stage[:, :].ap[0][0]
            soff = stage[:, :].offset
            rows = [(WS * wr + DISP, 0, WS)] if wr < NWS - 1 else \
                [(52, 0, 4), (0, 4, 3)]
            for (r0, tr0, ntr) in rows:
                src = _ap(x_d, (img * NTOK + r0 * WW) * C,
                          [[C, WW], [WW * C, ntr], [1, C]])
                dst = _ap(stage, soff + tr0 * C,
                          [[pst, WW], [C, ntr], [1, C]])
                nc.sync.dma_start(dst, src)
            st[t] = {"stage": stage}

        def do_p1(t):
            """transposes + V-proj + ld build"""
            d = st[t]
            stage = d.pop("stage")
            xt = xt_pool.tile([128, 3 * TW], BF16, tag="xt")
            for k in range(3):
                xps = ps_t.tile([128, 392], BF16, tag="t")
                pxp = xps[:, :].ap[0][0]
                xpo = xps[:, :].offset
                for tr in range(WS):
                    nc.tensor.transpose(
                        xps[:, WW * tr:WW * tr + WW],
                        stage[0:WW, tr * C + 128 * k:tr * C + 128 * k + 128],
                        perm56[0:WW, 0:WW])
                # token order everywhere downstream: col = 56*tr + 7*win + tc
                nc.vector.tensor_scalar(xt[:, TW * k:TW * k + TW], xps[:, :],
                                        cn_sb[:, 21 + k:22 + k], None, op0=OP.add)
            vt = vt_pool.tile([128, 3 * TW], BF16, tag="vt")
            pvt = vt[:, :].ap[0][0]
            vtoff = vt[:, :].offset
            for kv in range(3):
                vps = ps_a.tile([128, TW], F32, tag="a")
                for k in range(3):
                    nc.tensor.matmul(vps[:, :],
                                     wv_b[:, C * k + 128 * kv:C * k + 128 * kv + 128],
                                     xt[:, TW * k:TW * k + TW],
                                     start=(k == 0), stop=(k == 2))
                pps = vps[:, :].ap[0][0]
                nc.scalar.activation(
                    _ap(vt, vtoff + N * kv,
                        [[pvt, 128], [WS, WS], [3 * N, WT], [1, WS]]),
                    _ap(vps, vps[:, :].offset,
                        [[pps, 128], [WW, WS], [WS, WT], [1, WS]]),
                    AF.Identity, bias=zero_t[:, :], scale=1.0)
            ld = ld_pool.tile([64, LDW], BF16, tag="ld")
            pld = ld[:, :].ap[0][0]
            ldoff = ld[:, :].offset
            for s4 in range(4):
                for q in range(2):
                    src = _ap(vt, vtoff + (32 * s4) * pvt,
                              [[pvt, 32], [N, 24], [1, N]])
                    dst = _ap(ld, ldoff + (32 * q) * pld + LDPAD + N * s4 - 6 * q,
                              [[pld, 32], [4 * N, 24], [1, N]])
                    nc.sync.dma_start(dst, src)
            d["xt"] = xt
            d["ld"] = (ld, pld, ldoff)

        def do_p2a(t):
            """out-proj + ybias evac + ysq"""
            d = st[t]
            ld, pld, ldoff = d.pop("ld")
            y = y_pool.tile([128, 3 * TW], BF16, tag="y")
            for kj in range(3):
                yps = ps_o.tile([128, TW], F32, tag="o")
                for r in range(6):
                    rhs = _ap(ld, ldoff + LDPAD + r,
                              [[pld, 64], [84, WS], [J, WT], [12, WS]])
                    nc.tensor.matmul(yps[:, :],
                                     wo_b[:, (r * 3 + kj) * 128:(r * 3 + kj) * 128 + 128],
                                     rhs, start=(r == 0), stop=(r == 5))
                nc.vector.tensor_add(y[:, TW * kj:TW * kj + TW], yps[:, :],
                                     yb_sb[:, TW * kj:TW * kj + TW])
            ysq = sm_pool.tile([128, 3 * TW], BF16, tag="ysq")
            for k in range(3):
                nc.vector.tensor_mul(ysq[:, TW * k:TW * k + TW],
                                     y[:, TW * k:TW * k + TW], y[:, TW * k:TW * k + TW])
            d["y"] = y
            d["ysq"] = ysq

        def _ln(y, ysq, gcol, skip, out):
            """matmul sums -> stats -> apply; out = (y-mu)*rst*g + skip."""
            s1 = ps_s.tile([128, TW], F32, tag="s")
            s2 = ps_s.tile([128, TW], F32, tag="s")
            for k in range(3):
                nc.tensor.matmul(s1[:, :], ones_b[:, :], y[:, TW * k:TW * k + TW],
                                 start=(k == 0), stop=(k == 2))
                nc.tensor.matmul(s2[:, :], ones_b[:, :], ysq[:, TW * k:TW * k + TW],
                                 start=(k == 0), stop=(k == 2))
            mu = sm_pool.tile([128, TW], F32, tag="mu")
            m2 = sm_pool.tile([128, TW], F32, tag="m2")
            veps = sm_pool.tile([128, TW], F32, tag="veps")
            rst = sm_pool.tile([128, TW], F32, tag="rst")
            nc.vector.tensor_copy(mu[:, :], s1[:, :])
            nc.scalar.square(m2[:, :], mu[:, :])
            nc.vector.tensor_sub(veps[:, :], s2[:, :], m2[:, :])
            # std = sqrt(var + eps) on Act (eps folded into the bias), then
            # 1/std via the fast custom-DVE reciprocal (~51 ULP, plenty here)
            nc.scalar.activation(veps[:, :], veps[:, :], AF.Sqrt,
                                 bias=eps_t[:, :], scale=1.0)
            nc.vector.reciprocal_approx_fast(rst[:, :], veps[:, :])
            for k in range(3):
                u = sm_pool.tile([128, TW], BF16, tag="u")
                v = sm_pool.tile([128, TW], BF16, tag="v")
                nc.vector.tensor_sub(u[:, :], y[:, TW * k:TW * k + TW], mu[:, :])
                nc.gpsimd.tensor_mul(v[:, :], u[:, :], rst[:, :])
                nc.vector.scalar_tensor_tensor(out[:, TW * k:TW * k + TW],
                                               v[:, :],
                                               cn_sb[:, gcol + k:gcol + k + 1],
                                               skip[:, TW * k:TW * k + TW],
                                               op0=OP.mult, op1=OP.add)

        def do_p2b(t):
            d = st[t]
            x2 = x2_pool.tile([128, 3 * TW], BF16, tag="x2")
            _ln(d.pop("y"), d.pop("ysq"), 15, d.pop("xt"), x2)
            d["x2"] = x2

        def do_p3(t):
            d = st[t]
            x2 = d["x2"]
            hsb = h_pool.tile([128, 12 * TW], BF16, tag="h")
            for m in range(12):
                hps = ps_m.tile([128, TW], F32, tag="m")
                for k in range(3):
                    nc.tensor.matmul(hps[:, :],
                                     w1_b[:, MLP * k + 128 * m:MLP * k + 128 * m + 128],
                                     x2[:, TW * k:TW * k + TW],
                                     start=(k == 0), stop=(k == 2))
                nc.scalar.activation(hsb[:, TW * m:TW * m + TW], hps[:, :],
                                     AF.Gelu, bias=cn_sb[:, m:m + 1], scale=1.0)
            d["hsb"] = hsb

        def do_p4a(t):
            d = st[t]
            hsb = d.pop("hsb")
            h2 = h2_pool.tile([128, 3 * TW], BF16, tag="h2")
            for kj in range(3):
                h2ps = ps_a.tile([128, TW], F32, tag="a")
                for k2 in range(12):
                    nc.tensor.matmul(h2ps[:, :],
                                     w2_b[:, C * k2 + 128 * kj:C * k2 + 128 * kj + 128],
                                     hsb[:, TW * k2:TW * k2 + TW],
                                     start=(k2 == 0), stop=(k2 == 11))
                nc.vector.tensor_scalar(h2[:, TW * kj:TW * kj + TW], h2ps[:, :],
                                        cn_sb[:, 12 + kj:13 + kj], None, op0=OP.add)
            hsq = sm_pool.tile([128, 3 * TW], BF16, tag="ysq")
            for k in range(3):
                nc.vector.tensor_mul(hsq[:, TW * k:TW * k + TW],
                                     h2[:, TW * k:TW * k + TW], h2[:, TW * k:TW * k + TW])
            d["h2"] = h2
            d["hsq"] = hsq

        def do_p4b(t):
            d = st[t]
            ocm = oc_pool.tile([128, 3 * TW], BF16, tag="oc")
            _ln(d.pop("h2"), d.pop("hsq"), 18, d.pop("x2"), ocm)
            d["ocm"] = ocm

        def do_p5(t):
            d = st.pop(t)
            img, wr = divmod(t, NWS)
            ocm = d["ocm"]
            poc = ocm[:, :].ap[0][0]
            oco = ocm[:, :].offset
            # paired: otr[56*(tr%2)+x, 384*(tr//2)+c]; 12 transposes not 21
            otr = ot_pool.tile([112, 4 * C], BF16, tag="ot")
            pot = otr[:, :].ap[0][0]
            poff = otr[:, :].offset
            for p in range(4):
                prow = 112 if p < 3 else 56
                ops_t = ps_t.tile([128, 392], BF16, tag="t")
                for k in range(3):
                    nc.tensor.transpose(
                        ops_t[0:prow, 128 * k:128 * k + 128],
                        ocm[:, TW * k + 112 * p:TW * k + 112 * p + prow],
                        identb[:, :])
                nc.vector.tensor_copy(otr[0:prow, p * C:p * C + C],
                                      ops_t[0:prow, 0:C])

            def scat(prow0, npair, r0, odd):
                base = poff + (56 * odd) * pot + 384 * prow0
                dst0 = (img * NTOK + r0 * WW) * C
                src = _ap(otr, base, [[pot, 53], [C, npair], [1, C]])
                dst = _ap(out_d, dst0 + DISP * C,
                          [[C, 53], [2 * WW * C, npair], [1, C]])
                nc.gpsimd.dma_start(dst, src)
                src = _ap(otr, base + 53 * pot, [[pot, 3], [C, npair], [1, C]])
                dst = _ap(out_d, dst0, [[C, 3], [2 * WW * C, npair], [1, C]])
                nc.gpsimd.dma_start(dst, src)

            if wr < NWS - 1:
                r0 = WS * wr + DISP
                scat(0, 4, r0, 0)
                scat(0, 3, r0 + 1, 1)
            else:
                scat(0, 2, 52, 0)
                scat(0, 2, 53, 1)
                scat(2, 2, 0, 0)
                scat(2, 1, 1, 1)

        for i in range(NT + 7):
            if i < NT:
                do_gather(i)
            if 0 <= i - 2 < NT:
                do_p2a(i - 2)
            if 0 <= i - 1 < NT:
                do_p1(i - 1)
            if 0 <= i - 5 < NT:
                do_p4a(i - 5)
            if 0 <= i - 7 < NT:
                do_p5(i - 7)
            if 0 <= i - 3 < NT:
                do_p2b(i - 3)
            if 0 <= i - 4 < NT:
                do_p3(i - 4)
            if 0 <= i - 6 < NT:
                do_p4b(i - 6)
    nc.compile()
    return nc


def _to_bf(a):
    return np.ascontiguousarray(a.astype(BF))


def prep_inputs(inputs):
    """Host-side weight/bias reformatting (layout only + tiny bias algebra)."""
    f = {k: np.asarray(v, dtype=np.float32) for k, v in inputs.items()}
    qkv_w, qkv_b = f["qkv_w"], f["qkv_b"]
    out_w, out_b = f["out_w"], f["out_b"]
    w1, b1, w2, b2 = f["mlp_w1"], f["mlp_b1"], f["mlp_w2"], f["mlp_b2"]
    g1, be1 = f["norm1_g"], f["norm1_b"]
    g2, be2 = f["norm2_g"], f["norm2_b"]

    wv = qkv_w[:, 2 * C:3 * C]
    bv = qkv_b[2 * C:3 * C]

    wv_t = np.zeros((128, 3 * C), np.float32)
    for k in range(3):
        wv_t[:, C * k:C * k + C] = wv[128 * k:128 * k + 128, :]

    wo_t = np.zeros((64, 18 * 128), np.float32)
    for r in range(6):
        for q in range(2):
            for kj in range(3):
                wo_t[32 * q:32 * q + 32, (r * 3 + kj) * 128:(r * 3 + kj) * 128 + 128] = \
                    out_w[32 * (r + 6 * q):32 * (r + 6 * q) + 32, 128 * kj:128 * kj + 128]

    w1_t = np.zeros((128, 3 * MLP), np.float32)
    for k in range(3):
        w1_t[:, MLP * k:MLP * k + MLP] = w1[128 * k:128 * k + 128, :]
    w2_t = np.zeros((128, 12 * C), np.float32)
    for k2 in range(12):
        w2_t[:, C * k2:C * k2 + C] = w2[128 * k2:128 * k2 + 128, :]

    # ybias[n2, :] = P(bv)[n2] @ out_w + out_b
    n2 = np.arange(N)[:, None]
    co = np.arange(12)[None, :]
    hh = (12 * n2 + co) // N                    # (49, 12)
    pb = bv.reshape(12, 32)[hh]                 # (49, 12, 32)
    ybias = pb.reshape(N, C) @ out_w + out_b    # (49, 384)
    yb_t = np.zeros((128, 3 * TW), np.float32)
    tr_ = np.arange(WS)[:, None, None]
    wn_ = np.arange(WT)[None, :, None]
    tc_ = np.arange(WS)[None, None, :]
    pc = (WS * tr_ + tc_ + 0 * wn_).reshape(TW)   # n2 per permuted column
    for kj in range(3):
        blk = ybias[:, 128 * kj:128 * kj + 128].T          # (128, 49)
        yb_t[:, TW * kj:TW * kj + TW] = blk[:, pc]

    b1p = b1 - be2 @ w1                          # mlp bias corrected for +be2 on x2
    be12 = be1 + be2
    cn_t = np.zeros((128, 27), np.float32)
    for m in range(12):
        cn_t[:, m] = b1p[128 * m:128 * m + 128]
    for k in range(3):
        cn_t[:, 12 + k] = b2[128 * k:128 * k + 128]
        cn_t[:, 15 + k] = g1[128 * k:128 * k + 128]
        cn_t[:, 18 + k] = g2[128 * k:128 * k + 128]
        cn_t[:, 21 + k] = be12[128 * k:128 * k + 128]

    ones_t = np.full((128, 128), 1.0 / C, np.float32)
    ident = np.eye(128, dtype=np.float32)

    # column-roll permutation for the input transposes
    perm = np.zeros((WW, WW), np.float32)
    jj = np.arange(WW)
    perm[(jj + DISP) % WW, jj] = 1.0

    return {
        "wv_t": _to_bf(wv_t), "wo_t": _to_bf(wo_t),
        "w1_t": _to_bf(w1_t), "w2_t": _to_bf(w2_t),
        "yb_t": np.ascontiguousarray(yb_t), "cn_t": np.ascontiguousarray(cn_t),
        "ones_t": _to_bf(ones_t), "identb_t": _to_bf(ident),
        "perm_t": _to_bf(perm),
    }


_CACHE = {}


def kernel(**inputs):
    if "nc" not in _CACHE:
        _CACHE["nc"] = build()
    nc = _CACHE["nc"]
    x = np.asarray(inputs["x"], dtype=np.float32).astype(BF)
    base = prep_inputs(inputs)
    in_maps = []
    for c in range(NCORES):
        m = dict(base)
        m["x"] = np.ascontiguousarray(
            x[IMGS * c:IMGS * (c + 1)].reshape(IMGS * NTOK, C))
        in_maps.append(m)
    import os
    trace = bool(int(os.environ.get("KERNEL_TRACE", "0")))
    res = run_bass_kernel_spmd(nc, in_maps, core_ids=list(range(NCORES)),
                               trace=trace)
    _CACHE["last_res"] = res
    out = np.concatenate([r["out"].reshape(IMGS, HH, WW, C) for r in res.results],
                         axis=0)
    return out



# revision 34
# speedup vs baseline: 1.1992x; 1.0089x over previous
"""Trainium2 Bass kernel v2 for nn_Encoder_59219009077683 (Swin-style block).

Math shortcut (from baseline): softmax row-sums are 1, so attention output = v
and the whole block is: window-gather -> V-proj -> fixed permutation (L_d
layout) -> out-proj -> LN1+skip -> MLP -> LN2+skip -> window-scatter.

v2 structural changes vs baseline:
  - gather/scatter via ONE SWDGE indirect DMA per tile (host-built index
    table; the cyclic roll/wrap is baked into the indices)
  - bf16 for all SBUF intermediates and matmuls (tolerance is 2e-2)
  - K=64 out-projection: L_d stored twice (quadrant q holds T[d, c+6q]) so
    the 12 K=32 chunk-matmuls become 6 K=64 matmuls per output block
  - host-folded biases: v-bias+out-bias -> ybias table added at y-evac;
    norm1_b+norm2_b added at x-transpose evac; mlp_b1 corrected by
    -W1^T norm2_b so the norm2 beta rides the x2 skip path
  - rsqrt via tensor_scalar(pow -0.5) on DVE: scalar engine runs only
    Gelu/Identity -> a single activation-table load
  - elementwise spread across DVE / Act / GpSimd
"""
import numpy as np
from contextlib import ExitStack

import concourse.bass as bass
import concourse.bacc as bacc
import concourse.tile as tile
from concourse import mybir
from concourse.bass_utils import run_bass_kernel_spmd, ml_dtypes

F32 = mybir.dt.float32
BF16 = mybir.dt.bfloat16
U32 = mybir.dt.uint32
AF = mybir.ActivationFunctionType
OP = mybir.AluOpType

B, HH, WW, C = 32, 56, 56, 384
NH, HD, WS, DISP, MLP = 12, 32, 7, 3, 1536
NWS = 8
N = 49
J = NH * N            # 588
NCORES = 8
IMGS = B // NCORES
WT = 8                # windows per tile (= one window row)
TW = WT * N           # 392
NTOK = HH * WW        # tokens per image
EPS = 1e-5
LDPAD = 16
LDW = LDPAD + J * WT + 8

BF = ml_dtypes.bfloat16


def _ap(t, offset, dims):
    tt = t.tensor if hasattr(t, "tensor") else t
    return bass.AP(tensor=tt, offset=offset, ap=[list(d) for d in dims])


def build():
    nc = bacc.Bacc("TRN2", target_bir_lowering=False, debug=False, num_devices=NCORES)
    x_d = nc.dram_tensor("x", [IMGS * NTOK, C], BF16, kind="ExternalInput")
    wv_d = nc.dram_tensor("wv_t", [128, 3 * C], BF16, kind="ExternalInput")
    wo_d = nc.dram_tensor("wo_t", [64, 18 * 128], BF16, kind="ExternalInput")
    w1_d = nc.dram_tensor("w1_t", [128, 3 * MLP], BF16, kind="ExternalInput")
    w2_d = nc.dram_tensor("w2_t", [128, 12 * C], BF16, kind="ExternalInput")
    yb_d = nc.dram_tensor("yb_t", [128, 3 * TW], F32, kind="ExternalInput")
    cn_d = nc.dram_tensor("cn_t", [128, 27], F32, kind="ExternalInput")
    on_d = nc.dram_tensor("ones_t", [128, 128], BF16, kind="ExternalInput")
    idb_d = nc.dram_tensor("identb_t", [128, 128], BF16, kind="ExternalInput")
    pm_d = nc.dram_tensor("perm_t", [WW, WW], BF16, kind="ExternalInput")
    out_d = nc.dram_tensor("out", [IMGS * NTOK, C], F32, kind="ExternalOutput")

    with tile.TileContext(nc) as tc, ExitStack() as ctx:
        wpool = ctx.enter_context(tc.tile_pool(name="w", bufs=1))
        stage_pool = ctx.enter_context(tc.tile_pool(name="stage", bufs=4))
        xt_pool = ctx.enter_context(tc.tile_pool(name="xt", bufs=4))
        vt_pool = ctx.enter_context(tc.tile_pool(name="vt", bufs=3))
        ld_pool = ctx.enter_context(tc.tile_pool(name="ld", bufs=2))
        y_pool = ctx.enter_context(tc.tile_pool(name="y", bufs=3))
        x2_pool = ctx.enter_context(tc.tile_pool(name="x2", bufs=4))
        h_pool = ctx.enter_context(tc.tile_pool(name="h", bufs=3))
        h2_pool = ctx.enter_context(tc.tile_pool(name="h2", bufs=3))
        oc_pool = ctx.enter_context(tc.tile_pool(name="oc", bufs=3))
        ot_pool = ctx.enter_context(tc.tile_pool(name="ot", bufs=3))
        sm_pool = ctx.enter_context(tc.tile_pool(name="sm", bufs=5))
        ps_t = ctx.enter_context(tc.tile_pool(name="pst", bufs=1, space="PSUM"))
        ps_a = ctx.enter_context(tc.tile_pool(name="psa", bufs=2, space="PSUM"))
        ps_m = ctx.enter_context(tc.tile_pool(name="psm", bufs=2, space="PSUM"))
        ps_s = ctx.enter_context(tc.tile_pool(name="pss", bufs=1, space="PSUM"))
        ps_o = ctx.enter_context(tc.tile_pool(name="pso", bufs=2, space="PSUM"))

        # ---------- one-time setup: weights straight from host layout ----------
        wv_b = wpool.tile([128, 3 * C], BF16)
        wo_b = wpool.tile([64, 18 * 128], BF16)
        w1_b = wpool.tile([128, 3 * MLP], BF16)
        w2_b = wpool.tile([128, 12 * C], BF16)
        yb_sb = wpool.tile([128, 3 * TW], F32)
        cn_sb = wpool.tile([128, 27], F32)
        ones_b = wpool.tile([128, 128], BF16)
        identb = wpool.tile([128, 128], BF16)
        perm56 = wpool.tile([WW, WW], BF16)
        eps_t = wpool.tile([128, 1], F32)
        zero_t = wpool.tile([128, 1], F32)
        nc.vector.memset(eps_t[:, :], EPS)
        nc.vector.memset(zero_t[:, :], 0.0)
        nc.sync.dma_start(perm56[:, :], pm_d[:, :])
        nc.sync.dma_start(wv_b[:, :], wv_d[:, :])
        nc.sync.dma_start(wo_b[:, :], wo_d[:, :])
        nc.sync.dma_start(w1_b[:, :], w1_d[:, :])
        nc.sync.dma_start(w2_b[:, :], w2_d[:, :])
        nc.sync.dma_start(yb_sb[:, :], yb_d[:, :])
        nc.sync.dma_start(cn_sb[:, :], cn_d[:, :])
        nc.sync.dma_start(ones_b[:, :], on_d[:, :])
        nc.sync.dma_start(identb[:, :], idb_d[:, :])
        # PE must observe the identity via a transpose once before the loop so
        # later transposes carry <=1 sync wait (bare LDWEIGHTS limitation).
        dmy = ps_t.tile([128, 392], BF16, tag="t")
        nc.tensor.transpose(dmy[0:WW, 0:WW], perm56[0:WW, 0:WW], perm56[0:WW, 0:WW])
        dmy2 = ps_t.tile([128, 392], BF16, tag="t")
        nc.tensor.transpose(dmy2[0:128, 0:128], identb[:, :], identb[:, :])

        # ---------- software-pipelined main loop ----------
        # tile index t = img*8 + wr; phases skewed across iterations so every
        # engine (PE especially) always has ready work.
        NT = IMGS * NWS
        st = {}   # per-tile live state

        def do_gather(t):
            img, wr = divmod(t, NWS)
            stage = stage_pool.tile([WW, WS * C], BF16, tag="stage")
            pst = stage[:, :].ap[0][0]
            soff = stage[:, :].offset
            rows = [(WS * wr + DISP, 0, WS)] if wr < NWS - 1 else \
                [(52, 0, 4), (0, 4, 3)]
            for (r0, tr0, ntr) in rows:
                src = _ap(x_d, (img * NTOK + r0 * WW) * C,
                          [[C, WW], [WW * C, ntr], [1, C]])
                dst = _ap(stage, soff + tr0 * C,
                          [[pst, WW], [C, ntr], [1, C]])
                nc.sync.dma_start(dst, src)
            st[t] = {"stage": stage}

        def do_p1(t):
            """transposes + V-proj + ld build"""
            d = st[t]
            stage = d.pop("stage")
            xt = xt_pool.tile([128, 3 * TW], BF16, tag="xt")
            for k in range(3):
                xps = ps_t.tile([128, 392], BF16, tag="t")
                pxp = xps[:, :].ap[0][0]
                xpo = xps[:, :].offset
                for tr in range(WS):
                    nc.tensor.transpose(
                        xps[:, WW * tr:WW * tr + WW],
                        stage[0:WW, tr * C + 128 * k:tr * C + 128 * k + 128],
                        perm56[0:WW, 0:WW])
                # token order everywhere downstream: col = 56*tr + 7*win + tc
                nc.vector.tensor_scalar(xt[:, TW * k:TW * k + TW], xps[:, :],
                                        cn_sb[:, 21 + k:22 + k], None, op0=OP.add)
            vt = vt_pool.tile([128, 3 * TW], BF16, tag="vt")
            pvt = vt[:, :].ap[0][0]
            vtoff = vt[:, :].offset
            for kv in range(3):
                vps = ps_a.tile([128, TW], F32, tag="a")
                for k in range(3):
                    nc.tensor.matmul(vps[:, :],
                                     wv_b[:, C * k + 128 * kv:C * k + 128 * kv + 128],
                                     xt[:, TW * k:TW * k + TW],
                                     start=(k == 0), stop=(k == 2))
                pps = vps[:, :].ap[0][0]
                nc.scalar.activation(
                    _ap(vt, vtoff + N * kv,
                        [[pvt, 128], [WS, WS], [3 * N, WT], [1, WS]]),
                    _ap(vps, vps[:, :].offset,
                        [[pps, 128], [WW, WS], [WS, WT], [1, WS]]),
                    AF.Identity, bias=zero_t[:, :], scale=1.0)
            ld = ld_pool.tile([64, LDW], BF16, tag="ld")
            pld = ld[:, :].ap[0][0]
            ldoff = ld[:, :].offset
            for s4 in range(4):
                for q in range(2):
                    src = _ap(vt, vtoff + (32 * s4) * pvt,
                              [[pvt, 32], [N, 24], [1, N]])
                    dst = _ap(ld, ldoff + (32 * q) * pld + LDPAD + N * s4 - 6 * q,
                              [[pld, 32], [4 * N, 24], [1, N]])
                    nc.sync.dma_start(dst, src)
            d["xt"] = xt
            d["ld"] = (ld, pld, ldoff)

        def do_p2a(t):
            """out-proj + ybias evac + ysq"""
            d = st[t]
            ld, pld, ldoff = d.pop("ld")
            y = y_pool.tile([128, 3 * TW], BF16, tag="y")
            for kj in range(3):
                yps = ps_o.tile([128, TW], F32, tag="o")
                for r in range(6):
                    rhs = _ap(ld, ldoff + LDPAD + r,
                              [[pld, 64], [84, WS], [J, WT], [12, WS]])
                    nc.tensor.matmul(yps[:, :],
                                     wo_b[:, (r * 3 + kj) * 128:(r * 3 + kj) * 128 + 128],
                                     rhs, start=(r == 0), stop=(r == 5))
                nc.vector.tensor_add(y[:, TW * kj:TW * kj + TW], yps[:, :],
                                     yb_sb[:, TW * kj:TW * kj + TW])
            ysq = sm_pool.tile([128, 3 * TW], BF16, tag="ysq")
            for k in range(3):
                nc.vector.tensor_mul(ysq[:, TW * k:TW * k + TW],
                                     y[:, TW * k:TW * k + TW], y[:, TW * k:TW * k + TW])
            d["y"] = y
            d["ysq"] = ysq

        def _ln(y, ysq, gcol, skip, out):
            """matmul sums -> stats -> apply; out = (y-mu)*rst*g + skip."""
            s1 = ps_s.tile([128, TW], F32, tag="s")
            s2 = ps_s.tile([128, TW], F32, tag="s")
            for k in range(3):
                nc.tensor.matmul(s1[:, :], ones_b[:, :], y[:, TW * k:TW * k + TW],
                                 start=(k == 0), stop=(k == 2))
                nc.tensor.matmul(s2[:, :], ones_b[:, :], ysq[:, TW * k:TW * k + TW],
                                 start=(k == 0), stop=(k == 2))
            mu = sm_pool.tile([128, TW], F32, tag="mu")
            m2 = sm_pool.tile([128, TW], F32, tag="m2")
            veps = sm_pool.tile([128, TW], F32, tag="veps")
            rst = sm_pool.tile([128, TW], F32, tag="rst")
            nc.vector.tensor_copy(mu[:, :], s1[:, :])
            nc.scalar.square(m2[:, :], mu[:, :])
            nc.vector.tensor_sub(veps[:, :], s2[:, :], m2[:, :])
            # std = sqrt(var + eps) on Act (eps folded into the bias), then
            # 1/std via the fast custom-DVE reciprocal (~51 ULP, plenty here)
            nc.scalar.activation(veps[:, :], veps[:, :], AF.Sqrt,
                                 bias=eps_t[:, :], scale=1.0)
            nc.vector.reciprocal_approx_fast(rst[:, :], veps[:, :])
            for k in range(3):
                u = sm_pool.tile([128, TW], BF16, tag="u")
                v = sm_pool.tile([128, TW], BF16, tag="v")
                nc.vector.tensor_sub(u[:, :], y[:, TW * k:TW * k + TW], mu[:, :])
                nc.vector.tensor_mul(v[:, :], u[:, :], rst[:, :])
                nc.vector.scalar_tensor_tensor(out[:, TW * k:TW * k + TW],
                                               v[:, :],
                                               cn_sb[:, gcol + k:gcol + k + 1],
                                               skip[:, TW * k:TW * k + TW],
                                               op0=OP.mult, op1=OP.add)

        def do_p2b(t):
            d = st[t]
            x2 = x2_pool.tile([128, 3 * TW], BF16, tag="x2")
            _ln(d.pop("y"), d.pop("ysq"), 15, d.pop("xt"), x2)
            d["x2"] = x2

        def do_p3(t):
            d = st[t]
            x2 = d["x2"]
            hsb = h_pool.tile([128, 12 * TW], BF16, tag="h")
            for m in range(12):
                hps = ps_m.tile([128, TW], F32, tag="m")
                for k in range(3):
                    nc.tensor.matmul(hps[:, :],
                                     w1_b[:, MLP * k + 128 * m:MLP * k + 128 * m + 128],
                                     x2[:, TW * k:TW * k + TW],
                                     start=(k == 0), stop=(k == 2))
                nc.scalar.activation(hsb[:, TW * m:TW * m + TW], hps[:, :],
                                     AF.Gelu, bias=cn_sb[:, m:m + 1], scale=1.0)
            d["hsb"] = hsb

        def do_p4a(t):
            d = st[t]
            hsb = d.pop("hsb")
            h2 = h2_pool.tile([128, 3 * TW], BF16, tag="h2")
            for kj in range(3):
                h2ps = ps_a.tile([128, TW], F32, tag="a")
                for k2 in range(12):
                    nc.tensor.matmul(h2ps[:, :],
                                     w2_b[:, C * k2 + 128 * kj:C * k2 + 128 * kj + 128],
                                     hsb[:, TW * k2:TW * k2 + TW],
                                     start=(k2 == 0), stop=(k2 == 11))
                nc.vector.tensor_scalar(h2[:, TW * kj:TW * kj + TW], h2ps[:, :],
                                        cn_sb[:, 12 + kj:13 + kj], None, op0=OP.add)
            hsq = sm_pool.tile([128, 3 * TW], BF16, tag="ysq")
            for k in range(3):
                nc.vector.tensor_mul(hsq[:, TW * k:TW * k + TW],
                                     h2[:, TW * k:TW * k + TW], h2[:, TW * k:TW * k + TW])
            d["h2"] = h2
            d["hsq"] = hsq

        def do_p4b(t):
            d = st[t]
            ocm = oc_pool.tile([128, 3 * TW], BF16, tag="oc")
            _ln(d.pop("h2"), d.pop("hsq"), 18, d.pop("x2"), ocm)
            d["ocm"] = ocm

        def do_p5(t):
            d = st.pop(t)
            img, wr = divmod(t, NWS)
            ocm = d["ocm"]
            poc = ocm[:, :].ap[0][0]
            oco = ocm[:, :].offset
            # paired: otr[56*(tr%2)+x, 384*(tr//2)+c]; 12 transposes not 21
            otr = ot_pool.tile([112, 4 * C], BF16, tag="ot")
            pot = otr[:, :].ap[0][0]
            poff = otr[:, :].offset
            for p in range(4):
                prow = 112 if p < 3 else 56
                ops_t = ps_t.tile([128, 392], BF16, tag="t")
                for k in range(3):
                    nc.tensor.transpose(
                        ops_t[0:prow, 128 * k:128 * k + 128],
                        ocm[:, TW * k + 112 * p:TW * k + 112 * p + prow],
                        identb[:, :])
                nc.vector.tensor_copy(otr[0:prow, p * C:p * C + C],
                                      ops_t[0:prow, 0:C])

            def scat(prow0, npair, r0, odd):
                base = poff + (56 * odd) * pot + 384 * prow0
                dst0 = (img * NTOK + r0 * WW) * C
                src = _ap(otr, base, [[pot, 53], [C, npair], [1, C]])
                dst = _ap(out_d, dst0 + DISP * C,
                          [[C, 53], [2 * WW * C, npair], [1, C]])
                nc.gpsimd.dma_start(dst, src)
                src = _ap(otr, base + 53 * pot, [[pot, 3], [C, npair], [1, C]])
                dst = _ap(out_d, dst0, [[C, 3], [2 * WW * C, npair], [1, C]])
                nc.gpsimd.dma_start(dst, src)

            if wr < NWS - 1:
                r0 = WS * wr + DISP
                scat(0, 4, r0, 0)
                scat(0, 3, r0 + 1, 1)
            else:
                scat(0, 2, 52, 0)
                scat(0, 2, 53, 1)
                scat(2, 2, 0, 0)
                scat(2, 1, 1, 1)

        for i in range(NT + 7):
            if i < NT:
                do_gather(i)
            if 0 <= i - 2 < NT:
                do_p2a(i - 2)
            if 0 <= i - 1 < NT:
                do_p1(i - 1)
            if 0 <= i - 5 < NT:
                do_p4a(i - 5)
            if 0 <= i - 7 < NT:
                do_p5(i - 7)
            if 0 <= i - 3 < NT:
                do_p2b(i - 3)
            if 0 <= i - 4 < NT:
                do_p3(i - 4)
            if 0 <= i - 6 < NT:
                do_p4b(i - 6)
    nc.compile()
    return nc


def _to_bf(a):
    return np.ascontiguousarray(a.astype(BF))


def prep_inputs(inputs):
    """Host-side weight/bias reformatting (layout only + tiny bias algebra)."""
    f = {k: np.asarray(v, dtype=np.float32) for k, v in inputs.items()}
    qkv_w, qkv_b = f["qkv_w"], f["qkv_b"]
    out_w, out_b = f["out_w"], f["out_b"]
    w1, b1, w2, b2 = f["mlp_w1"], f["mlp_b1"], f["mlp_w2"], f["mlp_b2"]
    g1, be1 = f["norm1_g"], f["norm1_b"]
    g2, be2 = f["norm2_g"], f["norm2_b"]

    wv = qkv_w[:, 2 * C:3 * C]
    bv = qkv_b[2 * C:3 * C]

    wv_t = np.zeros((128, 3 * C), np.float32)
    for k in range(3):
        wv_t[:, C * k:C * k + C] = wv[128 * k:128 * k + 128, :]

    wo_t = np.zeros((64, 18 * 128), np.float32)
    for r in range(6):
        for q in range(2):
            for kj in range(3):
                wo_t[32 * q:32 * q + 32, (r * 3 + kj) * 128:(r * 3 + kj) * 128 + 128] = \
                    out_w[32 * (r + 6 * q):32 * (r + 6 * q) + 32, 128 * kj:128 * kj + 128]

    w1_t = np.zeros((128, 3 * MLP), np.float32)
    for k in range(3):
        w1_t[:, MLP * k:MLP * k + MLP] = w1[128 * k:128 * k + 128, :]
    w2_t = np.zeros((128, 12 * C), np.float32)
    for k2 in range(12):
        w2_t[:, C * k2:C * k2 + C] = w2[128 * k2:128 * k2 + 128, :]

    # ybias[n2, :] = P(bv)[n2] @ out_w + out_b
    n2 = np.arange(N)[:, None]
    co = np.arange(12)[None, :]
    hh = (12 * n2 + co) // N                    # (49, 12)
    pb = bv.reshape(12, 32)[hh]                 # (49, 12, 32)
    ybias = pb.reshape(N, C) @ out_w + out_b    # (49, 384)
    yb_t = np.zeros((128, 3 * TW), np.float32)
    tr_ = np.arange(WS)[:, None, None]
    wn_ = np.arange(WT)[None, :, None]
    tc_ = np.arange(WS)[None, None, :]
    pc = (WS * tr_ + tc_ + 0 * wn_).reshape(TW)   # n2 per permuted column
    for kj in range(3):
        blk = ybias[:, 128 * kj:128 * kj + 128].T          # (128, 49)
        yb_t[:, TW * kj:TW * kj + TW] = blk[:, pc]

    b1p = b1 - be2 @ w1                          # mlp bias corrected for +be2 on x2
    be12 = be1 + be2
    cn_t = np.zeros((128, 27), np.float32)
    for m in range(12):
        cn_t[:, m] = b1p[128 * m:128 * m + 128]
    for k in range(3):
        cn_t[:, 12 + k] = b2[128 * k:128 * k + 128]
        cn_t[:, 15 + k] = g1[128 * k:128 * k + 128]
        cn_t[:, 18 + k] = g2[128 * k:128 * k + 128]
        cn_t[:, 21 + k] = be12[128 * k:128 * k + 128]

    ones_t = np.full((128, 128), 1.0 / C, np.float32)
    ident = np.eye(128, dtype=np.float32)

    # column-roll permutation for the input transposes
    perm = np.zeros((WW, WW), np.float32)
    jj = np.arange(WW)
    perm[(jj + DISP) % WW, jj] = 1.0

    return {
        "wv_t": _to_bf(wv_t), "wo_t": _to_bf(wo_t),
        "w1_t": _to_bf(w1_t), "w2_t": _to_bf(w2_t),
        "yb_t": np.ascontiguousarray(yb_t), "cn_t": np.ascontiguousarray(cn_t),
        "ones_t": _to_bf(ones_t), "identb_t": _to_bf(ident),
        "perm_t": _to_bf(perm),
    }


_CACHE = {}


def kernel(**inputs):
    if "nc" not in _CACHE:
        _CACHE["nc"] = build()
    nc = _CACHE["nc"]
    x = np.asarray(inputs["x"], dtype=np.float32).astype(BF)
    base = prep_inputs(inputs)
    in_maps = []
    for c in range(NCORES):
        m = dict(base)
        m["x"] = np.ascontiguousarray(
            x[IMGS * c:IMGS * (c + 1)].reshape(IMGS * NTOK, C))
        in_maps.append(m)
    import os
    trace = bool(int(os.environ.get("KERNEL_TRACE", "0")))
    res = run_bass_kernel_spmd(nc, in_maps, core_ids=list(range(NCORES)),
                               trace=trace)
    _CACHE["last_res"] = res
    out = np.concatenate([r["out"].reshape(IMGS, HH, WW, C) for r in res.results],
                         axis=0)
    return out



# revision 35
# speedup vs baseline: 1.2011x; 1.0016x over previous
"""Trainium2 Bass kernel v2 for nn_Encoder_59219009077683 (Swin-style block).

Math shortcut (from baseline): softmax row-sums are 1, so attention output = v
and the whole block is: window-gather -> V-proj -> fixed permutation (L_d
layout) -> out-proj -> LN1+skip -> MLP -> LN2+skip -> window-scatter.

v2 structural changes vs baseline:
  - gather/scatter via ONE SWDGE indirect DMA per tile (host-built index
    table; the cyclic roll/wrap is baked into the indices)
  - bf16 for all SBUF intermediates and matmuls (tolerance is 2e-2)
  - K=64 out-projection: L_d stored twice (quadrant q holds T[d, c+6q]) so
    the 12 K=32 chunk-matmuls become 6 K=64 matmuls per output block
  - host-folded biases: v-bias+out-bias -> ybias table added at y-evac;
    norm1_b+norm2_b added at x-transpose evac; mlp_b1 corrected by
    -W1^T norm2_b so the norm2 beta rides the x2 skip path
  - rsqrt via tensor_scalar(pow -0.5) on DVE: scalar engine runs only
    Gelu/Identity -> a single activation-table load
  - elementwise spread across DVE / Act / GpSimd
"""
import numpy as np
from contextlib import ExitStack

import concourse.bass as bass
import concourse.bacc as bacc
import concourse.tile as tile
from concourse import mybir
from concourse.bass_utils import run_bass_kernel_spmd, ml_dtypes

F32 = mybir.dt.float32
BF16 = mybir.dt.bfloat16
U32 = mybir.dt.uint32
AF = mybir.ActivationFunctionType
OP = mybir.AluOpType

B, HH, WW, C = 32, 56, 56, 384
NH, HD, WS, DISP, MLP = 12, 32, 7, 3, 1536
NWS = 8
N = 49
J = NH * N            # 588
NCORES = 8
IMGS = B // NCORES
WT = 8                # windows per tile (= one window row)
TW = WT * N           # 392
NTOK = HH * WW        # tokens per image
EPS = 1e-5
LDPAD = 16
LDW = LDPAD + J * WT + 8

BF = ml_dtypes.bfloat16


def _ap(t, offset, dims):
    tt = t.tensor if hasattr(t, "tensor") else t
    return bass.AP(tensor=tt, offset=offset, ap=[list(d) for d in dims])


def build():
    nc = bacc.Bacc("TRN2", target_bir_lowering=False, debug=False, num_devices=NCORES)
    x_d = nc.dram_tensor("x", [IMGS * NTOK, C], BF16, kind="ExternalInput")
    wv_d = nc.dram_tensor("wv_t", [128, 3 * C], BF16, kind="ExternalInput")
    wo_d = nc.dram_tensor("wo_t", [64, 18 * 128], BF16, kind="ExternalInput")
    w1_d = nc.dram_tensor("w1_t", [128, 3 * MLP], BF16, kind="ExternalInput")
    w2_d = nc.dram_tensor("w2_t", [128, 12 * C], BF16, kind="ExternalInput")
    yb_d = nc.dram_tensor("yb_t", [128, 3 * TW], F32, kind="ExternalInput")
    cn_d = nc.dram_tensor("cn_t", [128, 27], F32, kind="ExternalInput")
    on_d = nc.dram_tensor("ones_t", [128, 128], BF16, kind="ExternalInput")
    idb_d = nc.dram_tensor("identb_t", [128, 128], BF16, kind="ExternalInput")
    pm_d = nc.dram_tensor("perm_t", [WW, WW], BF16, kind="ExternalInput")
    out_d = nc.dram_tensor("out", [IMGS * NTOK, C], F32, kind="ExternalOutput")

    with tile.TileContext(nc) as tc, ExitStack() as ctx:
        wpool = ctx.enter_context(tc.tile_pool(name="w", bufs=1))
        stage_pool = ctx.enter_context(tc.tile_pool(name="stage", bufs=4))
        xt_pool = ctx.enter_context(tc.tile_pool(name="xt", bufs=4))
        vt_pool = ctx.enter_context(tc.tile_pool(name="vt", bufs=3))
        ld_pool = ctx.enter_context(tc.tile_pool(name="ld", bufs=2))
        y_pool = ctx.enter_context(tc.tile_pool(name="y", bufs=3))
        x2_pool = ctx.enter_context(tc.tile_pool(name="x2", bufs=4))
        h_pool = ctx.enter_context(tc.tile_pool(name="h", bufs=3))
        h2_pool = ctx.enter_context(tc.tile_pool(name="h2", bufs=3))
        oc_pool = ctx.enter_context(tc.tile_pool(name="oc", bufs=3))
        ot_pool = ctx.enter_context(tc.tile_pool(name="ot", bufs=3))
        sm_pool = ctx.enter_context(tc.tile_pool(name="sm", bufs=5))
        ps_t = ctx.enter_context(tc.tile_pool(name="pst", bufs=1, space="PSUM"))
        ps_a = ctx.enter_context(tc.tile_pool(name="psa", bufs=2, space="PSUM"))
        ps_m = ctx.enter_context(tc.tile_pool(name="psm", bufs=2, space="PSUM"))
        ps_s = ctx.enter_context(tc.tile_pool(name="pss", bufs=1, space="PSUM"))
        ps_o = ctx.enter_context(tc.tile_pool(name="pso", bufs=2, space="PSUM"))

        # ---------- one-time setup: weights straight from host layout ----------
        wv_b = wpool.tile([128, 3 * C], BF16)
        wo_b = wpool.tile([64, 18 * 128], BF16)
        w1_b = wpool.tile([128, 3 * MLP], BF16)
        w2_b = wpool.tile([128, 12 * C], BF16)
        yb_sb = wpool.tile([128, 3 * TW], F32)
        cn_sb = wpool.tile([128, 27], F32)
        ones_b = wpool.tile([128, 128], BF16)
        identb = wpool.tile([128, 128], BF16)
        perm56 = wpool.tile([WW, WW], BF16)
        eps_t = wpool.tile([128, 1], F32)
        zero_t = wpool.tile([128, 1], F32)
        nc.vector.memset(eps_t[:, :], EPS)
        nc.vector.memset(zero_t[:, :], 0.0)
        nc.sync.dma_start(perm56[:, :], pm_d[:, :])
        nc.sync.dma_start(wv_b[:, :], wv_d[:, :])
        nc.sync.dma_start(wo_b[:, :], wo_d[:, :])
        nc.sync.dma_start(w1_b[:, :], w1_d[:, :])
        nc.sync.dma_start(w2_b[:, :], w2_d[:, :])
        nc.sync.dma_start(yb_sb[:, :], yb_d[:, :])
        nc.sync.dma_start(cn_sb[:, :], cn_d[:, :])
        nc.sync.dma_start(ones_b[:, :], on_d[:, :])
        nc.sync.dma_start(identb[:, :], idb_d[:, :])
        # PE must observe the identity via a transpose once before the loop so
        # later transposes carry <=1 sync wait (bare LDWEIGHTS limitation).
        dmy = ps_t.tile([128, 392], BF16, tag="t")
        nc.tensor.transpose(dmy[0:WW, 0:WW], perm56[0:WW, 0:WW], perm56[0:WW, 0:WW])
        dmy2 = ps_t.tile([128, 392], BF16, tag="t")
        nc.tensor.transpose(dmy2[0:128, 0:128], identb[:, :], identb[:, :])

        # ---------- software-pipelined main loop ----------
        # tile index t = img*8 + wr; phases skewed across iterations so every
        # engine (PE especially) always has ready work.
        NT = IMGS * NWS
        st = {}   # per-tile live state

        def do_gather(t):
            img, wr = divmod(t, NWS)
            stage = stage_pool.tile([WW, WS * C], BF16, tag="stage")
            pst = stage[:, :].ap[0][0]
            soff = stage[:, :].offset
            rows = [(WS * wr + DISP, 0, WS)] if wr < NWS - 1 else \
                [(52, 0, 4), (0, 4, 3)]
            for (r0, tr0, ntr) in rows:
                src = _ap(x_d, (img * NTOK + r0 * WW) * C,
                          [[C, WW], [WW * C, ntr], [1, C]])
                dst = _ap(stage, soff + tr0 * C,
                          [[pst, WW], [C, ntr], [1, C]])
                nc.sync.dma_start(dst, src)
            st[t] = {"stage": stage}

        def do_p1(t):
            """transposes + V-proj + ld build"""
            d = st[t]
            stage = d.pop("stage")
            xt = xt_pool.tile([128, 3 * TW], BF16, tag="xt")
            for k in range(3):
                xps = ps_t.tile([128, 392], BF16, tag="t")
                pxp = xps[:, :].ap[0][0]
                xpo = xps[:, :].offset
                for tr in range(WS):
                    nc.tensor.transpose(
                        xps[:, WW * tr:WW * tr + WW],
                        stage[0:WW, tr * C + 128 * k:tr * C + 128 * k + 128],
                        perm56[0:WW, 0:WW])
                # token order everywhere downstream: col = 56*tr + 7*win + tc
                nc.vector.tensor_scalar(xt[:, TW * k:TW * k + TW], xps[:, :],
                                        cn_sb[:, 21 + k:22 + k], None, op0=OP.add)
            vt = vt_pool.tile([128, 3 * TW], BF16, tag="vt")
            pvt = vt[:, :].ap[0][0]
            vtoff = vt[:, :].offset
            for kv in range(3):
                vps = ps_a.tile([128, TW], F32, tag="a")
                for k in range(3):
                    nc.tensor.matmul(vps[:, :],
                                     wv_b[:, C * k + 128 * kv:C * k + 128 * kv + 128],
                                     xt[:, TW * k:TW * k + TW],
                                     start=(k == 0), stop=(k == 2))
                pps = vps[:, :].ap[0][0]
                nc.scalar.activation(
                    _ap(vt, vtoff + N * kv,
                        [[pvt, 128], [WS, WS], [3 * N, WT], [1, WS]]),
                    _ap(vps, vps[:, :].offset,
                        [[pps, 128], [WW, WS], [WS, WT], [1, WS]]),
                    AF.Identity, bias=zero_t[:, :], scale=1.0)
            ld = ld_pool.tile([64, LDW], BF16, tag="ld")
            pld = ld[:, :].ap[0][0]
            ldoff = ld[:, :].offset
            for s4 in range(4):
                for q in range(2):
                    src = _ap(vt, vtoff + (32 * s4) * pvt,
                              [[pvt, 32], [N, 24], [1, N]])
                    dst = _ap(ld, ldoff + (32 * q) * pld + LDPAD + N * s4 - 6 * q,
                              [[pld, 32], [4 * N, 24], [1, N]])
                    nc.sync.dma_start(dst, src)
            d["xt"] = xt
            d["ld"] = (ld, pld, ldoff)

        def do_p2a(t):
            """out-proj + ybias evac + ysq"""
            d = st[t]
            ld, pld, ldoff = d.pop("ld")
            y = y_pool.tile([128, 3 * TW], BF16, tag="y")
            for kj in range(3):
                yps = ps_o.tile([128, TW], F32, tag="o")
                for r in range(6):
                    rhs = _ap(ld, ldoff + LDPAD + r,
                              [[pld, 64], [84, WS], [J, WT], [12, WS]])
                    nc.tensor.matmul(yps[:, :],
                                     wo_b[:, (r * 3 + kj) * 128:(r * 3 + kj) * 128 + 128],
                                     rhs, start=(r == 0), stop=(r == 5))
                nc.vector.tensor_add(y[:, TW * kj:TW * kj + TW], yps[:, :],
                                     yb_sb[:, TW * kj:TW * kj + TW])
            ysq = sm_pool.tile([128, 3 * TW], BF16, tag="ysq")
            for k in range(3):
                nc.vector.tensor_mul(ysq[:, TW * k:TW * k + TW],
                                     y[:, TW * k:TW * k + TW], y[:, TW * k:TW * k + TW])
            d["y"] = y
            d["ysq"] = ysq

        def _ln(y, ysq, gcol, skip, out):
            """matmul sums -> stats -> apply; out = (y-mu)*rst*g + skip."""
            s1 = ps_s.tile([128, TW], F32, tag="s")
            s2 = ps_s.tile([128, TW], F32, tag="s")
            for k in range(3):
                nc.tensor.matmul(s1[:, :], ones_b[:, :], y[:, TW * k:TW * k + TW],
                                 start=(k == 0), stop=(k == 2))
                nc.tensor.matmul(s2[:, :], ones_b[:, :], ysq[:, TW * k:TW * k + TW],
                                 start=(k == 0), stop=(k == 2))
            mu = sm_pool.tile([128, TW], F32, tag="mu")
            m2 = sm_pool.tile([128, TW], F32, tag="m2")
            veps = sm_pool.tile([128, TW], F32, tag="veps")
            rst = sm_pool.tile([128, TW], F32, tag="rst")
            nc.vector.tensor_copy(mu[:, :], s1[:, :])
            nc.scalar.square(m2[:, :], mu[:, :])
            nc.vector.tensor_sub(veps[:, :], s2[:, :], m2[:, :])
            # std = sqrt(var + eps) on Act (eps folded into the bias), then
            # 1/std via the fast custom-DVE reciprocal (~51 ULP, plenty here)
            nc.scalar.activation(veps[:, :], veps[:, :], AF.Sqrt,
                                 bias=eps_t[:, :], scale=1.0)
            nc.vector.reciprocal_approx_fast(rst[:, :], veps[:, :])
            for k in range(3):
                u = sm_pool.tile([128, TW], BF16, tag="u")
                v = sm_pool.tile([128, TW], BF16, tag="v")
                nc.vector.tensor_sub(u[:, :], y[:, TW * k:TW * k + TW], mu[:, :])
                nc.vector.tensor_mul(v[:, :], u[:, :], rst[:, :])
                nc.vector.scalar_tensor_tensor(out[:, TW * k:TW * k + TW],
                                               v[:, :],
                                               cn_sb[:, gcol + k:gcol + k + 1],
                                               skip[:, TW * k:TW * k + TW],
                                               op0=OP.mult, op1=OP.add)

        def do_p2b(t):
            d = st[t]
            x2 = x2_pool.tile([128, 3 * TW], BF16, tag="x2")
            _ln(d.pop("y"), d.pop("ysq"), 15, d.pop("xt"), x2)
            d["x2"] = x2

        def do_p3(t):
            d = st[t]
            x2 = d["x2"]
            hsb = h_pool.tile([128, 12 * TW], BF16, tag="h")
            for m in range(12):
                hps = ps_m.tile([128, TW], F32, tag="m")
                for k in range(3):
                    nc.tensor.matmul(hps[:, :],
                                     w1_b[:, MLP * k + 128 * m:MLP * k + 128 * m + 128],
                                     x2[:, TW * k:TW * k + TW],
                                     start=(k == 0), stop=(k == 2))
                nc.scalar.activation(hsb[:, TW * m:TW * m + TW], hps[:, :],
                                     AF.Gelu, bias=cn_sb[:, m:m + 1], scale=1.0)
            d["hsb"] = hsb

        def do_p4a(t):
            d = st[t]
            hsb = d.pop("hsb")
            h2 = h2_pool.tile([128, 3 * TW], BF16, tag="h2")
            for kj in range(3):
                h2ps = ps_a.tile([128, TW], F32, tag="a")
                for k2 in range(12):
                    nc.tensor.matmul(h2ps[:, :],
                                     w2_b[:, C * k2 + 128 * kj:C * k2 + 128 * kj + 128],
                                     hsb[:, TW * k2:TW * k2 + TW],
                                     start=(k2 == 0), stop=(k2 == 11))
                nc.scalar.activation(h2[:, TW * kj:TW * kj + TW], h2ps[:, :],
                                     AF.Identity, bias=cn_sb[:, 12 + kj:13 + kj],
                                     scale=1.0)
            hsq = sm_pool.tile([128, 3 * TW], BF16, tag="ysq")
            for k in range(3):
                nc.vector.tensor_mul(hsq[:, TW * k:TW * k + TW],
                                     h2[:, TW * k:TW * k + TW], h2[:, TW * k:TW * k + TW])
            d["h2"] = h2
            d["hsq"] = hsq

        def do_p4b(t):
            d = st[t]
            ocm = oc_pool.tile([128, 3 * TW], BF16, tag="oc")
            _ln(d.pop("h2"), d.pop("hsq"), 18, d.pop("x2"), ocm)
            d["ocm"] = ocm

        def do_p5(t):
            d = st.pop(t)
            img, wr = divmod(t, NWS)
            ocm = d["ocm"]
            poc = ocm[:, :].ap[0][0]
            oco = ocm[:, :].offset
            # paired: otr[56*(tr%2)+x, 384*(tr//2)+c]; 12 transposes not 21
            otr = ot_pool.tile([112, 4 * C], BF16, tag="ot")
            pot = otr[:, :].ap[0][0]
            poff = otr[:, :].offset
            for p in range(4):
                prow = 112 if p < 3 else 56
                ops_t = ps_t.tile([128, 392], BF16, tag="t")
                for k in range(3):
                    nc.tensor.transpose(
                        ops_t[0:prow, 128 * k:128 * k + 128],
                        ocm[:, TW * k + 112 * p:TW * k + 112 * p + prow],
                        identb[:, :])
                nc.vector.tensor_copy(otr[0:prow, p * C:p * C + C],
                                      ops_t[0:prow, 0:C])

            def scat(prow0, npair, r0, odd):
                base = poff + (56 * odd) * pot + 384 * prow0
                dst0 = (img * NTOK + r0 * WW) * C
                src = _ap(otr, base, [[pot, 53], [C, npair], [1, C]])
                dst = _ap(out_d, dst0 + DISP * C,
                          [[C, 53], [2 * WW * C, npair], [1, C]])
                nc.gpsimd.dma_start(dst, src)
                src = _ap(otr, base + 53 * pot, [[pot, 3], [C, npair], [1, C]])
                dst = _ap(out_d, dst0, [[C, 3], [2 * WW * C, npair], [1, C]])
                nc.gpsimd.dma_start(dst, src)

            if wr < NWS - 1:
                r0 = WS * wr + DISP
                scat(0, 4, r0, 0)
                scat(0, 3, r0 + 1, 1)
            else:
                scat(0, 2, 52, 0)
                scat(0, 2, 53, 1)
                scat(2, 2, 0, 0)
                scat(2, 1, 1, 1)

        for i in range(NT + 7):
            if i < NT:
                do_gather(i)
            if 0 <= i - 2 < NT:
                do_p2a(i - 2)
            if 0 <= i - 1 < NT:
                do_p1(i - 1)
            if 0 <= i - 5 < NT:
                do_p4a(i - 5)
            if 0 <= i - 7 < NT:
                do_p5(i - 7)
            if 0 <= i - 3 < NT:
                do_p2b(i - 3)
            if 0 <= i - 4 < NT:
                do_p3(i - 4)
            if 0 <= i - 6 < NT:
                do_p4b(i - 6)
    nc.compile()
    return nc


def _to_bf(a):
    return np.ascontiguousarray(a.astype(BF))


def prep_inputs(inputs):
    """Host-side weight/bias reformatting (layout only + tiny bias algebra)."""
    f = {k: np.asarray(v, dtype=np.float32) for k, v in inputs.items()}
    qkv_w, qkv_b = f["qkv_w"], f["qkv_b"]
    out_w, out_b = f["out_w"], f["out_b"]
    w1, b1, w2, b2 = f["mlp_w1"], f["mlp_b1"], f["mlp_w2"], f["mlp_b2"]
    g1, be1 = f["norm1_g"], f["norm1_b"]
    g2, be2 = f["norm2_g"], f["norm2_b"]

    wv = qkv_w[:, 2 * C:3 * C]
    bv = qkv_b[2 * C:3 * C]

    wv_t = np.zeros((128, 3 * C), np.float32)
    for k in range(3):
        wv_t[:, C * k:C * k + C] = wv[128 * k:128 * k + 128, :]

    wo_t = np.zeros((64, 18 * 128), np.float32)
    for r in range(6):
        for q in range(2):
            for kj in range(3):
                wo_t[32 * q:32 * q + 32, (r * 3 + kj) * 128:(r * 3 + kj) * 128 + 128] = \
                    out_w[32 * (r + 6 * q):32 * (r + 6 * q) + 32, 128 * kj:128 * kj + 128]

    w1_t = np.zeros((128, 3 * MLP), np.float32)
    for k in range(3):
        w1_t[:, MLP * k:MLP * k + MLP] = w1[128 * k:128 * k + 128, :]
    w2_t = np.zeros((128, 12 * C), np.float32)
    for k2 in range(12):
        w2_t[:, C * k2:C * k2 + C] = w2[128 * k2:128 * k2 + 128, :]

    # ybias[n2, :] = P(bv)[n2] @ out_w + out_b
    n2 = np.arange(N)[:, None]
    co = np.arange(12)[None, :]
    hh = (12 * n2 + co) // N                    # (49, 12)
    pb = bv.reshape(12, 32)[hh]                 # (49, 12, 32)
    ybias = pb.reshape(N, C) @ out_w + out_b    # (49, 384)
    yb_t = np.zeros((128, 3 * TW), np.float32)
    tr_ = np.arange(WS)[:, None, None]
    wn_ = np.arange(WT)[None, :, None]
    tc_ = np.arange(WS)[None, None, :]
    pc = (WS * tr_ + tc_ + 0 * wn_).reshape(TW)   # n2 per permuted column
    for kj in range(3):
        blk = ybias[:, 128 * kj:128 * kj + 128].T          # (128, 49)
        yb_t[:, TW * kj:TW * kj + TW] = blk[:, pc]

    b1p = b1 - be2 @ w1                          # mlp bias corrected for +be2 on x2
    be12 = be1 + be2
    cn_t = np.zeros((128, 27), np.float32)
    for m in range(12):
        cn_t[:, m] = b1p[128 * m:128 * m + 128]
    for k in range(3):
        cn_t[:, 12 + k] = b2[128 * k:128 * k + 128]
        cn_t[:, 15 + k] = g1[128 * k:128 * k + 128]
        cn_t[:, 18 + k] = g2[128 * k:128 * k + 128]
        cn_t[:, 21 + k] = be12[128 * k:128 * k + 128]

    ones_t = np.full((128, 128), 1.0 / C, np.float32)
    ident = np.eye(128, dtype=np.float32)

    # column-roll permutation for the input transposes
    perm = np.zeros((WW, WW), np.float32)
    jj = np.arange(WW)
    perm[(jj + DISP) % WW, jj] = 1.0

    return {
        "wv_t": _to_bf(wv_t), "wo_t": _to_bf(wo_t),
        "w1_t": _to_bf(w1_t), "w2_t": _to_bf(w2_t),
        "yb_t": np.ascontiguousarray(yb_t), "cn_t": np.ascontiguousarray(cn_t),
        "ones_t": _to_bf(ones_t), "identb_t": _to_bf(ident),
        "perm_t": _to_bf(perm),
    }


_CACHE = {}


def kernel(**inputs):
    if "nc" not in _CACHE:
        _CACHE["nc"] = build()
    nc = _CACHE["nc"]
    x = np.asarray(inputs["x"], dtype=np.float32).astype(BF)
    base = prep_inputs(inputs)
    in_maps = []
    for c in range(NCORES):
        m = dict(base)
        m["x"] = np.ascontiguousarray(
            x[IMGS * c:IMGS * (c + 1)].reshape(IMGS * NTOK, C))
        in_maps.append(m)
    import os
    trace = bool(int(os.environ.get("KERNEL_TRACE", "0")))
    res = run_bass_kernel_spmd(nc, in_maps, core_ids=list(range(NCORES)),
                               trace=trace)
    _CACHE["last_res"] = res
    out = np.concatenate([r["out"].reshape(IMGS, HH, WW, C) for r in res.results],
                         axis=0)
    return out



# revision 37
# speedup vs baseline: 1.2067x; 1.0047x over previous
"""Trainium2 Bass kernel v2 for nn_Encoder_59219009077683 (Swin-style block).

Math shortcut (from baseline): softmax row-sums are 1, so attention output = v
and the whole block is: window-gather -> V-proj -> fixed permutation (L_d
layout) -> out-proj -> LN1+skip -> MLP -> LN2+skip -> window-scatter.

v2 structural changes vs baseline:
  - gather/scatter via ONE SWDGE indirect DMA per tile (host-built index
    table; the cyclic roll/wrap is baked into the indices)
  - bf16 for all SBUF intermediates and matmuls (tolerance is 2e-2)
  - K=64 out-projection: L_d stored twice (quadrant q holds T[d, c+6q]) so
    the 12 K=32 chunk-matmuls become 6 K=64 matmuls per output block
  - host-folded biases: v-bias+out-bias -> ybias table added at y-evac;
    norm1_b+norm2_b added at x-transpose evac; mlp_b1 corrected by
    -W1^T norm2_b so the norm2 beta rides the x2 skip path
  - rsqrt via tensor_scalar(pow -0.5) on DVE: scalar engine runs only
    Gelu/Identity -> a single activation-table load
  - elementwise spread across DVE / Act / GpSimd
"""
import numpy as np
from contextlib import ExitStack

import concourse.bass as bass
import concourse.bacc as bacc
import concourse.tile as tile
from concourse import mybir
from concourse.bass_utils import run_bass_kernel_spmd, ml_dtypes

F32 = mybir.dt.float32
BF16 = mybir.dt.bfloat16
U32 = mybir.dt.uint32
AF = mybir.ActivationFunctionType
OP = mybir.AluOpType

B, HH, WW, C = 32, 56, 56, 384
NH, HD, WS, DISP, MLP = 12, 32, 7, 3, 1536
NWS = 8
N = 49
J = NH * N            # 588
NCORES = 8
IMGS = B // NCORES
WT = 8                # windows per tile (= one window row)
TW = WT * N           # 392
NTOK = HH * WW        # tokens per image
EPS = 1e-5
LDPAD = 16
LDW = LDPAD + J * WT + 8

BF = ml_dtypes.bfloat16


def _ap(t, offset, dims):
    tt = t.tensor if hasattr(t, "tensor") else t
    return bass.AP(tensor=tt, offset=offset, ap=[list(d) for d in dims])


def build():
    nc = bacc.Bacc("TRN2", target_bir_lowering=False, debug=False, num_devices=NCORES)
    x_d = nc.dram_tensor("x", [IMGS * NTOK, C], BF16, kind="ExternalInput")
    wv_d = nc.dram_tensor("wv_t", [128, 3 * C], BF16, kind="ExternalInput")
    wo_d = nc.dram_tensor("wo_t", [64, 18 * 128], BF16, kind="ExternalInput")
    w1_d = nc.dram_tensor("w1_t", [128, 3 * MLP], BF16, kind="ExternalInput")
    w2_d = nc.dram_tensor("w2_t", [128, 12 * C], BF16, kind="ExternalInput")
    yb_d = nc.dram_tensor("yb_t", [128, 3 * TW], F32, kind="ExternalInput")
    cn_d = nc.dram_tensor("cn_t", [128, 27], F32, kind="ExternalInput")
    on_d = nc.dram_tensor("ones_t", [128, 128], BF16, kind="ExternalInput")
    idb_d = nc.dram_tensor("identb_t", [128, 128], BF16, kind="ExternalInput")
    pm_d = nc.dram_tensor("perm_t", [WW, WW], BF16, kind="ExternalInput")
    out_d = nc.dram_tensor("out", [IMGS * NTOK, C], F32, kind="ExternalOutput")

    with tile.TileContext(nc) as tc, ExitStack() as ctx:
        wpool = ctx.enter_context(tc.tile_pool(name="w", bufs=1))
        stage_pool = ctx.enter_context(tc.tile_pool(name="stage", bufs=4))
        xt_pool = ctx.enter_context(tc.tile_pool(name="xt", bufs=4))
        vt_pool = ctx.enter_context(tc.tile_pool(name="vt", bufs=3))
        ld_pool = ctx.enter_context(tc.tile_pool(name="ld", bufs=2))
        y_pool = ctx.enter_context(tc.tile_pool(name="y", bufs=3))
        x2_pool = ctx.enter_context(tc.tile_pool(name="x2", bufs=4))
        h_pool = ctx.enter_context(tc.tile_pool(name="h", bufs=3))
        h2_pool = ctx.enter_context(tc.tile_pool(name="h2", bufs=3))
        oc_pool = ctx.enter_context(tc.tile_pool(name="oc", bufs=3))
        ot_pool = ctx.enter_context(tc.tile_pool(name="ot", bufs=3))
        sm_pool = ctx.enter_context(tc.tile_pool(name="sm", bufs=5))
        ps_t = ctx.enter_context(tc.tile_pool(name="pst", bufs=1, space="PSUM"))
        ps_a = ctx.enter_context(tc.tile_pool(name="psa", bufs=2, space="PSUM"))
        ps_m = ctx.enter_context(tc.tile_pool(name="psm", bufs=2, space="PSUM"))
        ps_s = ctx.enter_context(tc.tile_pool(name="pss", bufs=1, space="PSUM"))
        ps_o = ctx.enter_context(tc.tile_pool(name="pso", bufs=2, space="PSUM"))

        # ---------- one-time setup: weights straight from host layout ----------
        wv_b = wpool.tile([128, 3 * C], BF16)
        wo_b = wpool.tile([64, 18 * 128], BF16)
        w1_b = wpool.tile([128, 3 * MLP], BF16)
        w2_b = wpool.tile([128, 12 * C], BF16)
        yb_sb = wpool.tile([128, 3 * TW], F32)
        cn_sb = wpool.tile([128, 27], F32)
        ones_b = wpool.tile([128, 128], BF16)
        identb = wpool.tile([128, 128], BF16)
        perm56 = wpool.tile([WW, WW], BF16)
        eps_t = wpool.tile([128, 1], F32)
        zero_t = wpool.tile([128, 1], F32)
        nc.vector.memset(eps_t[:, :], EPS)
        nc.vector.memset(zero_t[:, :], 0.0)
        nc.sync.dma_start(perm56[:, :], pm_d[:, :])
        nc.sync.dma_start(wv_b[:, :], wv_d[:, :])
        nc.sync.dma_start(wo_b[:, :], wo_d[:, :])
        nc.sync.dma_start(w1_b[:, :], w1_d[:, :])
        nc.sync.dma_start(w2_b[:, :], w2_d[:, :])
        nc.sync.dma_start(yb_sb[:, :], yb_d[:, :])
        nc.sync.dma_start(cn_sb[:, :], cn_d[:, :])
        nc.sync.dma_start(ones_b[:, :], on_d[:, :])
        nc.sync.dma_start(identb[:, :], idb_d[:, :])
        # PE must observe the identity via a transpose once before the loop so
        # later transposes carry <=1 sync wait (bare LDWEIGHTS limitation).
        dmy = ps_t.tile([128, 392], BF16, tag="t")
        nc.tensor.transpose(dmy[0:WW, 0:WW], perm56[0:WW, 0:WW], perm56[0:WW, 0:WW])
        dmy2 = ps_t.tile([128, 392], BF16, tag="t")
        nc.tensor.transpose(dmy2[0:128, 0:128], identb[:, :], identb[:, :])

        # ---------- software-pipelined main loop ----------
        # tile index t = img*8 + wr; phases skewed across iterations so every
        # engine (PE especially) always has ready work.
        NT = IMGS * NWS
        st = {}   # per-tile live state

        def do_gather(t):
            img, wr = divmod(t, NWS)
            stage = stage_pool.tile([WW, WS * C], BF16, tag="stage")
            pst = stage[:, :].ap[0][0]
            soff = stage[:, :].offset
            rows = [(WS * wr + DISP, 0, WS)] if wr < NWS - 1 else \
                [(52, 0, 4), (0, 4, 3)]
            for (r0, tr0, ntr) in rows:
                src = _ap(x_d, (img * NTOK + r0 * WW) * C,
                          [[C, WW], [WW * C, ntr], [1, C]])
                dst = _ap(stage, soff + tr0 * C,
                          [[pst, WW], [C, ntr], [1, C]])
                nc.sync.dma_start(dst, src)
            st[t] = {"stage": stage}

        def do_p1(t):
            """transposes + V-proj + ld build"""
            d = st[t]
            stage = d.pop("stage")
            xt = xt_pool.tile([128, 3 * TW], BF16, tag="xt")
            for k in range(3):
                xps = ps_t.tile([128, 392], BF16, tag="t")
                pxp = xps[:, :].ap[0][0]
                xpo = xps[:, :].offset
                for tr in range(WS):
                    nc.tensor.transpose(
                        xps[:, WW * tr:WW * tr + WW],
                        stage[0:WW, tr * C + 128 * k:tr * C + 128 * k + 128],
                        perm56[0:WW, 0:WW])
                # token order everywhere downstream: col = 56*tr + 7*win + tc
                nc.vector.tensor_scalar(xt[:, TW * k:TW * k + TW], xps[:, :],
                                        cn_sb[:, 21 + k:22 + k], None, op0=OP.add)
            vt = vt_pool.tile([128, 3 * TW], BF16, tag="vt")
            pvt = vt[:, :].ap[0][0]
            vtoff = vt[:, :].offset
            for kv in range(3):
                vps = ps_a.tile([128, TW], F32, tag="a")
                for k in range(3):
                    nc.tensor.matmul(vps[:, :],
                                     wv_b[:, C * k + 128 * kv:C * k + 128 * kv + 128],
                                     xt[:, TW * k:TW * k + TW],
                                     start=(k == 0), stop=(k == 2))
                pps = vps[:, :].ap[0][0]
                nc.scalar.activation(
                    _ap(vt, vtoff + N * kv,
                        [[pvt, 128], [WS, WS], [3 * N, WT], [1, WS]]),
                    _ap(vps, vps[:, :].offset,
                        [[pps, 128], [WW, WS], [WS, WT], [1, WS]]),
                    AF.Identity, bias=zero_t[:, :], scale=1.0)
            ld = ld_pool.tile([64, LDW], BF16, tag="ld")
            pld = ld[:, :].ap[0][0]
            ldoff = ld[:, :].offset
            for s4 in range(4):
                for q in range(2):
                    src = _ap(vt, vtoff + (32 * s4) * pvt,
                              [[pvt, 32], [N, 24], [1, N]])
                    dst = _ap(ld, ldoff + (32 * q) * pld + LDPAD + N * s4 - 6 * q,
                              [[pld, 32], [4 * N, 24], [1, N]])
                    nc.sync.dma_start(dst, src)
            d["xt"] = xt
            d["ld"] = (ld, pld, ldoff)

        def do_p2a(t):
            """out-proj + ybias evac + ysq"""
            d = st[t]
            ld, pld, ldoff = d.pop("ld")
            y = y_pool.tile([128, 3 * TW], BF16, tag="y")
            for kj in range(3):
                yps = ps_o.tile([128, TW], F32, tag="o")
                for r in range(6):
                    rhs = _ap(ld, ldoff + LDPAD + r,
                              [[pld, 64], [84, WS], [J, WT], [12, WS]])
                    nc.tensor.matmul(yps[:, :],
                                     wo_b[:, (r * 3 + kj) * 128:(r * 3 + kj) * 128 + 128],
                                     rhs, start=(r == 0), stop=(r == 5))
                nc.vector.tensor_add(y[:, TW * kj:TW * kj + TW], yps[:, :],
                                     yb_sb[:, TW * kj:TW * kj + TW])
            ysq = sm_pool.tile([128, 3 * TW], BF16, tag="ysq")
            for k in range(3):
                nc.vector.tensor_mul(ysq[:, TW * k:TW * k + TW],
                                     y[:, TW * k:TW * k + TW], y[:, TW * k:TW * k + TW])
            d["y"] = y
            d["ysq"] = ysq

        def _ln(y, ysq, gcol, skip, out):
            """matmul sums -> stats -> apply; out = (y-mu)*rst*g + skip."""
            s1 = ps_s.tile([128, TW], F32, tag="s")
            s2 = ps_s.tile([128, TW], F32, tag="s")
            for k in range(3):
                nc.tensor.matmul(s1[:, :], ones_b[:, :], y[:, TW * k:TW * k + TW],
                                 start=(k == 0), stop=(k == 2))
                nc.tensor.matmul(s2[:, :], ones_b[:, :], ysq[:, TW * k:TW * k + TW],
                                 start=(k == 0), stop=(k == 2))
            mu = sm_pool.tile([128, TW], BF16, tag="mu")
            m2 = sm_pool.tile([128, TW], F32, tag="m2")
            veps = sm_pool.tile([128, TW], F32, tag="veps")
            rst = sm_pool.tile([128, TW], F32, tag="rst")
            nc.vector.tensor_copy(mu[:, :], s1[:, :])
            nc.scalar.square(m2[:, :], mu[:, :])
            nc.vector.tensor_sub(veps[:, :], s2[:, :], m2[:, :])
            # std = sqrt(var + eps) on Act (eps folded into the bias), then
            # 1/std via the fast custom-DVE reciprocal (~51 ULP, plenty here)
            nc.scalar.activation(veps[:, :], veps[:, :], AF.Sqrt,
                                 bias=eps_t[:, :], scale=1.0)
            nc.vector.reciprocal_approx_fast(rst[:, :], veps[:, :])
            rstb = sm_pool.tile([128, TW], BF16, tag="rstb")
            nc.vector.tensor_copy(rstb[:, :], rst[:, :])
            for k in range(3):
                u = sm_pool.tile([128, TW], BF16, tag="u")
                v = sm_pool.tile([128, TW], BF16, tag="v")
                nc.vector.tensor_sub(u[:, :], y[:, TW * k:TW * k + TW], mu[:, :])
                nc.vector.tensor_mul(v[:, :], u[:, :], rstb[:, :])
                nc.vector.scalar_tensor_tensor(out[:, TW * k:TW * k + TW],
                                               v[:, :],
                                               cn_sb[:, gcol + k:gcol + k + 1],
                                               skip[:, TW * k:TW * k + TW],
                                               op0=OP.mult, op1=OP.add)

        def do_p2b(t):
            d = st[t]
            x2 = x2_pool.tile([128, 3 * TW], BF16, tag="x2")
            _ln(d.pop("y"), d.pop("ysq"), 15, d.pop("xt"), x2)
            d["x2"] = x2

        def do_p3(t):
            d = st[t]
            x2 = d["x2"]
            hsb = h_pool.tile([128, 12 * TW], BF16, tag="h")
            for m in range(12):
                hps = ps_m.tile([128, TW], F32, tag="m")
                for k in range(3):
                    nc.tensor.matmul(hps[:, :],
                                     w1_b[:, MLP * k + 128 * m:MLP * k + 128 * m + 128],
                                     x2[:, TW * k:TW * k + TW],
                                     start=(k == 0), stop=(k == 2))
                nc.scalar.activation(hsb[:, TW * m:TW * m + TW], hps[:, :],
                                     AF.Gelu, bias=cn_sb[:, m:m + 1], scale=1.0)
            d["hsb"] = hsb

        def do_p4a(t):
            d = st[t]
            hsb = d.pop("hsb")
            h2 = h2_pool.tile([128, 3 * TW], BF16, tag="h2")
            for kj in range(3):
                h2ps = ps_a.tile([128, TW], F32, tag="a")
                for k2 in range(12):
                    nc.tensor.matmul(h2ps[:, :],
                                     w2_b[:, C * k2 + 128 * kj:C * k2 + 128 * kj + 128],
                                     hsb[:, TW * k2:TW * k2 + TW],
                                     start=(k2 == 0), stop=(k2 == 11))
                nc.scalar.activation(h2[:, TW * kj:TW * kj + TW], h2ps[:, :],
                                     AF.Identity, bias=cn_sb[:, 12 + kj:13 + kj],
                                     scale=1.0)
            hsq = sm_pool.tile([128, 3 * TW], BF16, tag="ysq")
            for k in range(3):
                nc.vector.tensor_mul(hsq[:, TW * k:TW * k + TW],
                                     h2[:, TW * k:TW * k + TW], h2[:, TW * k:TW * k + TW])
            d["h2"] = h2
            d["hsq"] = hsq

        def do_p4b(t):
            d = st[t]
            ocm = oc_pool.tile([128, 3 * TW], BF16, tag="oc")
            _ln(d.pop("h2"), d.pop("hsq"), 18, d.pop("x2"), ocm)
            d["ocm"] = ocm

        def do_p5(t):
            d = st.pop(t)
            img, wr = divmod(t, NWS)
            ocm = d["ocm"]
            poc = ocm[:, :].ap[0][0]
            oco = ocm[:, :].offset
            # paired: otr[56*(tr%2)+x, 384*(tr//2)+c]; 12 transposes not 21
            otr = ot_pool.tile([112, 4 * C], BF16, tag="ot")
            pot = otr[:, :].ap[0][0]
            poff = otr[:, :].offset
            for p in range(4):
                prow = 112 if p < 3 else 56
                ops_t = ps_t.tile([128, 392], BF16, tag="t")
                for k in range(3):
                    nc.tensor.transpose(
                        ops_t[0:prow, 128 * k:128 * k + 128],
                        ocm[:, TW * k + 112 * p:TW * k + 112 * p + prow],
                        identb[:, :])
                nc.vector.tensor_copy(otr[0:prow, p * C:p * C + C],
                                      ops_t[0:prow, 0:C])

            def scat(prow0, npair, r0, odd):
                base = poff + (56 * odd) * pot + 384 * prow0
                dst0 = (img * NTOK + r0 * WW) * C
                src = _ap(otr, base, [[pot, 53], [C, npair], [1, C]])
                dst = _ap(out_d, dst0 + DISP * C,
                          [[C, 53], [2 * WW * C, npair], [1, C]])
                nc.gpsimd.dma_start(dst, src)
                src = _ap(otr, base + 53 * pot, [[pot, 3], [C, npair], [1, C]])
                dst = _ap(out_d, dst0, [[C, 3], [2 * WW * C, npair], [1, C]])
                nc.gpsimd.dma_start(dst, src)

            if wr < NWS - 1:
                r0 = WS * wr + DISP
                scat(0, 4, r0, 0)
                scat(0, 3, r0 + 1, 1)
            else:
                scat(0, 2, 52, 0)
                scat(0, 2, 53, 1)
                scat(2, 2, 0, 0)
                scat(2, 1, 1, 1)

        for i in range(NT + 7):
            if i < NT:
                do_gather(i)
            if 0 <= i - 2 < NT:
                do_p2a(i - 2)
            if 0 <= i - 1 < NT:
                do_p1(i - 1)
            if 0 <= i - 5 < NT:
                do_p4a(i - 5)
            if 0 <= i - 7 < NT:
                do_p5(i - 7)
            if 0 <= i - 3 < NT:
                do_p2b(i - 3)
            if 0 <= i - 4 < NT:
                do_p3(i - 4)
            if 0 <= i - 6 < NT:
                do_p4b(i - 6)
    nc.compile()
    return nc


def _to_bf(a):
    return np.ascontiguousarray(a.astype(BF))


def prep_inputs(inputs):
    """Host-side weight/bias reformatting (layout only + tiny bias algebra)."""
    f = {k: np.asarray(v, dtype=np.float32) for k, v in inputs.items()}
    qkv_w, qkv_b = f["qkv_w"], f["qkv_b"]
    out_w, out_b = f["out_w"], f["out_b"]
    w1, b1, w2, b2 = f["mlp_w1"], f["mlp_b1"], f["mlp_w2"], f["mlp_b2"]
    g1, be1 = f["norm1_g"], f["norm1_b"]
    g2, be2 = f["norm2_g"], f["norm2_b"]

    wv = qkv_w[:, 2 * C:3 * C]
    bv = qkv_b[2 * C:3 * C]

    wv_t = np.zeros((128, 3 * C), np.float32)
    for k in range(3):
        wv_t[:, C * k:C * k + C] = wv[128 * k:128 * k + 128, :]

    wo_t = np.zeros((64, 18 * 128), np.float32)
    for r in range(6):
        for q in range(2):
            for kj in range(3):
                wo_t[32 * q:32 * q + 32, (r * 3 + kj) * 128:(r * 3 + kj) * 128 + 128] = \
                    out_w[32 * (r + 6 * q):32 * (r + 6 * q) + 32, 128 * kj:128 * kj + 128]

    w1_t = np.zeros((128, 3 * MLP), np.float32)
    for k in range(3):
        w1_t[:, MLP * k:MLP * k + MLP] = w1[128 * k:128 * k + 128, :]
    w2_t = np.zeros((128, 12 * C), np.float32)
    for k2 in range(12):
        w2_t[:, C * k2:C * k2 + C] = w2[128 * k2:128 * k2 + 128, :]

    # ybias[n2, :] = P(bv)[n2] @ out_w + out_b
    n2 = np.arange(N)[:, None]
    co = np.arange(12)[None, :]
    hh = (12 * n2 + co) // N                    # (49, 12)
    pb = bv.reshape(12, 32)[hh]                 # (49, 12, 32)
    ybias = pb.reshape(N, C) @ out_w + out_b    # (49, 384)
    yb_t = np.zeros((128, 3 * TW), np.float32)
    tr_ = np.arange(WS)[:, None, None]
    wn_ = np.arange(WT)[None, :, None]
    tc_ = np.arange(WS)[None, None, :]
    pc = (WS * tr_ + tc_ + 0 * wn_).reshape(TW)   # n2 per permuted column
    for kj in range(3):
        blk = ybias[:, 128 * kj:128 * kj + 128].T          # (128, 49)
        yb_t[:, TW * kj:TW * kj + TW] = blk[:, pc]

    b1p = b1 - be2 @ w1                          # mlp bias corrected for +be2 on x2
    be12 = be1 + be2
    cn_t = np.zeros((128, 27), np.float32)
    for m in range(12):
        cn_t[:, m] = b1p[128 * m:128 * m + 128]
    for k in range(3):
        cn_t[:, 12 + k] = b2[128 * k:128 * k + 128]
        cn_t[:, 15 + k] = g1[128 * k:128 * k + 128]
        cn_t[:, 18 + k] = g2[128 * k:128 * k + 128]
        cn_t[:, 21 + k] = be12[128 * k:128 * k + 128]

    ones_t = np.full((128, 128), 1.0 / C, np.float32)
    ident = np.eye(128, dtype=np.float32)

    # column-roll permutation for the input transposes
    perm = np.zeros((WW, WW), np.float32)
    jj = np.arange(WW)
    perm[(jj + DISP) % WW, jj] = 1.0

    return {
        "wv_t": _to_bf(wv_t), "wo_t": _to_bf(wo_t),
        "w1_t": _to_bf(w1_t), "w2_t": _to_bf(w2_t),
        "yb_t": np.ascontiguousarray(yb_t), "cn_t": np.ascontiguousarray(cn_t),
        "ones_t": _to_bf(ones_t), "identb_t": _to_bf(ident),
        "perm_t": _to_bf(perm),
    }


_CACHE = {}


def kernel(**inputs):
    if "nc" not in _CACHE:
        _CACHE["nc"] = build()
    nc = _CACHE["nc"]
    x = np.asarray(inputs["x"], dtype=np.float32).astype(BF)
    base = prep_inputs(inputs)
    in_maps = []
    for c in range(NCORES):
        m = dict(base)
        m["x"] = np.ascontiguousarray(
            x[IMGS * c:IMGS * (c + 1)].reshape(IMGS * NTOK, C))
        in_maps.append(m)
    import os
    trace = bool(int(os.environ.get("KERNEL_TRACE", "0")))
    res = run_bass_kernel_spmd(nc, in_maps, core_ids=list(range(NCORES)),
                               trace=trace)
    _CACHE["last_res"] = res
    out = np.concatenate([r["out"].reshape(IMGS, HH, WW, C) for r in res.results],
                         axis=0)
    return out



# revision 39
# speedup vs baseline: 1.2189x; 1.0101x over previous
"""Trainium2 Bass kernel v2 for nn_Encoder_59219009077683 (Swin-style block).

Math shortcut (from baseline): softmax row-sums are 1, so attention output = v
and the whole block is: window-gather -> V-proj -> fixed permutation (L_d
layout) -> out-proj -> LN1+skip -> MLP -> LN2+skip -> window-scatter.

v2 structural changes vs baseline:
  - gather/scatter via ONE SWDGE indirect DMA per tile (host-built index
    table; the cyclic roll/wrap is baked into the indices)
  - bf16 for all SBUF intermediates and matmuls (tolerance is 2e-2)
  - K=64 out-projection: L_d stored twice (quadrant q holds T[d, c+6q]) so
    the 12 K=32 chunk-matmuls become 6 K=64 matmuls per output block
  - host-folded biases: v-bias+out-bias -> ybias table added at y-evac;
    norm1_b+norm2_b added at x-transpose evac; mlp_b1 corrected by
    -W1^T norm2_b so the norm2 beta rides the x2 skip path
  - rsqrt via tensor_scalar(pow -0.5) on DVE: scalar engine runs only
    Gelu/Identity -> a single activation-table load
  - elementwise spread across DVE / Act / GpSimd
"""
import numpy as np
from contextlib import ExitStack

import concourse.bass as bass
import concourse.bacc as bacc
import concourse.tile as tile
from concourse import mybir
from concourse.bass_utils import run_bass_kernel_spmd, ml_dtypes

F32 = mybir.dt.float32
BF16 = mybir.dt.bfloat16
U32 = mybir.dt.uint32
AF = mybir.ActivationFunctionType
OP = mybir.AluOpType

B, HH, WW, C = 32, 56, 56, 384
NH, HD, WS, DISP, MLP = 12, 32, 7, 3, 1536
NWS = 8
N = 49
J = NH * N            # 588
NCORES = 8
IMGS = B // NCORES
WT = 8                # windows per tile (= one window row)
TW = WT * N           # 392
NTOK = HH * WW        # tokens per image
EPS = 1e-5
LDPAD = 16
LDW = LDPAD + J * WT + 8

BF = ml_dtypes.bfloat16


def _ap(t, offset, dims):
    tt = t.tensor if hasattr(t, "tensor") else t
    return bass.AP(tensor=tt, offset=offset, ap=[list(d) for d in dims])


def build():
    nc = bacc.Bacc("TRN2", target_bir_lowering=False, debug=False, num_devices=NCORES)
    x_d = nc.dram_tensor("x", [IMGS * NTOK, C], BF16, kind="ExternalInput")
    wv_d = nc.dram_tensor("wv_t", [128, 3 * C], BF16, kind="ExternalInput")
    wo_d = nc.dram_tensor("wo_t", [64, 18 * 128], BF16, kind="ExternalInput")
    w1_d = nc.dram_tensor("w1_t", [128, 3 * MLP], BF16, kind="ExternalInput")
    w2_d = nc.dram_tensor("w2_t", [128, 12 * C], BF16, kind="ExternalInput")
    yb_d = nc.dram_tensor("yb_t", [128, 3 * TW], F32, kind="ExternalInput")
    cn_d = nc.dram_tensor("cn_t", [128, 27], F32, kind="ExternalInput")
    on_d = nc.dram_tensor("ones_t", [128, 128], BF16, kind="ExternalInput")
    idb_d = nc.dram_tensor("identb_t", [128, 128], BF16, kind="ExternalInput")
    pm_d = nc.dram_tensor("perm_t", [WW, WW], BF16, kind="ExternalInput")
    out_d = nc.dram_tensor("out", [IMGS * NTOK, C], F32, kind="ExternalOutput")

    with tile.TileContext(nc) as tc, ExitStack() as ctx:
        wpool = ctx.enter_context(tc.tile_pool(name="w", bufs=1))
        stage_pool = ctx.enter_context(tc.tile_pool(name="stage", bufs=4))
        xt_pool = ctx.enter_context(tc.tile_pool(name="xt", bufs=4))
        vt_pool = ctx.enter_context(tc.tile_pool(name="vt", bufs=3))
        ld_pool = ctx.enter_context(tc.tile_pool(name="ld", bufs=2))
        y_pool = ctx.enter_context(tc.tile_pool(name="y", bufs=3))
        x2_pool = ctx.enter_context(tc.tile_pool(name="x2", bufs=4))
        h_pool = ctx.enter_context(tc.tile_pool(name="h", bufs=3))
        h2_pool = ctx.enter_context(tc.tile_pool(name="h2", bufs=3))
        oc_pool = ctx.enter_context(tc.tile_pool(name="oc", bufs=3))
        ot_pool = ctx.enter_context(tc.tile_pool(name="ot", bufs=3))
        sm_pool = ctx.enter_context(tc.tile_pool(name="sm", bufs=5))
        ps_t = ctx.enter_context(tc.tile_pool(name="pst", bufs=1, space="PSUM"))
        ps_a = ctx.enter_context(tc.tile_pool(name="psa", bufs=2, space="PSUM"))
        ps_m = ctx.enter_context(tc.tile_pool(name="psm", bufs=2, space="PSUM"))
        ps_s = ctx.enter_context(tc.tile_pool(name="pss", bufs=1, space="PSUM"))
        ps_o = ctx.enter_context(tc.tile_pool(name="pso", bufs=2, space="PSUM"))

        # ---------- one-time setup: weights straight from host layout ----------
        wv_b = wpool.tile([128, 3 * C], BF16)
        wo_b = wpool.tile([64, 18 * 128], BF16)
        w1_b = wpool.tile([128, 3 * MLP], BF16)
        w2_b = wpool.tile([128, 12 * C], BF16)
        yb_sb = wpool.tile([128, 3 * TW], F32)
        cn_sb = wpool.tile([128, 27], F32)
        ones_b = wpool.tile([128, 128], BF16)
        identb = wpool.tile([128, 128], BF16)
        perm56 = wpool.tile([WW, WW], BF16)
        eps_t = wpool.tile([128, 1], F32)
        zero_t = wpool.tile([128, 1], F32)
        nc.vector.memset(eps_t[:, :], EPS)
        nc.vector.memset(zero_t[:, :], 0.0)
        nc.sync.dma_start(perm56[:, :], pm_d[:, :])
        nc.sync.dma_start(wv_b[:, :], wv_d[:, :])
        nc.sync.dma_start(wo_b[:, :], wo_d[:, :])
        nc.sync.dma_start(w1_b[:, :], w1_d[:, :])
        nc.sync.dma_start(w2_b[:, :], w2_d[:, :])
        nc.sync.dma_start(yb_sb[:, :], yb_d[:, :])
        nc.sync.dma_start(cn_sb[:, :], cn_d[:, :])
        nc.sync.dma_start(ones_b[:, :], on_d[:, :])
        nc.sync.dma_start(identb[:, :], idb_d[:, :])
        # PE must observe the identity via a transpose once before the loop so
        # later transposes carry <=1 sync wait (bare LDWEIGHTS limitation).
        dmy = ps_t.tile([128, 392], BF16, tag="t")
        nc.tensor.transpose(dmy[0:WW, 0:WW], perm56[0:WW, 0:WW], perm56[0:WW, 0:WW])
        dmy2 = ps_t.tile([128, 392], BF16, tag="t")
        nc.tensor.transpose(dmy2[0:128, 0:128], identb[:, :], identb[:, :])

        # ---------- software-pipelined main loop ----------
        # tile index t = img*8 + wr; phases skewed across iterations so every
        # engine (PE especially) always has ready work.
        NT = IMGS * NWS
        st = {}   # per-tile live state

        def do_gather(t):
            img, wr = divmod(t, NWS)
            stage = stage_pool.tile([WW, WS * C], BF16, tag="stage")
            pst = stage[:, :].ap[0][0]
            soff = stage[:, :].offset
            rows = [(WS * wr + DISP, 0, WS)] if wr < NWS - 1 else \
                [(52, 0, 4), (0, 4, 3)]
            for (r0, tr0, ntr) in rows:
                src = _ap(x_d, (img * NTOK + r0 * WW) * C,
                          [[C, WW], [WW * C, ntr], [1, C]])
                dst = _ap(stage, soff + tr0 * C,
                          [[pst, WW], [C, ntr], [1, C]])
                nc.sync.dma_start(dst, src)
            st[t] = {"stage": stage}

        def do_p1(t):
            """transposes + V-proj + ld build"""
            d = st[t]
            stage = d.pop("stage")
            xt = xt_pool.tile([128, 3 * TW], BF16, tag="xt")
            for k in range(3):
                xps = ps_t.tile([128, 392], BF16, tag="t")
                pxp = xps[:, :].ap[0][0]
                xpo = xps[:, :].offset
                for tr in range(WS):
                    nc.tensor.transpose(
                        xps[:, WW * tr:WW * tr + WW],
                        stage[0:WW, tr * C + 128 * k:tr * C + 128 * k + 128],
                        perm56[0:WW, 0:WW])
                # token order everywhere downstream: col = 56*tr + 7*win + tc
                nc.vector.tensor_scalar(xt[:, TW * k:TW * k + TW], xps[:, :],
                                        cn_sb[:, 21 + k:22 + k], None, op0=OP.add)
            vt = vt_pool.tile([128, 3 * TW], BF16, tag="vt")
            pvt = vt[:, :].ap[0][0]
            vtoff = vt[:, :].offset
            for kv in range(3):
                vps = ps_a.tile([128, TW], F32, tag="a")
                for k in range(3):
                    nc.tensor.matmul(vps[:, :],
                                     wv_b[:, C * k + 128 * kv:C * k + 128 * kv + 128],
                                     xt[:, TW * k:TW * k + TW],
                                     start=(k == 0), stop=(k == 2))
                pps = vps[:, :].ap[0][0]
                nc.scalar.activation(
                    _ap(vt, vtoff + N * kv,
                        [[pvt, 128], [WS, WS], [3 * N, WT], [1, WS]]),
                    _ap(vps, vps[:, :].offset,
                        [[pps, 128], [WW, WS], [WS, WT], [1, WS]]),
                    AF.Identity, bias=zero_t[:, :], scale=1.0)
            ld = ld_pool.tile([64, LDW], BF16, tag="ld")
            pld = ld[:, :].ap[0][0]
            ldoff = ld[:, :].offset
            for s4 in range(4):
                for q in range(2):
                    src = _ap(vt, vtoff + (32 * s4) * pvt,
                              [[pvt, 32], [N, 24], [1, N]])
                    dst = _ap(ld, ldoff + (32 * q) * pld + LDPAD + N * s4 - 6 * q,
                              [[pld, 32], [4 * N, 24], [1, N]])
                    nc.sync.dma_start(dst, src)
            d["xt"] = xt
            d["ld"] = (ld, pld, ldoff)

        def do_p2a(t):
            """out-proj + ybias evac + ysq"""
            d = st[t]
            ld, pld, ldoff = d.pop("ld")
            y = y_pool.tile([128, 3 * TW], BF16, tag="y")
            for kj in range(3):
                yps = ps_o.tile([128, TW], F32, tag="o")
                for r in range(6):
                    rhs = _ap(ld, ldoff + LDPAD + r,
                              [[pld, 64], [84, WS], [J, WT], [12, WS]])
                    nc.tensor.matmul(yps[:, :],
                                     wo_b[:, (r * 3 + kj) * 128:(r * 3 + kj) * 128 + 128],
                                     rhs, start=(r == 0), stop=(r == 5))
                nc.vector.tensor_add(y[:, TW * kj:TW * kj + TW], yps[:, :],
                                     yb_sb[:, TW * kj:TW * kj + TW])
            ysq = sm_pool.tile([128, 3 * TW], BF16, tag="ysq")
            for k in range(3):
                nc.vector.tensor_mul(ysq[:, TW * k:TW * k + TW],
                                     y[:, TW * k:TW * k + TW], y[:, TW * k:TW * k + TW])
            d["y"] = y
            d["ysq"] = ysq

        def _ln(y, ysq, gcol, skip, out):
            """matmul sums -> stats -> apply; out = (y-mu)*rst*g + skip."""
            s1 = ps_s.tile([128, TW], F32, tag="s")
            s2 = ps_s.tile([128, TW], F32, tag="s")
            for k in range(3):
                nc.tensor.matmul(s1[:, :], ones_b[:, :], y[:, TW * k:TW * k + TW],
                                 start=(k == 0), stop=(k == 2))
                nc.tensor.matmul(s2[:, :], ones_b[:, :], ysq[:, TW * k:TW * k + TW],
                                 start=(k == 0), stop=(k == 2))
            mu = sm_pool.tile([128, TW], BF16, tag="mu")
            m2 = sm_pool.tile([128, TW], BF16, tag="m2")
            veps = sm_pool.tile([128, TW], F32, tag="veps")
            rst = sm_pool.tile([128, TW], F32, tag="rst")
            nc.vector.tensor_copy(mu[:, :], s1[:, :])
            nc.vector.tensor_mul(m2[:, :], mu[:, :], mu[:, :])
            nc.vector.tensor_sub(veps[:, :], s2[:, :], m2[:, :])
            # std = sqrt(var + eps) on Act (eps folded into the bias), then
            # 1/std via the fast custom-DVE reciprocal (~51 ULP, plenty here)
            nc.scalar.activation(veps[:, :], veps[:, :], AF.Sqrt,
                                 bias=eps_t[:, :], scale=1.0)
            nc.vector.reciprocal_approx_fast(rst[:, :], veps[:, :])
            rstb = sm_pool.tile([128, TW], BF16, tag="rstb")
            nc.vector.tensor_copy(rstb[:, :], rst[:, :])
            for k in range(3):
                u = sm_pool.tile([128, TW], BF16, tag="u")
                v = sm_pool.tile([128, TW], BF16, tag="v")
                nc.vector.tensor_sub(u[:, :], y[:, TW * k:TW * k + TW], mu[:, :])
                nc.vector.tensor_mul(v[:, :], u[:, :], rstb[:, :])
                nc.vector.scalar_tensor_tensor(out[:, TW * k:TW * k + TW],
                                               v[:, :],
                                               cn_sb[:, gcol + k:gcol + k + 1],
                                               skip[:, TW * k:TW * k + TW],
                                               op0=OP.mult, op1=OP.add)

        def do_p2b(t):
            d = st[t]
            x2 = x2_pool.tile([128, 3 * TW], BF16, tag="x2")
            _ln(d.pop("y"), d.pop("ysq"), 15, d.pop("xt"), x2)
            d["x2"] = x2

        def do_p3(t):
            d = st[t]
            x2 = d["x2"]
            hsb = h_pool.tile([128, 12 * TW], BF16, tag="h")
            for m in range(12):
                hps = ps_m.tile([128, TW], F32, tag="m")
                for k in range(3):
                    nc.tensor.matmul(hps[:, :],
                                     w1_b[:, MLP * k + 128 * m:MLP * k + 128 * m + 128],
                                     x2[:, TW * k:TW * k + TW],
                                     start=(k == 0), stop=(k == 2))
                nc.scalar.activation(hsb[:, TW * m:TW * m + TW], hps[:, :],
                                     AF.Gelu, bias=cn_sb[:, m:m + 1], scale=1.0)
            d["hsb"] = hsb

        def do_p4a(t):
            d = st[t]
            hsb = d.pop("hsb")
            h2 = h2_pool.tile([128, 3 * TW], BF16, tag="h2")
            for kj in range(3):
                h2ps = ps_a.tile([128, TW], F32, tag="a")
                for k2 in range(12):
                    nc.tensor.matmul(h2ps[:, :],
                                     w2_b[:, C * k2 + 128 * kj:C * k2 + 128 * kj + 128],
                                     hsb[:, TW * k2:TW * k2 + TW],
                                     start=(k2 == 0), stop=(k2 == 11))
                nc.scalar.activation(h2[:, TW * kj:TW * kj + TW], h2ps[:, :],
                                     AF.Identity, bias=cn_sb[:, 12 + kj:13 + kj],
                                     scale=1.0)
            hsq = sm_pool.tile([128, 3 * TW], BF16, tag="ysq")
            for k in range(3):
                nc.vector.tensor_mul(hsq[:, TW * k:TW * k + TW],
                                     h2[:, TW * k:TW * k + TW], h2[:, TW * k:TW * k + TW])
            d["h2"] = h2
            d["hsq"] = hsq

        def do_p4b(t):
            d = st[t]
            ocm = oc_pool.tile([128, 3 * TW], BF16, tag="oc")
            _ln(d.pop("h2"), d.pop("hsq"), 18, d.pop("x2"), ocm)
            d["ocm"] = ocm

        def do_p5(t):
            d = st.pop(t)
            img, wr = divmod(t, NWS)
            ocm = d["ocm"]
            poc = ocm[:, :].ap[0][0]
            oco = ocm[:, :].offset
            # paired: otr[56*(tr%2)+x, 384*(tr//2)+c]; 12 transposes not 21
            otr = ot_pool.tile([112, 4 * C], BF16, tag="ot")
            pot = otr[:, :].ap[0][0]
            poff = otr[:, :].offset
            for p in range(4):
                prow = 112 if p < 3 else 56
                ops_t = ps_t.tile([128, 392], BF16, tag="t")
                for k in range(3):
                    nc.tensor.transpose(
                        ops_t[0:prow, 128 * k:128 * k + 128],
                        ocm[:, TW * k + 112 * p:TW * k + 112 * p + prow],
                        identb[:, :])
                nc.vector.tensor_copy(otr[0:prow, p * C:p * C + C],
                                      ops_t[0:prow, 0:C])

            def scat(prow0, npair, r0, odd):
                base = poff + (56 * odd) * pot + 384 * prow0
                dst0 = (img * NTOK + r0 * WW) * C
                src = _ap(otr, base, [[pot, 53], [C, npair], [1, C]])
                dst = _ap(out_d, dst0 + DISP * C,
                          [[C, 53], [2 * WW * C, npair], [1, C]])
                nc.gpsimd.dma_start(dst, src)
                src = _ap(otr, base + 53 * pot, [[pot, 3], [C, npair], [1, C]])
                dst = _ap(out_d, dst0, [[C, 3], [2 * WW * C, npair], [1, C]])
                nc.gpsimd.dma_start(dst, src)

            if wr < NWS - 1:
                r0 = WS * wr + DISP
                scat(0, 4, r0, 0)
                scat(0, 3, r0 + 1, 1)
            else:
                scat(0, 2, 52, 0)
                scat(0, 2, 53, 1)
                scat(2, 2, 0, 0)
                scat(2, 1, 1, 1)

        for i in range(NT + 7):
            if i < NT:
                do_gather(i)
            if 0 <= i - 2 < NT:
                do_p2a(i - 2)
            if 0 <= i - 1 < NT:
                do_p1(i - 1)
            if 0 <= i - 5 < NT:
                do_p4a(i - 5)
            if 0 <= i - 7 < NT:
                do_p5(i - 7)
            if 0 <= i - 3 < NT:
                do_p2b(i - 3)
            if 0 <= i - 4 < NT:
                do_p3(i - 4)
            if 0 <= i - 6 < NT:
                do_p4b(i - 6)
    nc.compile()
    return nc


def _to_bf(a):
    return np.ascontiguousarray(a.astype(BF))


def prep_inputs(inputs):
    """Host-side weight/bias reformatting (layout only + tiny bias algebra)."""
    f = {k: np.asarray(v, dtype=np.float32) for k, v in inputs.items()}
    qkv_w, qkv_b = f["qkv_w"], f["qkv_b"]
    out_w, out_b = f["out_w"], f["out_b"]
    w1, b1, w2, b2 = f["mlp_w1"], f["mlp_b1"], f["mlp_w2"], f["mlp_b2"]
    g1, be1 = f["norm1_g"], f["norm1_b"]
    g2, be2 = f["norm2_g"], f["norm2_b"]

    wv = qkv_w[:, 2 * C:3 * C]
    bv = qkv_b[2 * C:3 * C]

    wv_t = np.zeros((128, 3 * C), np.float32)
    for k in range(3):
        wv_t[:, C * k:C * k + C] = wv[128 * k:128 * k + 128, :]

    wo_t = np.zeros((64, 18 * 128), np.float32)
    for r in range(6):
        for q in range(2):
            for kj in range(3):
                wo_t[32 * q:32 * q + 32, (r * 3 + kj) * 128:(r * 3 + kj) * 128 + 128] = \
                    out_w[32 * (r + 6 * q):32 * (r + 6 * q) + 32, 128 * kj:128 * kj + 128]

    w1_t = np.zeros((128, 3 * MLP), np.float32)
    for k in range(3):
        w1_t[:, MLP * k:MLP * k + MLP] = w1[128 * k:128 * k + 128, :]
    w2_t = np.zeros((128, 12 * C), np.float32)
    for k2 in range(12):
        w2_t[:, C * k2:C * k2 + C] = w2[128 * k2:128 * k2 + 128, :]

    # ybias[n2, :] = P(bv)[n2] @ out_w + out_b
    n2 = np.arange(N)[:, None]
    co = np.arange(12)[None, :]
    hh = (12 * n2 + co) // N                    # (49, 12)
    pb = bv.reshape(12, 32)[hh]                 # (49, 12, 32)
    ybias = pb.reshape(N, C) @ out_w + out_b    # (49, 384)
    yb_t = np.zeros((128, 3 * TW), np.float32)
    tr_ = np.arange(WS)[:, None, None]
    wn_ = np.arange(WT)[None, :, None]
    tc_ = np.arange(WS)[None, None, :]
    pc = (WS * tr_ + tc_ + 0 * wn_).reshape(TW)   # n2 per permuted column
    for kj in range(3):
        blk = ybias[:, 128 * kj:128 * kj + 128].T          # (128, 49)
        yb_t[:, TW * kj:TW * kj + TW] = blk[:, pc]

    b1p = b1 - be2 @ w1                          # mlp bias corrected for +be2 on x2
    be12 = be1 + be2
    cn_t = np.zeros((128, 27), np.float32)
    for m in range(12):
        cn_t[:, m] = b1p[128 * m:128 * m + 128]
    for k in range(3):
        cn_t[:, 12 + k] = b2[128 * k:128 * k + 128]
        cn_t[:, 15 + k] = g1[128 * k:128 * k + 128]
        cn_t[:, 18 + k] = g2[128 * k:128 * k + 128]
        cn_t[:, 21 + k] = be12[128 * k:128 * k + 128]

    ones_t = np.full((128, 128), 1.0 / C, np.float32)
    ident = np.eye(128, dtype=np.float32)

    # column-roll permutation for the input transposes
    perm = np.zeros((WW, WW), np.float32)
    jj = np.arange(WW)
    perm[(jj + DISP) % WW, jj] = 1.0

    return {
        "wv_t": _to_bf(wv_t), "wo_t": _to_bf(wo_t),
        "w1_t": _to_bf(w1_t), "w2_t": _to_bf(w2_t),
        "yb_t": np.ascontiguousarray(yb_t), "cn_t": np.ascontiguousarray(cn_t),
        "ones_t": _to_bf(ones_t), "identb_t": _to_bf(ident),
        "perm_t": _to_bf(perm),
    }


_CACHE = {}


def kernel(**inputs):
    if "nc" not in _CACHE:
        _CACHE["nc"] = build()
    nc = _CACHE["nc"]
    x = np.asarray(inputs["x"], dtype=np.float32).astype(BF)
    base = prep_inputs(inputs)
    in_maps = []
    for c in range(NCORES):
        m = dict(base)
        m["x"] = np.ascontiguousarray(
            x[IMGS * c:IMGS * (c + 1)].reshape(IMGS * NTOK, C))
        in_maps.append(m)
    import os
    trace = bool(int(os.environ.get("KERNEL_TRACE", "0")))
    res = run_bass_kernel_spmd(nc, in_maps, core_ids=list(range(NCORES)),
                               trace=trace)
    _CACHE["last_res"] = res
    out = np.concatenate([r["out"].reshape(IMGS, HH, WW, C) for r in res.results],
                         axis=0)
    return out



# revision 41
# speedup vs baseline: 1.2267x; 1.0064x over previous
"""Trainium2 Bass kernel v2 for nn_Encoder_59219009077683 (Swin-style block).

Math shortcut (from baseline): softmax row-sums are 1, so attention output = v
and the whole block is: window-gather -> V-proj -> fixed permutation (L_d
layout) -> out-proj -> LN1+skip -> MLP -> LN2+skip -> window-scatter.

v2 structural changes vs baseline:
  - gather/scatter via ONE SWDGE indirect DMA per tile (host-built index
    table; the cyclic roll/wrap is baked into the indices)
  - bf16 for all SBUF intermediates and matmuls (tolerance is 2e-2)
  - K=64 out-projection: L_d stored twice (quadrant q holds T[d, c+6q]) so
    the 12 K=32 chunk-matmuls become 6 K=64 matmuls per output block
  - host-folded biases: v-bias+out-bias -> ybias table added at y-evac;
    norm1_b+norm2_b added at x-transpose evac; mlp_b1 corrected by
    -W1^T norm2_b so the norm2 beta rides the x2 skip path
  - rsqrt via tensor_scalar(pow -0.5) on DVE: scalar engine runs only
    Gelu/Identity -> a single activation-table load
  - elementwise spread across DVE / Act / GpSimd
"""
import numpy as np
from contextlib import ExitStack

import concourse.bass as bass
import concourse.bacc as bacc
import concourse.tile as tile
from concourse import mybir
from concourse.bass_utils import run_bass_kernel_spmd, ml_dtypes

F32 = mybir.dt.float32
BF16 = mybir.dt.bfloat16
U32 = mybir.dt.uint32
AF = mybir.ActivationFunctionType
OP = mybir.AluOpType

B, HH, WW, C = 32, 56, 56, 384
NH, HD, WS, DISP, MLP = 12, 32, 7, 3, 1536
NWS = 8
N = 49
J = NH * N            # 588
NCORES = 8
IMGS = B // NCORES
WT = 8                # windows per tile (= one window row)
TW = WT * N           # 392
NTOK = HH * WW        # tokens per image
EPS = 1e-5
LDPAD = 16
LDW = LDPAD + J * WT + 8

BF = ml_dtypes.bfloat16


def _ap(t, offset, dims):
    tt = t.tensor if hasattr(t, "tensor") else t
    return bass.AP(tensor=tt, offset=offset, ap=[list(d) for d in dims])


def build():
    nc = bacc.Bacc("TRN2", target_bir_lowering=False, debug=False, num_devices=NCORES)
    x_d = nc.dram_tensor("x", [IMGS * NTOK, C], BF16, kind="ExternalInput")
    wv_d = nc.dram_tensor("wv_t", [128, 3 * C], BF16, kind="ExternalInput")
    wo_d = nc.dram_tensor("wo_t", [64, 18 * 128], BF16, kind="ExternalInput")
    w1_d = nc.dram_tensor("w1_t", [128, 3 * MLP], BF16, kind="ExternalInput")
    w2_d = nc.dram_tensor("w2_t", [128, 12 * C], BF16, kind="ExternalInput")
    yb_d = nc.dram_tensor("yb_t", [128, 3 * TW], F32, kind="ExternalInput")
    cn_d = nc.dram_tensor("cn_t", [128, 27], F32, kind="ExternalInput")
    on_d = nc.dram_tensor("ones_t", [128, 128], BF16, kind="ExternalInput")
    idb_d = nc.dram_tensor("identb_t", [128, 128], BF16, kind="ExternalInput")
    pm_d = nc.dram_tensor("perm_t", [WW, WW], BF16, kind="ExternalInput")
    out_d = nc.dram_tensor("out", [IMGS * NTOK, C], F32, kind="ExternalOutput")

    with tile.TileContext(nc) as tc, ExitStack() as ctx:
        wpool = ctx.enter_context(tc.tile_pool(name="w", bufs=1))
        stage_pool = ctx.enter_context(tc.tile_pool(name="stage", bufs=4))
        xt_pool = ctx.enter_context(tc.tile_pool(name="xt", bufs=4))
        vt_pool = ctx.enter_context(tc.tile_pool(name="vt", bufs=3))
        ld_pool = ctx.enter_context(tc.tile_pool(name="ld", bufs=2))
        y_pool = ctx.enter_context(tc.tile_pool(name="y", bufs=3))
        x2_pool = ctx.enter_context(tc.tile_pool(name="x2", bufs=4))
        h_pool = ctx.enter_context(tc.tile_pool(name="h", bufs=3))
        h2_pool = ctx.enter_context(tc.tile_pool(name="h2", bufs=3))
        oc_pool = ctx.enter_context(tc.tile_pool(name="oc", bufs=3))
        ot_pool = ctx.enter_context(tc.tile_pool(name="ot", bufs=3))
        sm_pool = ctx.enter_context(tc.tile_pool(name="sm", bufs=5))
        ps_t = ctx.enter_context(tc.tile_pool(name="pst", bufs=1, space="PSUM"))
        ps_a = ctx.enter_context(tc.tile_pool(name="psa", bufs=2, space="PSUM"))
        ps_m = ctx.enter_context(tc.tile_pool(name="psm", bufs=2, space="PSUM"))
        ps_s = ctx.enter_context(tc.tile_pool(name="pss", bufs=1, space="PSUM"))
        ps_o = ctx.enter_context(tc.tile_pool(name="pso", bufs=2, space="PSUM"))

        # ---------- one-time setup: weights straight from host layout ----------
        wv_b = wpool.tile([128, 3 * C], BF16)
        wo_b = wpool.tile([64, 18 * 128], BF16)
        w1_b = wpool.tile([128, 3 * MLP], BF16)
        w2_b = wpool.tile([128, 12 * C], BF16)
        yb_sb = wpool.tile([128, 3 * TW], F32)
        cn_sb = wpool.tile([128, 27], F32)
        ones_b = wpool.tile([128, 128], BF16)
        identb = wpool.tile([128, 128], BF16)
        perm56 = wpool.tile([WW, WW], BF16)
        eps_t = wpool.tile([128, 1], F32)
        zero_t = wpool.tile([128, 1], F32)
        nc.vector.memset(eps_t[:, :], EPS)
        nc.vector.memset(zero_t[:, :], 0.0)
        nc.sync.dma_start(perm56[:, :], pm_d[:, :])
        nc.sync.dma_start(wv_b[:, :], wv_d[:, :])
        nc.sync.dma_start(wo_b[:, :], wo_d[:, :])
        nc.sync.dma_start(w1_b[:, :], w1_d[:, :])
        nc.sync.dma_start(w2_b[:, :], w2_d[:, :])
        nc.sync.dma_start(yb_sb[:, :], yb_d[:, :])
        nc.sync.dma_start(cn_sb[:, :], cn_d[:, :])
        nc.sync.dma_start(ones_b[:, :], on_d[:, :])
        nc.sync.dma_start(identb[:, :], idb_d[:, :])
        # PE must observe the identity via a transpose once before the loop so
        # later transposes carry <=1 sync wait (bare LDWEIGHTS limitation).
        dmy = ps_t.tile([128, 392], BF16, tag="t")
        nc.tensor.transpose(dmy[0:WW, 0:WW], perm56[0:WW, 0:WW], perm56[0:WW, 0:WW])
        dmy2 = ps_t.tile([128, 392], BF16, tag="t")
        nc.tensor.transpose(dmy2[0:128, 0:128], identb[:, :], identb[:, :])

        # ---------- software-pipelined main loop ----------
        # tile index t = img*8 + wr; phases skewed across iterations so every
        # engine (PE especially) always has ready work.
        NT = IMGS * NWS
        st = {}   # per-tile live state

        def do_gather(t):
            img, wr = divmod(t, NWS)
            stage = stage_pool.tile([WW, WS * C], BF16, tag="stage")
            pst = stage[:, :].ap[0][0]
            soff = stage[:, :].offset
            rows = [(WS * wr + DISP, 0, WS)] if wr < NWS - 1 else \
                [(52, 0, 4), (0, 4, 3)]
            for (r0, tr0, ntr) in rows:
                src = _ap(x_d, (img * NTOK + r0 * WW) * C,
                          [[C, WW], [WW * C, ntr], [1, C]])
                dst = _ap(stage, soff + tr0 * C,
                          [[pst, WW], [C, ntr], [1, C]])
                nc.sync.dma_start(dst, src)
            st[t] = {"stage": stage}

        def do_p1(t):
            """transposes + V-proj + ld build"""
            d = st[t]
            stage = d.pop("stage")
            xt = xt_pool.tile([128, 3 * TW], BF16, tag="xt")
            for k in range(3):
                xps = ps_t.tile([128, 392], BF16, tag="t")
                pxp = xps[:, :].ap[0][0]
                xpo = xps[:, :].offset
                for tr in range(WS):
                    nc.tensor.transpose(
                        xps[:, WW * tr:WW * tr + WW],
                        stage[0:WW, tr * C + 128 * k:tr * C + 128 * k + 128],
                        perm56[0:WW, 0:WW])
                # token order everywhere downstream: col = 56*tr + 7*win + tc
                nc.vector.tensor_scalar(xt[:, TW * k:TW * k + TW], xps[:, :],
                                        cn_sb[:, 21 + k:22 + k], None, op0=OP.add)
            vt = vt_pool.tile([128, 3 * TW], BF16, tag="vt")
            pvt = vt[:, :].ap[0][0]
            vtoff = vt[:, :].offset
            for kv in range(3):
                vps = ps_a.tile([128, TW], F32, tag="a")
                for k in range(3):
                    nc.tensor.matmul(vps[:, :],
                                     wv_b[:, C * k + 128 * kv:C * k + 128 * kv + 128],
                                     xt[:, TW * k:TW * k + TW],
                                     start=(k == 0), stop=(k == 2))
                pps = vps[:, :].ap[0][0]
                nc.scalar.activation(
                    _ap(vt, vtoff + N * kv,
                        [[pvt, 128], [WS, WS], [3 * N, WT], [1, WS]]),
                    _ap(vps, vps[:, :].offset,
                        [[pps, 128], [WW, WS], [WS, WT], [1, WS]]),
                    AF.Identity, bias=zero_t[:, :], scale=1.0)
            ld = ld_pool.tile([64, LDW], BF16, tag="ld")
            pld = ld[:, :].ap[0][0]
            ldoff = ld[:, :].offset
            for s4 in range(4):
                for q in range(2):
                    src = _ap(vt, vtoff + (32 * s4) * pvt,
                              [[pvt, 32], [N, 24], [1, N]])
                    dst = _ap(ld, ldoff + (32 * q) * pld + LDPAD + N * s4 - 6 * q,
                              [[pld, 32], [4 * N, 24], [1, N]])
                    nc.sync.dma_start(dst, src)
            d["xt"] = xt
            d["ld"] = (ld, pld, ldoff)

        def do_p2a(t):
            """out-proj + ybias evac + ysq"""
            d = st[t]
            ld, pld, ldoff = d.pop("ld")
            y = y_pool.tile([128, 3 * TW], BF16, tag="y")
            for kj in range(3):
                yps = ps_o.tile([128, TW], F32, tag="o")
                for r in range(6):
                    rhs = _ap(ld, ldoff + LDPAD + r,
                              [[pld, 64], [84, WS], [J, WT], [12, WS]])
                    nc.tensor.matmul(yps[:, :],
                                     wo_b[:, (r * 3 + kj) * 128:(r * 3 + kj) * 128 + 128],
                                     rhs, start=(r == 0), stop=(r == 5))
                nc.vector.tensor_add(y[:, TW * kj:TW * kj + TW], yps[:, :],
                                     yb_sb[:, TW * kj:TW * kj + TW])
            ysq = sm_pool.tile([128, 3 * TW], BF16, tag="ysq")
            for k in range(3):
                nc.vector.tensor_mul(ysq[:, TW * k:TW * k + TW],
                                     y[:, TW * k:TW * k + TW], y[:, TW * k:TW * k + TW])
            d["y"] = y
            d["ysq"] = ysq

        def _ln(y, ysq, gcol, skip, out):
            """matmul sums -> stats -> apply; out = (y-mu)*rst*g + skip."""
            s1 = ps_s.tile([128, TW], F32, tag="s")
            s2 = ps_s.tile([128, TW], F32, tag="s")
            for k in range(3):
                nc.tensor.matmul(s1[:, :], ones_b[:, :], y[:, TW * k:TW * k + TW],
                                 start=(k == 0), stop=(k == 2))
                nc.tensor.matmul(s2[:, :], ones_b[:, :], ysq[:, TW * k:TW * k + TW],
                                 start=(k == 0), stop=(k == 2))
            mu = sm_pool.tile([128, TW], BF16, tag="mu")
            m2 = sm_pool.tile([128, TW], BF16, tag="m2")
            veps = sm_pool.tile([128, TW], F32, tag="veps")
            rst = sm_pool.tile([128, TW], F32, tag="rst")
            nc.vector.tensor_copy(mu[:, :], s1[:, :])
            nc.vector.tensor_mul(m2[:, :], mu[:, :], mu[:, :])
            nc.vector.tensor_sub(veps[:, :], s2[:, :], m2[:, :])
            # std = sqrt(var + eps) on Act (eps folded into the bias), then
            # 1/std via the fast custom-DVE reciprocal (~51 ULP, plenty here)
            nc.scalar.activation(veps[:, :], veps[:, :], AF.Sqrt,
                                 bias=eps_t[:, :], scale=1.0)
            nc.vector.reciprocal_approx_fast(rst[:, :], veps[:, :])
            rstb = sm_pool.tile([128, TW], BF16, tag="rstb")
            nc.vector.tensor_copy(rstb[:, :], rst[:, :])
            for k in range(3):
                u = sm_pool.tile([128, TW], BF16, tag="u")
                v = sm_pool.tile([128, TW], BF16, tag="v")
                nc.vector.tensor_sub(u[:, :], y[:, TW * k:TW * k + TW], mu[:, :])
                nc.vector.tensor_mul(v[:, :], u[:, :], rstb[:, :])
                nc.vector.scalar_tensor_tensor(out[:, TW * k:TW * k + TW],
                                               v[:, :],
                                               cn_sb[:, gcol + k:gcol + k + 1],
                                               skip[:, TW * k:TW * k + TW],
                                               op0=OP.mult, op1=OP.add)

        def do_p2b(t):
            d = st[t]
            x2 = x2_pool.tile([128, 3 * TW], BF16, tag="x2")
            _ln(d.pop("y"), d.pop("ysq"), 15, d.pop("xt"), x2)
            d["x2"] = x2

        def do_p3(t):
            d = st[t]
            x2 = d["x2"]
            hsb = h_pool.tile([128, 12 * TW], BF16, tag="h")
            for m in range(12):
                hps = ps_m.tile([128, TW], F32, tag="m")
                for k in range(3):
                    nc.tensor.matmul(hps[:, :],
                                     w1_b[:, MLP * k + 128 * m:MLP * k + 128 * m + 128],
                                     x2[:, TW * k:TW * k + TW],
                                     start=(k == 0), stop=(k == 2))
                nc.scalar.activation(hsb[:, TW * m:TW * m + TW], hps[:, :],
                                     AF.Gelu, bias=cn_sb[:, m:m + 1], scale=1.0)
            d["hsb"] = hsb

        def do_p4a(t):
            d = st[t]
            hsb = d.pop("hsb")
            h2 = h2_pool.tile([128, 3 * TW], BF16, tag="h2")
            for kj in range(3):
                h2ps = ps_a.tile([128, TW], F32, tag="a")
                for k2 in range(12):
                    nc.tensor.matmul(h2ps[:, :],
                                     w2_b[:, C * k2 + 128 * kj:C * k2 + 128 * kj + 128],
                                     hsb[:, TW * k2:TW * k2 + TW],
                                     start=(k2 == 0), stop=(k2 == 11))
                nc.vector.tensor_scalar(h2[:, TW * kj:TW * kj + TW], h2ps[:, :],
                                        cn_sb[:, 12 + kj:13 + kj], None, op0=OP.add)
            hsq = sm_pool.tile([128, 3 * TW], BF16, tag="ysq")
            for k in range(3):
                nc.vector.tensor_mul(hsq[:, TW * k:TW * k + TW],
                                     h2[:, TW * k:TW * k + TW], h2[:, TW * k:TW * k + TW])
            d["h2"] = h2
            d["hsq"] = hsq

        def do_p4b(t):
            d = st[t]
            ocm = oc_pool.tile([128, 3 * TW], BF16, tag="oc")
            _ln(d.pop("h2"), d.pop("hsq"), 18, d.pop("x2"), ocm)
            d["ocm"] = ocm

        def do_p5(t):
            d = st.pop(t)
            img, wr = divmod(t, NWS)
            ocm = d["ocm"]
            poc = ocm[:, :].ap[0][0]
            oco = ocm[:, :].offset
            # paired: otr[56*(tr%2)+x, 384*(tr//2)+c]; 12 transposes not 21
            otr = ot_pool.tile([112, 4 * C], BF16, tag="ot")
            pot = otr[:, :].ap[0][0]
            poff = otr[:, :].offset
            for p in range(4):
                prow = 112 if p < 3 else 56
                ops_t = ps_t.tile([128, 392], BF16, tag="t")
                for k in range(3):
                    nc.tensor.transpose(
                        ops_t[0:prow, 128 * k:128 * k + 128],
                        ocm[:, TW * k + 112 * p:TW * k + 112 * p + prow],
                        identb[:, :])
                nc.vector.tensor_copy(otr[0:prow, p * C:p * C + C],
                                      ops_t[0:prow, 0:C])

            def scat(prow0, npair, r0, odd):
                base = poff + (56 * odd) * pot + 384 * prow0
                dst0 = (img * NTOK + r0 * WW) * C
                src = _ap(otr, base, [[pot, 53], [C, npair], [1, C]])
                dst = _ap(out_d, dst0 + DISP * C,
                          [[C, 53], [2 * WW * C, npair], [1, C]])
                nc.gpsimd.dma_start(dst, src)
                src = _ap(otr, base + 53 * pot, [[pot, 3], [C, npair], [1, C]])
                dst = _ap(out_d, dst0, [[C, 3], [2 * WW * C, npair], [1, C]])
                nc.gpsimd.dma_start(dst, src)

            if wr < NWS - 1:
                r0 = WS * wr + DISP
                scat(0, 4, r0, 0)
                scat(0, 3, r0 + 1, 1)
            else:
                scat(0, 2, 52, 0)
                scat(0, 2, 53, 1)
                scat(2, 2, 0, 0)
                scat(2, 1, 1, 1)

        for i in range(NT + 7):
            if i < NT:
                do_gather(i)
            if 0 <= i - 2 < NT:
                do_p2a(i - 2)
            if 0 <= i - 1 < NT:
                do_p1(i - 1)
            if 0 <= i - 5 < NT:
                do_p4a(i - 5)
            if 0 <= i - 7 < NT:
                do_p5(i - 7)
            if 0 <= i - 3 < NT:
                do_p2b(i - 3)
            if 0 <= i - 4 < NT:
                do_p3(i - 4)
            if 0 <= i - 6 < NT:
                do_p4b(i - 6)
    nc.compile()
    return nc


def _to_bf(a):
    return np.ascontiguousarray(a.astype(BF))


def prep_inputs(inputs):
    """Host-side weight/bias reformatting (layout only + tiny bias algebra)."""
    f = {k: np.asarray(v, dtype=np.float32) for k, v in inputs.items()}
    qkv_w, qkv_b = f["qkv_w"], f["qkv_b"]
    out_w, out_b = f["out_w"], f["out_b"]
    w1, b1, w2, b2 = f["mlp_w1"], f["mlp_b1"], f["mlp_w2"], f["mlp_b2"]
    g1, be1 = f["norm1_g"], f["norm1_b"]
    g2, be2 = f["norm2_g"], f["norm2_b"]

    wv = qkv_w[:, 2 * C:3 * C]
    bv = qkv_b[2 * C:3 * C]

    wv_t = np.zeros((128, 3 * C), np.float32)
    for k in range(3):
        wv_t[:, C * k:C * k + C] = wv[128 * k:128 * k + 128, :]

    wo_t = np.zeros((64, 18 * 128), np.float32)
    for r in range(6):
        for q in range(2):
            for kj in range(3):
                wo_t[32 * q:32 * q + 32, (r * 3 + kj) * 128:(r * 3 + kj) * 128 + 128] = \
                    out_w[32 * (r + 6 * q):32 * (r + 6 * q) + 32, 128 * kj:128 * kj + 128]

    w1_t = np.zeros((128, 3 * MLP), np.float32)
    for k in range(3):
        w1_t[:, MLP * k:MLP * k + MLP] = w1[128 * k:128 * k + 128, :]
    w2_t = np.zeros((128, 12 * C), np.float32)
    for k2 in range(12):
        w2_t[:, C * k2:C * k2 + C] = w2[128 * k2:128 * k2 + 128, :]

    # ybias[n2, :] = P(bv)[n2] @ out_w + out_b
    n2 = np.arange(N)[:, None]
    co = np.arange(12)[None, :]
    hh = (12 * n2 + co) // N                    # (49, 12)
    pb = bv.reshape(12, 32)[hh]                 # (49, 12, 32)
    ybias = pb.reshape(N, C) @ out_w + out_b    # (49, 384)
    yb_t = np.zeros((128, 3 * TW), np.float32)
    tr_ = np.arange(WS)[:, None, None]
    wn_ = np.arange(WT)[None, :, None]
    tc_ = np.arange(WS)[None, None, :]
    pc = (WS * tr_ + tc_ + 0 * wn_).reshape(TW)   # n2 per permuted column
    for kj in range(3):
        blk = ybias[:, 128 * kj:128 * kj + 128].T          # (128, 49)
        yb_t[:, TW * kj:TW * kj + TW] = blk[:, pc]

    b1p = b1 - be2 @ w1                          # mlp bias corrected for +be2 on x2
    be12 = be1 + be2
    cn_t = np.zeros((128, 27), np.float32)
    for m in range(12):
        cn_t[:, m] = b1p[128 * m:128 * m + 128]
    for k in range(3):
        cn_t[:, 12 + k] = b2[128 * k:128 * k + 128]
        cn_t[:, 15 + k] = g1[128 * k:128 * k + 128]
        cn_t[:, 18 + k] = g2[128 * k:128 * k + 128]
        cn_t[:, 21 + k] = be12[128 * k:128 * k + 128]

    ones_t = np.full((128, 128), 1.0 / C, np.float32)
    ident = np.eye(128, dtype=np.float32)

    # column-roll permutation for the input transposes
    perm = np.zeros((WW, WW), np.float32)
    jj = np.arange(WW)
    perm[(jj + DISP) % WW, jj] = 1.0

    return {
        "wv_t": _to_bf(wv_t), "wo_t": _to_bf(wo_t),
        "w1_t": _to_bf(w1_t), "w2_t": _to_bf(w2_t),
        "yb_t": np.ascontiguousarray(yb_t), "cn_t": np.ascontiguousarray(cn_t),
        "ones_t": _to_bf(ones_t), "identb_t": _to_bf(ident),
        "perm_t": _to_bf(perm),
    }


_CACHE = {}


def kernel(**inputs):
    if "nc" not in _CACHE:
        _CACHE["nc"] = build()
    nc = _CACHE["nc"]
    x = np.asarray(inputs["x"], dtype=np.float32).astype(BF)
    base = prep_inputs(inputs)
    in_maps = []
    for c in range(NCORES):
        m = dict(base)
        m["x"] = np.ascontiguousarray(
            x[IMGS * c:IMGS * (c + 1)].reshape(IMGS * NTOK, C))
        in_maps.append(m)
    import os
    trace = bool(int(os.environ.get("KERNEL_TRACE", "0")))
    res = run_bass_kernel_spmd(nc, in_maps, core_ids=list(range(NCORES)),
                               trace=trace)
    _CACHE["last_res"] = res
    out = np.concatenate([r["out"].reshape(IMGS, HH, WW, C) for r in res.results],
                         axis=0)
    return out

